# revision 1
# baseline (speedup 1.0000x reference)
"""Bass/Tile kernel for nn_DeepRelativeST on one NeuronCore (1/8 data-parallel
shard over the batch axis).

Per-core: R=2048 rows (8 batches x 256 pos), D=512, DFF=2048, H=8, dep=64,
Ll=32 local l values, 256 (l,h) softmax pairs split into two l-parity tiles:
tile p holds pair (h, l=2q+p) at partition h*16+q.

Key math (derived from reference.py):
  qs[l,h,j] = (x @ wq_headsum)[l*64+j, h]     (full Q GEMM never needed)
  ks likewise; V = x @ wv (full GEMM).
  abar[l,h,k,m] = rel[l,h,k,m-k+63] * (m<=k)  (host-gathered skew)
  r1 = sum_m abar*ks ; t = sum_m abar*m (HOST precomputed from rel)
  r2 = r1 + NEG*t ; cu = sc^2 * R1 * qs
  logits[j,k] = cu[j]*r2[k] (+ causal NEG mask)
  p = softmax_k ; o = p @ V-block
  out row = l*64 + h*8 + j//8, col = (j%8)*64 + n   (torch raw-reshape scramble)
"""
import numpy as np
from contextlib import ExitStack

import concourse.bass as bass
import concourse.tile as tile
from concourse import bacc
from concourse import mybir

F32 = mybir.dt.float32
AX = mybir.AxisListType
OP = mybir.AluOpType
ACTF = mybir.ActivationFunctionType

R, D, DFF, NH, DEP, LL = 2048, 512, 2048, 8, 64, 32
NEG, EPS, SC2 = -1e9, 1e-5, 1.0 / 64.0
RT, DT, FT = R // 128, D // 128, DFF // 128


def host_inputs(inp, core):
    f = lambda k: np.ascontiguousarray(np.asarray(inp[k], np.float32))
    bs = slice(core * 8, core * 8 + 8)
    ls = slice(core * 32, core * 32 + 32)
    Xe = f('X_en')[bs].reshape(R, 64)
    Xd = f('X_de')[bs].reshape(R, 64)

    def wqk_heads(wq, wk):
        a = wq.reshape(D, NH, DEP).sum(-1)
        b = wk.reshape(D, NH, DEP).sum(-1)
        return np.ascontiguousarray(np.concatenate([a, b], 1))  # [512,16]

    km = np.arange(64)
    kk, mm = np.meshgrid(km, km, indexing='ij')   # [k, m]

    def rel_arrange(rel):
        r = rel[ls]                                # [32,8,64,64] = [l,h,k,c]
        # abar[l,h,k,m] = r[l,h,k,m-k+63] if m<=k else 0
        c = mm - kk + 63
        valid = (mm <= kk)
        cs = np.clip(c, 0, 63)
        ab = np.take_along_axis(
            r.reshape(LL, NH, 64, 64), cs.reshape(1, 1, 64, 64), axis=3)
        ab = ab * valid.reshape(1, 1, 64, 64)
        t = (ab * mm.reshape(1, 1, 64, 64)).sum(-1)     # [l,h,k]
        abT = ab.transpose(1, 0, 2, 3)                  # [h,l,k,m]
        tT = t.transpose(1, 0, 2)                       # [h,l,k]
        A = np.empty((2, 128, 4096), np.float32)
        Tt = np.empty((2, 128, 64), np.float32)
        for p in range(2):
            A[p] = abT[:, p::2].reshape(128, 4096)
            Tt[p] = tT[:, p::2].reshape(128, 64)
        return A, Tt

    A_e, t_e = rel_arrange(f('enc_rel'))
    A_d1, t_d1 = rel_arrange(f('dec_rel1'))
    A_d2, t_d2 = rel_arrange(f('dec_rel2'))
    caus = np.triu(np.full((64, 64), NEG, np.float32), 1)  # [j,k]: k>j
    CAUS = np.broadcast_to(caus.reshape(1, 4096), (128, 4096)).copy()

    return {
        'XeT': np.ascontiguousarray(Xe.T), 'XdT': np.ascontiguousarray(Xd.T),
        'W_in': f('W_in'), 'B_in': f('B_in').reshape(1, D),
        'enc_wv': f('enc_wv'), 'dec_wv1': f('dec_wv1'), 'dec_wv2': f('dec_wv2'),
        'enc_wqk': wqk_heads(f('enc_wq'), f('enc_wk')),
        'dec_wqk1': wqk_heads(f('dec_wq1'), f('dec_wk1')),
        'dec_wqk2': wqk_heads(f('dec_wq2'), f('dec_wk2')),
        'enc_A': A_e, 'enc_t': t_e,
        'dec1_A': A_d1, 'dec1_t': t_d1,
        'dec2_A': A_d2, 'dec2_t': t_d2,
        'enc_w1': f('enc_w1'), 'enc_b1': f('enc_b1').reshape(1, DFF),
        'enc_w2': f('enc_w2'), 'enc_b2': f('enc_b2').reshape(1, D),
        'dec_w1': f('dec_w1'), 'dec_b1': f('dec_b1').reshape(1, DFF),
        'dec_w2': f('dec_w2'), 'dec_b2': f('dec_b2').reshape(1, D),
        'W_out': f('W_out'), 'B_out': f('B_out').reshape(1, 64),
        'CAUS': CAUS, 'I128': np.eye(128, dtype=np.float32),
    }


IN_SHAPES = {
    'XeT': (64, R), 'XdT': (64, R), 'W_in': (64, D), 'B_in': (1, D),
    'enc_wv': (D, D), 'dec_wv1': (D, D), 'dec_wv2': (D, D),
    'enc_wqk': (D, 16), 'dec_wqk1': (D, 16), 'dec_wqk2': (D, 16),
    'enc_A': (2, 128, 4096), 'dec1_A': (2, 128, 4096), 'dec2_A': (2, 128, 4096),
    'enc_t': (2, 128, 64), 'dec1_t': (2, 128, 64), 'dec2_t': (2, 128, 64),
    'enc_w1': (D, DFF), 'enc_b1': (1, DFF), 'enc_w2': (DFF, D), 'enc_b2': (1, D),
    'dec_w1': (D, DFF), 'dec_b1': (1, DFF), 'dec_w2': (DFF, D), 'dec_b2': (1, D),
    'W_out': (D, 64), 'B_out': (1, 64),
    'CAUS': (128, 4096), 'I128': (128, 128),
}


def declare_io(nc):
    hi = {k: nc.dram_tensor(k, list(s), F32, kind="ExternalInput").ap()
          for k, s in IN_SHAPES.items()}
    out = nc.dram_tensor('out', [R, 64], F32, kind="ExternalOutput").ap()
    return hi, out


def build(ctx: ExitStack, tc: tile.TileContext, hi, out_ap, dbg=None):
    nc = tc.nc
    consts = ctx.enter_context(tc.tile_pool(name="consts", bufs=1))
    wpool = ctx.enter_context(tc.tile_pool(name="wpool", bufs=1))
    work = ctx.enter_context(tc.tile_pool(name="work", bufs=3))
    preQ = ctx.enter_context(tc.tile_pool(name="preQ", bufs=8))
    small = ctx.enter_context(tc.tile_pool(name="small", bufs=1))
    bigP = ctx.enter_context(tc.tile_pool(name="bigP", bufs=1))
    psA = ctx.enter_context(tc.tile_pool(name="psA", bufs=3, space="PSUM"))
    psB = ctx.enter_context(tc.tile_pool(name="psB", bufs=4, space="PSUM"))
    dram = ctx.enter_context(tc.tile_pool(name="dram", bufs=1, space="DRAM"))

    I128 = consts.tile([128, 128], F32, tag="I128", name="I128")
    nc.sync.dma_start(I128[:], hi['I128'][:])
    ones1 = consts.tile([1, D], F32, tag="ones1", name="ones1")
    nc.vector.memset(ones1[:], 1.0)
    epsc = consts.tile([128, 1], F32, tag="epsc", name="epsc")
    nc.vector.memset(epsc[:], EPS)
    W_in = consts.tile([64, D], F32, tag="W_in", name="W_in")
    nc.sync.dma_start(W_in[:], hi['W_in'][:])
    B_in = consts.tile([1, D], F32, tag="B_in", name="B_in")
    nc.sync.dma_start(B_in[:], hi['B_in'][:])

    # DRAM scratch: transposed activations live here, streamed at use.
    xTd = {nm: dram.tile([DT, 128, R], F32, tag=f"xTd_{nm}", name=f"xTd_{nm}")
           for nm in ('xe', 'xd', 'm', 'o1', 'eo', 'c', 'of')}
    aD = dram.tile([R, D], F32, tag="aD", name="aD")
    vD = dram.tile([R, D], F32, tag="vD", name="vD")
    mnD = dram.tile([R, D], F32, tag="mnD", name="mnD")

    def copy_ps(dst, src):
        nc.scalar.copy(dst, src)

    # ---------- embed: x.T = (X@W_in+B).T streamed to DRAM ------------------
    def embed_T_toD(x_in_ap, dst):
        for ct in range(DT):
            for rc in range(4):
                xin = work.tile([64, 512], F32, tag="xin", name="xin")
                nc.sync.dma_start(xin[:], x_in_ap[:, rc * 512:(rc + 1) * 512])
                ps = psA.tile([128, 512], F32, tag="psa", name="psa")
                nc.tensor.matmul(ps[:], lhsT=W_in[:, ct * 128:(ct + 1) * 128],
                                 rhs=xin[:], start=True, stop=False)
                nc.tensor.matmul(ps[:], lhsT=B_in[:, ct * 128:(ct + 1) * 128],
                                 rhs=ones1[:, 0:512], start=False, stop=True)
                t = work.tile([128, 512], F32, tag="toD", name="toD", bufs=2)
                copy_ps(t[:], ps[:])
                nc.sync.dma_start(dst[ct, :, rc * 512:(rc + 1) * 512], t[:])

    def embed_nat_ps(x_in_ap, rt):
        xin = work.tile([64, 128], F32, tag="xin2", name="xin2")
        nc.sync.dma_start(xin[:], x_in_ap[:, rt * 128:(rt + 1) * 128])
        ps = psA.tile([128, 512], F32, tag="psa", name="psa")
        nc.tensor.matmul(ps[:], lhsT=xin[:], rhs=W_in[:], start=True, stop=False)
        nc.tensor.matmul(ps[:], lhsT=ones1[:, 0:128], rhs=B_in[:],
                         start=False, stop=True)
        return ps

    # ---------- layernorm over one group of 4 row-tiles ---------------------
    def ln_group4(g, pre_fn, out_cb):
        """pre_fn(rt) -> [128,512] AP (lazy); out_cb(rt, src, nmu, rstd)."""
        if True:
            sx = small.tile([128, 4], F32, tag="sx", name="sx", bufs=2)
            sx2 = small.tile([128, 4], F32, tag="sx2", name="sx2", bufs=2)
            pres = []
            for i in range(4):
                pa = pre_fn(g * 4 + i)
                pres.append(pa)
                scr = work.tile([128, D], F32, tag="lnscr", name="lnscr")
                nc.scalar.activation(scr[:], pa, ACTF.Copy,
                                     accum_out=sx[:, i:i + 1])
                nc.scalar.activation(scr[:], pa, ACTF.Square,
                                     accum_out=sx2[:, i:i + 1])
            negmu = small.tile([128, 4], F32, tag="negmu", name="negmu", bufs=2)
            nc.vector.tensor_scalar(out=negmu[:], in0=sx[:], scalar1=-1.0 / D,
                                    scalar2=None, op0=OP.mult)
            mu2 = small.tile([128, 4], F32, tag="mu2", name="mu2", bufs=2)
            nc.vector.tensor_tensor(out=mu2[:], in0=negmu[:], in1=negmu[:],
                                    op=OP.mult)
            var = small.tile([128, 4], F32, tag="var", name="var", bufs=2)
            nc.vector.scalar_tensor_tensor(out=var[:], in0=sx2[:],
                                           scalar=1.0 / D, in1=mu2[:],
                                           op0=OP.mult, op1=OP.subtract)
            std = small.tile([128, 4], F32, tag="std", name="std", bufs=2)
            nc.scalar.activation(std[:], var[:], ACTF.Sqrt, bias=epsc[:])
            rstd = small.tile([128, 4], F32, tag="rstd", name="rstd", bufs=2)
            nc.vector.reciprocal(rstd[:], std[:])
            for i in range(4):
                out_cb(g * 4 + i, pres[i], negmu[:, i:i + 1], rstd[:, i:i + 1])

    # ---------- attention ---------------------------------------------------
    def attention(xqTd, xkvTd, wv_ap, wqk_ap, A_ap, t_ap, causal):
        # V GEMM (x.T-stationary tiles streamed from DRAM) -> vD
        wv = wpool.tile([128, 4 * D], F32, tag="wv", name="wv")
        for dt in range(DT):
            nc.sync.dma_start(wv[:, dt * D:(dt + 1) * D],
                              wv_ap[dt * 128:(dt + 1) * 128, :])
        for rt in range(RT):
            ps = psA.tile([128, 512], F32, tag="psa", name="psa")
            for dt in range(DT):
                xl = work.tile([128, 128], F32, tag="xlT", name="xlT")
                nc.sync.dma_start(xl[:], xkvTd[dt, :, rt * 128:(rt + 1) * 128])
                nc.tensor.matmul(ps[:], lhsT=xl[:],
                                 rhs=wv[:, dt * D:(dt + 1) * D],
                                 start=(dt == 0), stop=(dt == DT - 1))
            vt = work.tile([128, D], F32, tag="Vtile", name="Vtile")
            copy_ps(vt[:], ps[:])
            nc.sync.dma_start(vD[rt * 128:(rt + 1) * 128, :], vt[:])

        # qs / ks GEMMs (W-stationary, M=8)
        wqk = wpool.tile([128, 4 * 16], F32, tag="wqk", name="wqk")
        for dt in range(DT):
            nc.sync.dma_start(wqk[:, dt * 16:(dt + 1) * 16],
                              wqk_ap[dt * 128:(dt + 1) * 128, :])
        qT = work.tile([8, R], F32, tag="qT", name="qT", bufs=1)
        kT = work.tile([8, R], F32, tag="kT", name="kT", bufs=1)
        for (dst, colofs, srcTd) in ((qT, 0, xqTd), (kT, 8, xkvTd)):
            for rc in range(4):
                ps = psB.tile([8, 512], F32, tag="psbq", name="psbq", bufs=1)
                for dt in range(DT):
                    xc = work.tile([128, 512], F32, tag="xcT", name="xcT")
                    nc.sync.dma_start(xc[:], srcTd[dt, :, rc * 512:(rc + 1) * 512])
                    nc.tensor.matmul(
                        ps[:], lhsT=wqk[:, dt * 16 + colofs: dt * 16 + colofs + 8],
                        rhs=xc[:], start=(dt == 0), stop=(dt == DT - 1))
                copy_ps(dst[:, rc * 512:(rc + 1) * 512], ps[:])

        qs_pp = small.tile([128, 2 * 64], F32, tag="qs_pp", name="qs_pp")
        ks_pp = small.tile([128, 2 * 64], F32, tag="ks_pp", name="ks_pp")
        qD = dram.tile([8, R], F32, tag="qD", name="qD")
        kD = dram.tile([8, R], F32, tag="kD", name="kD")
        for (src, bounce, dst) in ((qT, qD, qs_pp), (kT, kD, ks_pp)):
            nc.sync.dma_start(bounce[:], src[:])
            nc.sync.dma_start(
                dst[:], bounce[:].rearrange("h (q f) -> (h q) f", q=16))

        # r1 = sum_m abar*ks, computed in 4 column chunks of 16 k per parity
        r1 = small.tile([128, 2 * 64], F32, tag="r1", name="r1")
        for p in range(2):
            for kc in range(4):
                A = work.tile([128, 1024], F32, tag="Achunk", name="Achunk", bufs=2)
                nc.scalar.dma_start(A[:], A_ap[p][:, kc * 1024:(kc + 1) * 1024])
                A3 = A[:].rearrange("a (k m) -> a k m", k=16)
                nc.gpsimd.tensor_tensor(
                    out=A3, in0=A3,
                    in1=ks_pp[:, p * 64:(p + 1) * 64][:, None, :]
                        .broadcast_to([128, 16, 64]), op=OP.mult)
                nc.vector.tensor_reduce(
                    out=r1[:, p * 64 + kc * 16: p * 64 + (kc + 1) * 16],
                    in_=A3, axis=AX.X, op=OP.add)
        tH = small.tile([128, 2 * 64], F32, tag="tH", name="tH")
        nc.sync.dma_start(tH[:].rearrange("a (p k) -> a p k", p=2),
                          t_ap[:].rearrange("p a k -> a p k"))
        r2 = small.tile([128, 2 * 64], F32, tag="r2", name="r2")
        nc.vector.scalar_tensor_tensor(out=r2[:], in0=tH[:], scalar=NEG,
                                       in1=r1[:], op0=OP.mult, op1=OP.add)
        R1s = small.tile([128, 2], F32, tag="R1s", name="R1s")
        nc.vector.tensor_reduce(out=R1s[:],
                                in_=r1[:].rearrange("a (p k) -> a p k", p=2),
                                axis=AX.X, op=OP.add)
        nc.vector.tensor_scalar(out=R1s[:], in0=R1s[:], scalar1=SC2,
                                scalar2=None, op0=OP.mult)
        cu = small.tile([128, 2 * 64], F32, tag="cu", name="cu")
        for p in range(2):
            nc.vector.tensor_scalar(out=cu[:, p * 64:(p + 1) * 64],
                                    in0=qs_pp[:, p * 64:(p + 1) * 64],
                                    scalar1=R1s[:, p:p + 1], scalar2=None,
                                    op0=OP.mult)

        # M = rowmax of logits (rank-1 trick; scans for causal)
        M = small.tile([128, 2 * 64], F32, tag="Mm", name="Mm")
        t1 = small.tile([128, 64], F32, tag="Mt1", name="Mt1")
        t2 = small.tile([128, 64], F32, tag="Mt2", name="Mt2")
        if not causal:
            wmax = small.tile([128, 2], F32, tag="wmax", name="wmax")
            wmin = small.tile([128, 2], F32, tag="wmin", name="wmin")
            nc.vector.tensor_reduce(out=wmax[:],
                                    in_=r2[:].rearrange("a (p k) -> a p k", p=2),
                                    axis=AX.X, op=OP.max)
            nc.vector.tensor_reduce(out=wmin[:],
                                    in_=r2[:].rearrange("a (p k) -> a p k", p=2),
                                    axis=AX.X, op=OP.min)
            for p in range(2):
                sl = slice(p * 64, (p + 1) * 64)
                nc.vector.tensor_scalar(out=M[:, sl], in0=cu[:, sl],
                                        scalar1=wmax[:, p:p + 1], scalar2=None,
                                        op0=OP.mult)
                nc.vector.tensor_scalar(out=t1[:], in0=cu[:, sl],
                                        scalar1=wmin[:, p:p + 1], scalar2=None,
                                        op0=OP.mult)
                nc.vector.tensor_tensor(out=M[:, sl], in0=M[:, sl], in1=t1[:],
                                        op=OP.max)
        else:
            pm = small.tile([128, 128], F32, tag="pm", name="pm")
            pn = small.tile([128, 128], F32, tag="pn", name="pn")
            sm = small.tile([128, 128], F32, tag="sm", name="sm")
            sn = small.tile([128, 128], F32, tag="sn", name="sn")
            for p in range(2):
                sl = slice(p * 64, (p + 1) * 64)
                w_ = r2[:, sl]
                wr = r2[:, sl][:, ::-1]
                nc.vector.tensor_tensor_scan(out=pm[:, sl], data0=w_, data1=w_,
                                             initial=-3e38, op0=OP.max, op1=OP.bypass)
                nc.vector.tensor_tensor_scan(out=pn[:, sl], data0=w_, data1=w_,
                                             initial=3e38, op0=OP.min, op1=OP.bypass)
                nc.vector.tensor_tensor_scan(out=sm[:, sl][:, ::-1], data0=wr,
                                             data1=wr, initial=-3e38,
                                             op0=OP.max, op1=OP.bypass)
                nc.vector.tensor_tensor_scan(out=sn[:, sl][:, ::-1], data0=wr,
                                             data1=wr, initial=3e38,
                                             op0=OP.min, op1=OP.bypass)
            for p in range(2):
                sl = slice(p * 64, (p + 1) * 64)
                nc.vector.tensor_tensor(out=M[:, sl], in0=cu[:, sl],
                                        in1=pm[:, sl], op=OP.mult)
                nc.vector.tensor_tensor(out=t1[:], in0=cu[:, sl], in1=pn[:, sl],
                                        op=OP.mult)
                nc.vector.tensor_tensor(out=M[:, sl], in0=M[:, sl], in1=t1[:],
                                        op=OP.max)
                j63 = slice(p * 64, p * 64 + 63)
                cs = cu[:, j63]
                nc.vector.tensor_tensor(out=t1[:, 0:63], in0=cs,
                                        in1=sm[:, p * 64 + 1:(p + 1) * 64],
                                        op=OP.mult)
                nc.vector.tensor_tensor(out=t2[:, 0:63], in0=cs,
                                        in1=sn[:, p * 64 + 1:(p + 1) * 64],
                                        op=OP.mult)
                nc.vector.tensor_tensor(out=t1[:, 0:63], in0=t1[:, 0:63],
                                        in1=t2[:, 0:63], op=OP.max)
                nc.vector.tensor_scalar(out=t1[:, 0:63], in0=t1[:, 0:63],
                                        scalar1=NEG, scalar2=None, op0=OP.add)
                nc.vector.tensor_tensor(out=M[:, j63], in0=M[:, j63],
                                        in1=t1[:, 0:63], op=OP.max)

        # E chunks of 16 j: build/mask/-M/exp/Z/scale -> transpose to PT -> PV
        Zrec = small.tile([128, 2 * 64], F32, tag="Zrec", name="Zrec")
        for p in range(2):
            PT = bigP.tile([64, 64 * 128], F32, tag="PT", name="PT")
            PT4 = PT[:].rearrange("k (j pp) -> k j pp", j=64)
            for jc in range(4):
                jsl = slice(p * 64 + jc * 16, p * 64 + (jc + 1) * 16)
                E = work.tile([128, 1024], F32, tag="Echunk", name="Echunk", bufs=2)
                E3 = E[:].rearrange("a (j k) -> a j k", j=16)
                nc.vector.tensor_tensor(
                    out=E3, in0=cu[:, jsl][:, :, None].broadcast_to([128, 16, 64]),
                    in1=r2[:, p * 64:(p + 1) * 64][:, None, :]
                        .broadcast_to([128, 16, 64]), op=OP.mult)
                if causal:
                    CS = work.tile([128, 1024], F32, tag="CSchunk", name="CSchunk",
                                   bufs=2)
                    nc.scalar.dma_start(CS[:], hi['CAUS'][:, jc * 1024:(jc + 1) * 1024])
                    nc.gpsimd.tensor_tensor(out=E[:], in0=E[:], in1=CS[:], op=OP.add)
                nc.vector.tensor_tensor(
                    out=E3, in0=E3,
                    in1=M[:, jsl][:, :, None].broadcast_to([128, 16, 64]),
                    op=OP.subtract)
                nc.scalar.activation(E[:], E[:], ACTF.Exp)
                nc.vector.tensor_reduce(out=Zrec[:, jsl], in_=E3, axis=AX.X,
                                        op=OP.add)
                nc.vector.reciprocal(Zrec[:, jsl], Zrec[:, jsl])
                nc.gpsimd.tensor_tensor(
                    out=E3, in0=E3,
                    in1=Zrec[:, jsl][:, :, None].broadcast_to([128, 16, 64]),
                    op=OP.mult)
                for jb in range(0, 16, 4):
                    ps = psB.tile([64, 512], F32, tag="psb", name="psb")
                    for q in range(4):
                        nc.tensor.transpose(
                            ps[:, q * 128:(q + 1) * 128],
                            E[:, (jb + q) * 64:(jb + q + 1) * 64], I128[:])
                    copy_ps(PT[:, (jc * 16 + jb) * 128:(jc * 16 + jb + 4) * 128],
                            ps[:])

            # PV for this parity: half-banks [64, 512], pairs (h, q=b)
            for b in range(RT):
                vt = work.tile([64, D], F32, tag="Vload", name="Vload")
                nc.scalar.dma_start(vt[:], vD[(2 * b + p) * 64:(2 * b + p + 1) * 64, :])
                bank = psA.tile([64, 512], F32, tag="psa", name="psa")
                for h in range(NH):
                    pr = h * 16 + b
                    nc.tensor.matmul(
                        bank[:, h * 64:(h + 1) * 64],
                        lhsT=PT4[:, :, pr],
                        rhs=vt[:, h * 64:(h + 1) * 64],
                        start=True, stop=True)
                stag = work.tile([64, 512], F32, tag="stag", name="stag")
                copy_ps(stag[:], bank[:])
                for h in range(NH):
                    base = (2 * b + p) * 64 + h * 8
                    nc.sync.dma_start(
                        aD[base:base + 8, :],
                        stag[:, h * 64:(h + 1) * 64])

    # ---------- residual + LN from aD -------------------------------------
    def resid_ln(other_nat_cb, out_cb):
        def pre_fn(rt):
            at = work.tile([128, D], F32, tag="aload", name="aload")
            nc.sync.dma_start(at[:], aD[rt * 128:(rt + 1) * 128, :])
            pt = preQ.tile([128, D], F32, tag="pre", name="pre")
            nc.vector.tensor_tensor(out=pt[:], in0=at[:], in1=other_nat_cb(rt),
                                    op=OP.add)
            return pt[:]
        for g in range(RT // 4):
            ln_group4(g, pre_fn, out_cb)

    def ln_out_to_TD(dst_dram, also_nat_dram=None):
        """LN out_cb that immediately transposes each tile into dst_dram."""
        def cb(rt, src, negmu, rstd):
            ot = work.tile([128, D], F32, tag="lnout", name="lnout", bufs=4)
            nc.vector.tensor_scalar(out=ot[:], in0=src, scalar1=negmu,
                                    scalar2=rstd, op0=OP.add, op1=OP.mult)
            if also_nat_dram is not None:
                nc.sync.dma_start(also_nat_dram[rt * 128:(rt + 1) * 128, :], ot[:])
            ps = psB.tile([128, 512], F32, tag="psb", name="psb")
            for cb_ in range(4):
                nc.tensor.transpose(ps[:, cb_ * 128:(cb_ + 1) * 128],
                                    ot[:, cb_ * 128:(cb_ + 1) * 128], I128[:])
            t = work.tile([128, 512], F32, tag="toD", name="toD", bufs=2)
            copy_ps(t[:], ps[:])
            nc.sync.dma_start(
                dst_dram[:, :, rt * 128:(rt + 1) * 128].rearrange("c a r -> a c r"),
                t[:].rearrange("a (c r) -> a c r", c=4))
        return cb

    # ---------- FFN ---------------------------------------------------------
    def ffn(xTd, resTd, w1_ap, b1_ap, w2_ap, b2_ap, out_cb):
        b2 = small.tile([1, D], F32, tag="b2", name="b2")
        nc.sync.dma_start(b2[:], b2_ap[:])
        for rc in range(4):
            xcs = []
            for dt in range(DT):
                xc = work.tile([128, 512], F32, tag=f"xfc{dt}", name=f"xfc{dt}",
                               bufs=1)
                nc.sync.dma_start(xc[:], xTd[dt, :, rc * 512:(rc + 1) * 512])
                xcs.append(xc)
            ps2 = [psB.tile([128, 512], F32, tag="psb", name="psb")
                   for _ in range(4)]
            for ff in range(FT):
                w1f = work.tile([128, 512], F32, tag="w1f", name="w1f")
                nc.scalar.dma_start(
                    w1f[:].rearrange("a (d c) -> a d c", d=4),
                    w1_ap[:, ff * 128:(ff + 1) * 128]
                        .rearrange("(d a) c -> a d c", d=4))
                b1f = small.tile([1, 128], F32, tag="b1f", name="b1f", bufs=3)
                nc.sync.dma_start(b1f[:], b1_ap[:, ff * 128:(ff + 1) * 128])
                ps1 = psA.tile([128, 512], F32, tag="psa", name="psa")
                for dt in range(DT):
                    nc.tensor.matmul(ps1[:],
                                     lhsT=w1f[:, dt * 128:(dt + 1) * 128],
                                     rhs=xcs[dt][:], start=(dt == 0), stop=False)
                nc.tensor.matmul(ps1[:], lhsT=b1f[:], rhs=ones1[:, 0:512],
                                 start=False, stop=True)
                f1f = work.tile([128, 512], F32, tag="f1f", name="f1f")
                nc.scalar.activation(f1f[:], ps1[:], ACTF.Relu)
                w2f = work.tile([128, 512], F32, tag="w2f", name="w2f")
                nc.sync.dma_start(w2f[:], w2_ap[ff * 128:(ff + 1) * 128, :])
                for rl in range(4):
                    nc.tensor.matmul(ps2[rl][:],
                                     lhsT=f1f[:, rl * 128:(rl + 1) * 128],
                                     rhs=w2f[:], start=(ff == 0), stop=False)
            def pre_fn(rt):
                rl = rt % 4
                nc.tensor.matmul(ps2[rl][:], lhsT=ones1[:, 0:128], rhs=b2[:],
                                 start=False, stop=False)
                for ct in range(DT):
                    rtl = work.tile([128, 128], F32, tag="rload", name="rload",
                                    bufs=4)
                    nc.scalar.dma_start(rtl[:], resTd[ct, :, rt * 128:(rt + 1) * 128])
                    nc.tensor.matmul(ps2[rl][:, ct * 128:(ct + 1) * 128],
                                     lhsT=rtl[:], rhs=I128[:], start=False,
                                     stop=(ct == DT - 1))
                pt = preQ.tile([128, D], F32, tag="pre", name="pre")
                copy_ps(pt[:], ps2[rl][:])
                return pt[:]
            ln_group4(rc, pre_fn, out_cb)

    # ======================= pipeline =======================
    # P1: dec1 (causal) on x_de
    embed_T_toD(hi['XdT'], xTd['xd'])
    attention(xTd['xd'], xTd['xd'], hi['dec_wv1'], hi['dec_wqk1'],
              [hi['dec1_A'][p] for p in range(2)], hi['dec1_t'], True)
    resid_ln(lambda rt: embed_nat_ps(hi['XdT'], rt)[:],
             ln_out_to_TD(xTd['m'], also_nat_dram=mnD))

    # P2: encoder self-attn on x_en
    embed_T_toD(hi['XeT'], xTd['xe'])
    attention(xTd['xe'], xTd['xe'], hi['enc_wv'], hi['enc_wqk'],
              [hi['enc_A'][p] for p in range(2)], hi['enc_t'], False)
    resid_ln(lambda rt: embed_nat_ps(hi['XeT'], rt)[:], ln_out_to_TD(xTd['o1']))

    # P3: encoder FFN
    ffn(xTd['o1'], xTd['o1'], hi['enc_w1'], hi['enc_b1'], hi['enc_w2'],
        hi['enc_b2'], ln_out_to_TD(xTd['eo']))

    # P4: dec2 cross-attn
    attention(xTd['m'], xTd['eo'], hi['dec_wv2'], hi['dec_wqk2'],
              [hi['dec2_A'][p] for p in range(2)], hi['dec2_t'], False)

    def m_reload(rt):
        t = work.tile([128, D], F32, tag="mload", name="mload", bufs=2)
        nc.sync.dma_start(t[:], mnD[rt * 128:(rt + 1) * 128, :])
        return t[:]
    resid_ln(m_reload, ln_out_to_TD(xTd['c']))

    # P5: decoder FFN
    ffn(xTd['c'], xTd['c'], hi['dec_w1'], hi['dec_b1'], hi['dec_w2'],
        hi['dec_b2'], ln_out_to_TD(xTd['of']))

    # P6: final projection + softmax
    Wo = wpool.tile([128, 4 * 64], F32, tag="Wo", name="Wo")
    for dt in range(DT):
        nc.sync.dma_start(Wo[:, dt * 64:(dt + 1) * 64],
                          hi['W_out'][dt * 128:(dt + 1) * 128, :])
    Bo = small.tile([1, 64], F32, tag="Bo", name="Bo")
    nc.sync.dma_start(Bo[:], hi['B_out'][:])
    for rt in range(RT):
        ps = psB.tile([128, 64], F32, tag="psbq", name="psbo", bufs=1)
        for dt in range(DT):
            ol = work.tile([128, 128], F32, tag="rload", name="rload", bufs=4)
            nc.sync.dma_start(ol[:], xTd['of'][dt, :, rt * 128:(rt + 1) * 128])
            nc.tensor.matmul(ps[:], lhsT=ol[:], rhs=Wo[:, dt * 64:(dt + 1) * 64],
                             start=(dt == 0), stop=False)
        nc.tensor.matmul(ps[:], lhsT=ones1[:, 0:128], rhs=Bo[:],
                         start=False, stop=True)
        mx = small.tile([128, 1], F32, tag="mx", name="mx")
        nc.vector.tensor_reduce(out=mx[:], in_=ps[:], axis=AX.X, op=OP.max,
                                negate=True)
        ex = work.tile([128, 64], F32, tag="ex", name="ex")
        nc.scalar.activation(ex[:], ps[:], ACTF.Exp, bias=mx[:])
        zs = small.tile([128, 1], F32, tag="zs", name="zs")
        nc.vector.tensor_reduce(out=zs[:], in_=ex[:], axis=AX.X, op=OP.add)
        rz = small.tile([128, 1], F32, tag="rz", name="rz")
        nc.vector.reciprocal(rz[:], zs[:])
        oo = work.tile([128, 64], F32, tag="oo", name="oo")
        nc.vector.tensor_scalar(out=oo[:], in0=ex[:], scalar1=rz[:],
                                scalar2=None, op0=OP.mult)
        nc.sync.dma_start(out_ap[rt * 128:(rt + 1) * 128, :], oo[:])


# ============================================================================
# 8-core SPMD wrapper: kernel(**inputs) -> full output
# ============================================================================
import threading

_CACHE = {}


def _get_program():
    if 'nc' not in _CACHE:
        nc = bacc.Bacc("TRN2", target_bir_lowering=False, debug=False)
        hi, out_ap = declare_io(nc)
        with tile.TileContext(nc, trace_sim=False) as tc:
            with ExitStack() as ctx:
                build(ctx, tc, hi, out_ap)
        nc.compile()
        _CACHE['nc'] = nc
    return _CACHE['nc']


def kernel(**inputs):
    from concourse.bass_utils import run_bass_kernel_spmd
    nc = _get_program()
    in_maps = [host_inputs(inputs, core) for core in range(8)]
    res = run_bass_kernel_spmd(nc, in_maps, list(range(8)))
    outs = [res.results[c]['out'] for c in range(8)]
    full = np.concatenate(outs, 0)          # [16384, 64] rows = (b, L)
    return full.reshape(64, 256, 64)



# revision 7
# speedup vs baseline: 4.4224x; 4.4224x over previous
"""Bass/Tile kernel for nn_DeepRelativeST on 8 NeuronCores (1/8 data-parallel
shard over the flat (b*L) row axis; 8 batches = 32 contiguous l-blocks per
core, so attention is core-local).

Per-core: R=2048 rows (8 batches x 256 pos), D=512, DFF=2048, H=8, dep=64,
Ll=32 local l values, 256 (l,h) softmax pairs split into two l-parity tiles:
tile p holds pair (h, l=2q+p) at partition h*16+q.

Key math (derived from reference.py):
  qs[l,h,j] = (x @ wq_headsum)[l*64+j, h]     (full Q GEMM never needed)
  ks likewise; V = x @ wv (full GEMM).
  abar[l,h,k,m] = rel[l,h,k,m-k+63] * (m<=k)  (host-gathered skew)
  r1 = sum_m abar*ks ; t = sum_m abar*m (HOST precomputed from rel)
  r2 = r1 + NEG*t ; cu = sc^2 * R1 * qs with R1 = sum_m colsum[m]*ks[m]
  (colsum = sum_k abar[.,k,.] HOST precomputed: keeps cu exact so abar can
   ship as fp8 -- r1's precision only affects softmax temperature-negligible
   terms; validated to 5.6e-5 max rel err on the host mirror.)
  logits[j,k] = cu[j]*r2[k] (+ causal NEG mask)
  p = softmax_k ; o = p @ V-block
  out row = l*64 + h*8 + j//8, col = (j%8)*64 + n   (torch raw-reshape scramble)

Transfer plan (the dispatch wall-clock is dominated by the ~35 MB/s axon
tunnel): abar ships as fp8e4 (1/4 bytes); all replicated weights ship as 1/8
row-shards and are AllGathered on-device (HBM Shared scratch); the causal
mask is built on device from a [1,4096] row; output returns as bf16.
"""
import numpy as np
from contextlib import ExitStack

import ml_dtypes

import concourse.bass as bass
import concourse.tile as tile
from concourse import bacc
from concourse import mybir

F32 = mybir.dt.float32
FP8 = mybir.dt.float8e4
BF16 = mybir.dt.bfloat16
AX = mybir.AxisListType
OP = mybir.AluOpType
ACTF = mybir.ActivationFunctionType

R, D, DFF, NH, DEP, LL = 2048, 512, 2048, 8, 64, 32
NEG, EPS, SC2 = -1e9, 1e-5, 1.0 / 64.0
RT, DT, FT = R // 128, D // 128, DFF // 128
NC8 = [[0, 1, 2, 3, 4, 5, 6, 7]]

# replicated weights: name -> full (rows, cols); shipped as [rows//8, cols]
REPW = {
    'W_in': (64, 512),
    'enc_wv': (512, 512), 'dec_wv1': (512, 512), 'dec_wv2': (512, 512),
    'enc_wqk': (512, 16), 'dec_wqk1': (512, 16), 'dec_wqk2': (512, 16),
    'enc_w1': (512, 2048), 'enc_w2': (2048, 512),
    'dec_w1': (512, 2048), 'dec_w2': (2048, 512),
    'W_out': (512, 64), 'I128': (128, 128),
}


def host_inputs(inp, core):
    f = lambda k: np.ascontiguousarray(np.asarray(inp[k], np.float32))
    bs = slice(core * 8, core * 8 + 8)
    ls = slice(core * 32, core * 32 + 32)
    Xe = f('X_en')[bs].reshape(R, 64)
    Xd = f('X_de')[bs].reshape(R, 64)

    def wqk_heads(wq, wk):
        a = wq.reshape(D, NH, DEP).sum(-1)
        b = wk.reshape(D, NH, DEP).sum(-1)
        return np.ascontiguousarray(np.concatenate([a, b], 1))  # [512,16]

    km = np.arange(64)
    kk, mm = np.meshgrid(km, km, indexing='ij')   # [k, m]

    def rel_arrange(rel):
        r = rel[ls]                                # [32,8,64,64] = [l,h,k,c]
        # abar[l,h,k,m] = r[l,h,k,m-k+63] if m<=k else 0
        c = mm - kk + 63
        valid = (mm <= kk)
        cs = np.clip(c, 0, 63)
        ab = np.take_along_axis(
            r.reshape(LL, NH, 64, 64), cs.reshape(1, 1, 64, 64), axis=3)
        ab = ab * valid.reshape(1, 1, 64, 64)
        t = (ab * mm.reshape(1, 1, 64, 64)).sum(-1)     # [l,h,k]   exact
        csum = ab.sum(-2)                               # [l,h,m]   exact
        abT = ab.transpose(1, 0, 2, 3)                  # [h,l,k,m]
        tT = t.transpose(1, 0, 2)                       # [h,l,k]
        cT = csum.transpose(1, 0, 2)                    # [h,l,m]
        A8 = np.empty((2, 128, 4096), ml_dtypes.float8_e4m3)
        Tt = np.empty((2, 128, 64), np.float32)
        Cs = np.empty((2, 128, 64), np.float32)
        for p in range(2):
            A8[p] = abT[:, p::2].reshape(128, 4096).astype(ml_dtypes.float8_e4m3)
            Tt[p] = tT[:, p::2].reshape(128, 64)
            Cs[p] = cT[:, p::2].reshape(128, 64)
        return A8, Tt, Cs

    A_e, t_e, c_e = rel_arrange(f('enc_rel'))
    A_d1, t_d1, c_d1 = rel_arrange(f('dec_rel1'))
    A_d2, t_d2, c_d2 = rel_arrange(f('dec_rel2'))
    caus_row = np.ascontiguousarray(
        np.triu(np.full((64, 64), NEG, np.float32), 1).reshape(1, 4096))

    out = {
        'XeT': np.ascontiguousarray(Xe.T), 'XdT': np.ascontiguousarray(Xd.T),
        'B_in': f('B_in').reshape(1, D),
        'enc_A': A_e, 'enc_t': t_e, 'enc_cs': c_e,
        'dec1_A': A_d1, 'dec1_t': t_d1, 'dec1_cs': c_d1,
        'dec2_A': A_d2, 'dec2_t': t_d2, 'dec2_cs': c_d2,
        'enc_b1': f('enc_b1').reshape(1, DFF), 'enc_b2': f('enc_b2').reshape(1, D),
        'dec_b1': f('dec_b1').reshape(1, DFF), 'dec_b2': f('dec_b2').reshape(1, D),
        'B_out': f('B_out').reshape(1, 64),
        'caus_row': caus_row,
    }
    fulls = {
        'W_in': f('W_in'),
        'enc_wv': f('enc_wv'), 'dec_wv1': f('dec_wv1'), 'dec_wv2': f('dec_wv2'),
        'enc_wqk': wqk_heads(f('enc_wq'), f('enc_wk')),
        'dec_wqk1': wqk_heads(f('dec_wq1'), f('dec_wk1')),
        'dec_wqk2': wqk_heads(f('dec_wq2'), f('dec_wk2')),
        'enc_w1': f('enc_w1'), 'enc_w2': f('enc_w2'),
        'dec_w1': f('dec_w1'), 'dec_w2': f('dec_w2'),
        'W_out': f('W_out'), 'I128': np.eye(128, dtype=np.float32),
    }
    for nm, (r, c) in REPW.items():
        sh = r // 8
        out[nm] = np.ascontiguousarray(fulls[nm][core * sh:(core + 1) * sh])
    return out


IN_SHAPES = {
    'XeT': ((64, R), F32), 'XdT': ((64, R), F32), 'B_in': ((1, D), F32),
    'enc_A': ((2, 128, 4096), FP8), 'dec1_A': ((2, 128, 4096), FP8),
    'dec2_A': ((2, 128, 4096), FP8),
    'enc_t': ((2, 128, 64), F32), 'dec1_t': ((2, 128, 64), F32),
    'dec2_t': ((2, 128, 64), F32),
    'enc_cs': ((2, 128, 64), F32), 'dec1_cs': ((2, 128, 64), F32),
    'dec2_cs': ((2, 128, 64), F32),
    'enc_b1': ((1, DFF), F32), 'enc_b2': ((1, D), F32),
    'dec_b1': ((1, DFF), F32), 'dec_b2': ((1, D), F32),
    'B_out': ((1, 64), F32), 'caus_row': ((1, 4096), F32),
    **{nm: ((r // 8, c), F32) for nm, (r, c) in REPW.items()},
}


def declare_io(nc):
    hi = {k: nc.dram_tensor(k, list(s), dt, kind="ExternalInput").ap()
          for k, (s, dt) in IN_SHAPES.items()}
    out = nc.dram_tensor('out', [R, 64], BF16, kind="ExternalOutput").ap()
    return hi, out


def build(ctx: ExitStack, tc: tile.TileContext, hi, out_ap, dbg=None):
    nc = tc.nc
    consts = ctx.enter_context(tc.tile_pool(name="consts", bufs=1))
    wpool = ctx.enter_context(tc.tile_pool(name="wpool", bufs=1))
    work = ctx.enter_context(tc.tile_pool(name="work", bufs=3))
    preQ = ctx.enter_context(tc.tile_pool(name="preQ", bufs=8))
    small = ctx.enter_context(tc.tile_pool(name="small", bufs=1))
    bigP = ctx.enter_context(tc.tile_pool(name="bigP", bufs=1))
    psA = ctx.enter_context(tc.tile_pool(name="psA", bufs=3, space="PSUM"))
    psB = ctx.enter_context(tc.tile_pool(name="psB", bufs=4, space="PSUM"))
    dram = ctx.enter_context(tc.tile_pool(name="dram", bufs=1, space="DRAM"))

    # ---------- gather replicated weights from 1/8 shards -------------------
    gw = {}
    for nm, (r, c) in REPW.items():
        loc = dram.tile([r // 8, c], F32, tag=f"agl_{nm}", name=f"agl_{nm}")
        nc.sync.dma_start(loc[:], hi[nm][:])
        full = dram.tile([r, c], F32, addr_space="Shared",
                         tag=f"agf_{nm}", name=f"agf_{nm}")
        nc.gpsimd.collective_compute(
            "AllGather", OP.bypass, replica_groups=NC8,
            ins=[loc[:]], outs=[full[:]])
        gw[nm] = full

    I128 = consts.tile([128, 128], F32, tag="I128", name="I128")
    nc.sync.dma_start(I128[:], gw['I128'][:])
    ones1 = consts.tile([1, D], F32, tag="ones1", name="ones1")
    nc.vector.memset(ones1[:], 1.0)
    epsc = consts.tile([128, 1], F32, tag="epsc", name="epsc")
    nc.vector.memset(epsc[:], EPS)
    W_in = consts.tile([64, D], F32, tag="W_in", name="W_in")
    nc.sync.dma_start(W_in[:], gw['W_in'][:])
    B_in = consts.tile([1, D], F32, tag="B_in", name="B_in")
    nc.sync.dma_start(B_in[:], hi['B_in'][:])

    # causal mask [128, 4096] built on device from the [1,4096] row into
    # DRAM scratch (PE partition-broadcast), streamed back at use like the
    # baseline's shipped CAUS.
    causD = dram.tile([128, 4096], F32, tag="causD", name="causD")
    for q in range(8):
        cr = work.tile([1, 512], F32, tag="xin", name="crowc")
        nc.sync.dma_start(cr[:], hi['caus_row'][:, q * 512:(q + 1) * 512])
        ps = psA.tile([128, 512], F32, tag="psa", name="psa")
        nc.tensor.matmul(ps[:], lhsT=ones1[:, 0:128], rhs=cr[:],
                         start=True, stop=True)
        st = work.tile([128, 512], F32, tag="toD", name="toD", bufs=2)
        nc.scalar.copy(st[:], ps[:])
        nc.sync.dma_start(causD[:, q * 512:(q + 1) * 512], st[:])

    # DRAM scratch: transposed activations live here, streamed at use.
    xTd = {nm: dram.tile([DT, 128, R], F32, tag=f"xTd_{nm}", name=f"xTd_{nm}")
           for nm in ('xe', 'xd', 'm', 'o1', 'eo', 'c', 'of')}
    aD = dram.tile([R, D], F32, tag="aD", name="aD")
    vD = dram.tile([R, D], F32, tag="vD", name="vD")
    mnD = dram.tile([R, D], F32, tag="mnD", name="mnD")

    def copy_ps(dst, src):
        nc.scalar.copy(dst, src)

    # ---------- embed: x.T = (X@W_in+B).T streamed to DRAM ------------------
    def embed_T_toD(x_in_ap, dst):
        for ct in range(DT):
            for rc in range(4):
                xin = work.tile([64, 512], F32, tag="xin", name="xin")
                nc.sync.dma_start(xin[:], x_in_ap[:, rc * 512:(rc + 1) * 512])
                ps = psA.tile([128, 512], F32, tag="psa", name="psa")
                nc.tensor.matmul(ps[:], lhsT=W_in[:, ct * 128:(ct + 1) * 128],
                                 rhs=xin[:], start=True, stop=False)
                nc.tensor.matmul(ps[:], lhsT=B_in[:, ct * 128:(ct + 1) * 128],
                                 rhs=ones1[:, 0:512], start=False, stop=True)
                t = work.tile([128, 512], F32, tag="toD", name="toD", bufs=2)
                copy_ps(t[:], ps[:])
                nc.sync.dma_start(dst[ct, :, rc * 512:(rc + 1) * 512], t[:])

    def embed_nat_ps(x_in_ap, rt):
        xin = work.tile([64, 128], F32, tag="xin2", name="xin2")
        nc.sync.dma_start(xin[:], x_in_ap[:, rt * 128:(rt + 1) * 128])
        ps = psA.tile([128, 512], F32, tag="psa", name="psa")
        nc.tensor.matmul(ps[:], lhsT=xin[:], rhs=W_in[:], start=True, stop=False)
        nc.tensor.matmul(ps[:], lhsT=ones1[:, 0:128], rhs=B_in[:],
                         start=False, stop=True)
        return ps

    # ---------- layernorm over one group of 4 row-tiles ---------------------
    def ln_group4(g, pre_fn, out_cb):
        """pre_fn(rt) -> [128,512] AP (lazy); out_cb(rt, src, nmu, rstd)."""
        if True:
            sx = small.tile([128, 4], F32, tag="sx", name="sx", bufs=2)
            sx2 = small.tile([128, 4], F32, tag="sx2", name="sx2", bufs=2)
            pres = []
            for i in range(4):
                pa = pre_fn(g * 4 + i)
                pres.append(pa)
                scr = work.tile([128, D], F32, tag="lnscr", name="lnscr")
                nc.scalar.activation(scr[:], pa, ACTF.Copy,
                                     accum_out=sx[:, i:i + 1])
                nc.scalar.activation(scr[:], pa, ACTF.Square,
                                     accum_out=sx2[:, i:i + 1])
            negmu = small.tile([128, 4], F32, tag="negmu", name="negmu", bufs=2)
            nc.vector.tensor_scalar(out=negmu[:], in0=sx[:], scalar1=-1.0 / D,
                                    scalar2=None, op0=OP.mult)
            mu2 = small.tile([128, 4], F32, tag="mu2", name="mu2", bufs=2)
            nc.vector.tensor_tensor(out=mu2[:], in0=negmu[:], in1=negmu[:],
                                    op=OP.mult)
            var = small.tile([128, 4], F32, tag="var", name="var", bufs=2)
            nc.vector.scalar_tensor_tensor(out=var[:], in0=sx2[:],
                                           scalar=1.0 / D, in1=mu2[:],
                                           op0=OP.mult, op1=OP.subtract)
            std = small.tile([128, 4], F32, tag="std", name="std", bufs=2)
            nc.scalar.activation(std[:], var[:], ACTF.Sqrt, bias=epsc[:])
            rstd = small.tile([128, 4], F32, tag="rstd", name="rstd", bufs=2)
            nc.vector.reciprocal(rstd[:], std[:])
            for i in range(4):
                out_cb(g * 4 + i, pres[i], negmu[:, i:i + 1], rstd[:, i:i + 1])

    # ---------- attention ---------------------------------------------------
    def attention(xqTd, xkvTd, wv_ap, wqk_ap, A_ap, t_ap, cs_ap, causal):
        # V GEMM (x.T-stationary tiles streamed from DRAM) -> vD
        wv = wpool.tile([128, 4 * D], F32, tag="wv", name="wv")
        for dt in range(DT):
            nc.sync.dma_start(wv[:, dt * D:(dt + 1) * D],
                              wv_ap[dt * 128:(dt + 1) * 128, :])
        for rt in range(RT):
            ps = psA.tile([128, 512], F32, tag="psa", name="psa")
            for dt in range(DT):
                xl = work.tile([128, 128], F32, tag="xlT", name="xlT")
                nc.sync.dma_start(xl[:], xkvTd[dt, :, rt * 128:(rt + 1) * 128])
                nc.tensor.matmul(ps[:], lhsT=xl[:],
                                 rhs=wv[:, dt * D:(dt + 1) * D],
                                 start=(dt == 0), stop=(dt == DT - 1))
            vt = work.tile([128, D], F32, tag="Vtile", name="Vtile")
            copy_ps(vt[:], ps[:])
            nc.sync.dma_start(vD[rt * 128:(rt + 1) * 128, :], vt[:])

        # qs / ks GEMMs (W-stationary, M=8)
        wqk = wpool.tile([128, 4 * 16], F32, tag="wqk", name="wqk")
        for dt in range(DT):
            nc.sync.dma_start(wqk[:, dt * 16:(dt + 1) * 16],
                              wqk_ap[dt * 128:(dt + 1) * 128, :])
        qT = work.tile([8, R], F32, tag="qT", name="qT", bufs=1)
        kT = work.tile([8, R], F32, tag="kT", name="kT", bufs=1)
        for (dst, colofs, srcTd) in ((qT, 0, xqTd), (kT, 8, xkvTd)):
            for rc in range(4):
                ps = psB.tile([8, 512], F32, tag="psbq", name="psbq", bufs=1)
                for dt in range(DT):
                    xc = work.tile([128, 512], F32, tag="xcT", name="xcT")
                    nc.sync.dma_start(xc[:], srcTd[dt, :, rc * 512:(rc + 1) * 512])
                    nc.tensor.matmul(
                        ps[:], lhsT=wqk[:, dt * 16 + colofs: dt * 16 + colofs + 8],
                        rhs=xc[:], start=(dt == 0), stop=(dt == DT - 1))
                copy_ps(dst[:, rc * 512:(rc + 1) * 512], ps[:])

        qs_pp = small.tile([128, 2 * 64], F32, tag="qs_pp", name="qs_pp")
        ks_pp = small.tile([128, 2 * 64], F32, tag="ks_pp", name="ks_pp")
        qD = dram.tile([8, R], F32, tag="qD", name="qD")
        kD = dram.tile([8, R], F32, tag="kD", name="kD")
        for (src, bounce, dst) in ((qT, qD, qs_pp), (kT, kD, ks_pp)):
            nc.sync.dma_start(bounce[:], src[:])
            nc.sync.dma_start(
                dst[:], bounce[:].rearrange("h (q f) -> (h q) f", q=16))

        # r1 = sum_m abar*ks (abar arrives fp8; upcast then mult-reduce)
        r1 = small.tile([128, 2 * 64], F32, tag="r1", name="r1")
        for p in range(2):
            for kc in range(4):
                A8t = work.tile([128, 1024], FP8, tag="A8chunk", name="A8chunk",
                                bufs=1)
                nc.scalar.dma_start(A8t[:], A_ap[p][:, kc * 1024:(kc + 1) * 1024])
                A = work.tile([128, 1024], F32, tag="Achunk", name="Achunk", bufs=1)
                nc.vector.tensor_copy(A[:], A8t[:])
                A3 = A[:].rearrange("a (k m) -> a k m", k=16)
                nc.gpsimd.tensor_tensor(
                    out=A3, in0=A3,
                    in1=ks_pp[:, p * 64:(p + 1) * 64][:, None, :]
                        .broadcast_to([128, 16, 64]), op=OP.mult)
                nc.vector.tensor_reduce(
                    out=r1[:, p * 64 + kc * 16: p * 64 + (kc + 1) * 16],
                    in_=A3, axis=AX.X, op=OP.add)
        tH = small.tile([128, 2 * 64], F32, tag="tH", name="tH")
        nc.sync.dma_start(tH[:].rearrange("a (p k) -> a p k", p=2),
                          t_ap[:].rearrange("p a k -> a p k"))
        r2 = small.tile([128, 2 * 64], F32, tag="r2", name="r2")
        nc.vector.scalar_tensor_tensor(out=r2[:], in0=tH[:], scalar=NEG,
                                       in1=r1[:], op0=OP.mult, op1=OP.add)
        # R1 exact via host colsum: R1[p] = sum_m colsum[m]*ks[m]
        csH = small.tile([128, 2 * 64], F32, tag="csH", name="csH")
        nc.sync.dma_start(csH[:].rearrange("a (p k) -> a p k", p=2),
                          cs_ap[:].rearrange("p a k -> a p k"))
        csk = small.tile([128, 2 * 64], F32, tag="csk", name="csk")
        nc.vector.tensor_tensor(out=csk[:], in0=csH[:], in1=ks_pp[:], op=OP.mult)
        R1s = small.tile([128, 2], F32, tag="R1s", name="R1s")
        nc.vector.tensor_reduce(out=R1s[:],
                                in_=csk[:].rearrange("a (p k) -> a p k", p=2),
                                axis=AX.X, op=OP.add)
        nc.vector.tensor_scalar(out=R1s[:], in0=R1s[:], scalar1=SC2,
                                scalar2=None, op0=OP.mult)
        cu = small.tile([128, 2 * 64], F32, tag="cu", name="cu")
        for p in range(2):
            nc.vector.tensor_scalar(out=cu[:, p * 64:(p + 1) * 64],
                                    in0=qs_pp[:, p * 64:(p + 1) * 64],
                                    scalar1=R1s[:, p:p + 1], scalar2=None,
                                    op0=OP.mult)

        # M = rowmax of logits (rank-1 trick; scans for causal)
        M = small.tile([128, 2 * 64], F32, tag="Mm", name="Mm")
        t1 = small.tile([128, 64], F32, tag="Mt1", name="Mt1")
        t2 = small.tile([128, 64], F32, tag="Mt2", name="Mt2")
        if not causal:
            wmax = small.tile([128, 2], F32, tag="wmax", name="wmax")
            wmin = small.tile([128, 2], F32, tag="wmin", name="wmin")
            nc.vector.tensor_reduce(out=wmax[:],
                                    in_=r2[:].rearrange("a (p k) -> a p k", p=2),
                                    axis=AX.X, op=OP.max)
            nc.vector.tensor_reduce(out=wmin[:],
                                    in_=r2[:].rearrange("a (p k) -> a p k", p=2),
                                    axis=AX.X, op=OP.min)
            for p in range(2):
                sl = slice(p * 64, (p + 1) * 64)
                nc.vector.tensor_scalar(out=M[:, sl], in0=cu[:, sl],
                                        scalar1=wmax[:, p:p + 1], scalar2=None,
                                        op0=OP.mult)
                nc.vector.tensor_scalar(out=t1[:], in0=cu[:, sl],
                                        scalar1=wmin[:, p:p + 1], scalar2=None,
                                        op0=OP.mult)
                nc.vector.tensor_tensor(out=M[:, sl], in0=M[:, sl], in1=t1[:],
                                        op=OP.max)
        else:
            pm = small.tile([128, 128], F32, tag="pm", name="pm")
            pn = small.tile([128, 128], F32, tag="pn", name="pn")
            sm = small.tile([128, 128], F32, tag="sm", name="sm")
            sn = small.tile([128, 128], F32, tag="sn", name="sn")
            for p in range(2):
                sl = slice(p * 64, (p + 1) * 64)
                w_ = r2[:, sl]
                wr = r2[:, sl][:, ::-1]
                nc.vector.tensor_tensor_scan(out=pm[:, sl], data0=w_, data1=w_,
                                             initial=-3e38, op0=OP.max, op1=OP.bypass)
                nc.vector.tensor_tensor_scan(out=pn[:, sl], data0=w_, data1=w_,
                                             initial=3e38, op0=OP.min, op1=OP.bypass)
                nc.vector.tensor_tensor_scan(out=sm[:, sl][:, ::-1], data0=wr,
                                             data1=wr, initial=-3e38,
                                             op0=OP.max, op1=OP.bypass)
                nc.vector.tensor_tensor_scan(out=sn[:, sl][:, ::-1], data0=wr,
                                             data1=wr, initial=3e38,
                                             op0=OP.min, op1=OP.bypass)
            for p in range(2):
                sl = slice(p * 64, (p + 1) * 64)
                nc.vector.tensor_tensor(out=M[:, sl], in0=cu[:, sl],
                                        in1=pm[:, sl], op=OP.mult)
                nc.vector.tensor_tensor(out=t1[:], in0=cu[:, sl], in1=pn[:, sl],
                                        op=OP.mult)
                nc.vector.tensor_tensor(out=M[:, sl], in0=M[:, sl], in1=t1[:],
                                        op=OP.max)
                j63 = slice(p * 64, p * 64 + 63)
                cs = cu[:, j63]
                nc.vector.tensor_tensor(out=t1[:, 0:63], in0=cs,
                                        in1=sm[:, p * 64 + 1:(p + 1) * 64],
                                        op=OP.mult)
                nc.vector.tensor_tensor(out=t2[:, 0:63], in0=cs,
                                        in1=sn[:, p * 64 + 1:(p + 1) * 64],
                                        op=OP.mult)
                nc.vector.tensor_tensor(out=t1[:, 0:63], in0=t1[:, 0:63],
                                        in1=t2[:, 0:63], op=OP.max)
                nc.vector.tensor_scalar(out=t1[:, 0:63], in0=t1[:, 0:63],
                                        scalar1=NEG, scalar2=None, op0=OP.add)
                nc.vector.tensor_tensor(out=M[:, j63], in0=M[:, j63],
                                        in1=t1[:, 0:63], op=OP.max)

        # E chunks of 16 j: build/mask/-M/exp/Z/scale -> transpose to PT -> PV
        Zrec = small.tile([128, 2 * 64], F32, tag="Zrec", name="Zrec")
        for p in range(2):
            PT = bigP.tile([64, 64 * 128], F32, tag="PT", name="PT")
            PT4 = PT[:].rearrange("k (j pp) -> k j pp", j=64)
            for jc in range(4):
                jsl = slice(p * 64 + jc * 16, p * 64 + (jc + 1) * 16)
                E = work.tile([128, 1024], F32, tag="Echunk", name="Echunk", bufs=2)
                E3 = E[:].rearrange("a (j k) -> a j k", j=16)
                nc.vector.tensor_tensor(
                    out=E3, in0=cu[:, jsl][:, :, None].broadcast_to([128, 16, 64]),
                    in1=r2[:, p * 64:(p + 1) * 64][:, None, :]
                        .broadcast_to([128, 16, 64]), op=OP.mult)
                if causal:
                    CS = work.tile([128, 1024], F32, tag="CSchunk", name="CSchunk",
                                   bufs=2)
                    nc.scalar.dma_start(CS[:], causD[:, jc * 1024:(jc + 1) * 1024])
                    nc.gpsimd.tensor_tensor(out=E[:], in0=E[:], in1=CS[:], op=OP.add)
                nc.vector.tensor_tensor(
                    out=E3, in0=E3,
                    in1=M[:, jsl][:, :, None].broadcast_to([128, 16, 64]),
                    op=OP.subtract)
                nc.scalar.activation(E[:], E[:], ACTF.Exp)
                nc.vector.tensor_reduce(out=Zrec[:, jsl], in_=E3, axis=AX.X,
                                        op=OP.add)
                nc.vector.reciprocal(Zrec[:, jsl], Zrec[:, jsl])
                nc.gpsimd.tensor_tensor(
                    out=E3, in0=E3,
                    in1=Zrec[:, jsl][:, :, None].broadcast_to([128, 16, 64]),
                    op=OP.mult)
                for jb in range(0, 16, 4):
                    ps = psB.tile([64, 512], F32, tag="psb", name="psb")
                    for q in range(4):
                        nc.tensor.transpose(
                            ps[:, q * 128:(q + 1) * 128],
                            E[:, (jb + q) * 64:(jb + q + 1) * 64], I128[:])
                    copy_ps(PT[:, (jc * 16 + jb) * 128:(jc * 16 + jb + 4) * 128],
                            ps[:])

            # PV for this parity: half-banks [64, 512], pairs (h, q=b)
            for b in range(RT):
                vt = work.tile([64, D], F32, tag="Vload", name="Vload")
                nc.scalar.dma_start(vt[:], vD[(2 * b + p) * 64:(2 * b + p + 1) * 64, :])
                bank = psA.tile([64, 512], F32, tag="psa", name="psa")
                for h in range(NH):
                    pr = h * 16 + b
                    nc.tensor.matmul(
                        bank[:, h * 64:(h + 1) * 64],
                        lhsT=PT4[:, :, pr],
                        rhs=vt[:, h * 64:(h + 1) * 64],
                        start=True, stop=True)
                stag = work.tile([64, 512], F32, tag="stag", name="stag")
                copy_ps(stag[:], bank[:])
                for h in range(NH):
                    base = (2 * b + p) * 64 + h * 8
                    nc.sync.dma_start(
                        aD[base:base + 8, :],
                        stag[:, h * 64:(h + 1) * 64])

    # ---------- residual + LN from aD -------------------------------------
    def resid_ln(other_nat_cb, out_cb):
        def pre_fn(rt):
            at = work.tile([128, D], F32, tag="aload", name="aload")
            nc.sync.dma_start(at[:], aD[rt * 128:(rt + 1) * 128, :])
            pt = preQ.tile([128, D], F32, tag="pre", name="pre")
            nc.vector.tensor_tensor(out=pt[:], in0=at[:], in1=other_nat_cb(rt),
                                    op=OP.add)
            return pt[:]
        for g in range(RT // 4):
            ln_group4(g, pre_fn, out_cb)

    def ln_out_to_TD(dst_dram, also_nat_dram=None):
        """LN out_cb that immediately transposes each tile into dst_dram."""
        def cb(rt, src, negmu, rstd):
            ot = work.tile([128, D], F32, tag="lnout", name="lnout", bufs=4)
            nc.vector.tensor_scalar(out=ot[:], in0=src, scalar1=negmu,
                                    scalar2=rstd, op0=OP.add, op1=OP.mult)
            if also_nat_dram is not None:
                nc.sync.dma_start(also_nat_dram[rt * 128:(rt + 1) * 128, :], ot[:])
            ps = psB.tile([128, 512], F32, tag="psb", name="psb")
            for cb_ in range(4):
                nc.tensor.transpose(ps[:, cb_ * 128:(cb_ + 1) * 128],
                                    ot[:, cb_ * 128:(cb_ + 1) * 128], I128[:])
            t = work.tile([128, 512], F32, tag="toD", name="toD", bufs=2)
            copy_ps(t[:], ps[:])
            nc.sync.dma_start(
                dst_dram[:, :, rt * 128:(rt + 1) * 128].rearrange("c a r -> a c r"),
                t[:].rearrange("a (c r) -> a c r", c=4))
        return cb

    # ---------- FFN ---------------------------------------------------------
    def ffn(xTd, resTd, w1_ap, b1_ap, w2_ap, b2_ap, out_cb):
        b2 = small.tile([1, D], F32, tag="b2", name="b2")
        nc.sync.dma_start(b2[:], b2_ap[:])
        for rc in range(4):
            xcs = []
            for dt in range(DT):
                xc = work.tile([128, 512], F32, tag=f"xfc{dt}", name=f"xfc{dt}",
                               bufs=1)
                nc.sync.dma_start(xc[:], xTd[dt, :, rc * 512:(rc + 1) * 512])
                xcs.append(xc)
            ps2 = [psB.tile([128, 512], F32, tag="psb", name="psb")
                   for _ in range(4)]
            for ff in range(FT):
                w1f = work.tile([128, 512], F32, tag="w1f", name="w1f")
                nc.scalar.dma_start(
                    w1f[:].rearrange("a (d c) -> a d c", d=4),
                    w1_ap[:, ff * 128:(ff + 1) * 128]
                        .rearrange("(d a) c -> a d c", d=4))
                b1f = small.tile([1, 128], F32, tag="b1f", name="b1f", bufs=3)
                nc.sync.dma_start(b1f[:], b1_ap[:, ff * 128:(ff + 1) * 128])
                ps1 = psA.tile([128, 512], F32, tag="psa", name="psa")
                for dt in range(DT):
                    nc.tensor.matmul(ps1[:],
                                     lhsT=w1f[:, dt * 128:(dt + 1) * 128],
                                     rhs=xcs[dt][:], start=(dt == 0), stop=False)
                nc.tensor.matmul(ps1[:], lhsT=b1f[:], rhs=ones1[:, 0:512],
                                 start=False, stop=True)
                f1f = work.tile([128, 512], F32, tag="f1f", name="f1f")
                nc.scalar.activation(f1f[:], ps1[:], ACTF.Relu)
                w2f = work.tile([128, 512], F32, tag="w2f", name="w2f")
                nc.sync.dma_start(w2f[:], w2_ap[ff * 128:(ff + 1) * 128, :])
                for rl in range(4):
                    nc.tensor.matmul(ps2[rl][:],
                                     lhsT=f1f[:, rl * 128:(rl + 1) * 128],
                                     rhs=w2f[:], start=(ff == 0), stop=False)
            def pre_fn(rt):
                rl = rt % 4
                nc.tensor.matmul(ps2[rl][:], lhsT=ones1[:, 0:128], rhs=b2[:],
                                 start=False, stop=False)
                for ct in range(DT):
                    rtl = work.tile([128, 128], F32, tag="rload", name="rload",
                                    bufs=4)
                    nc.scalar.dma_start(rtl[:], resTd[ct, :, rt * 128:(rt + 1) * 128])
                    nc.tensor.matmul(ps2[rl][:, ct * 128:(ct + 1) * 128],
                                     lhsT=rtl[:], rhs=I128[:], start=False,
                                     stop=(ct == DT - 1))
                pt = preQ.tile([128, D], F32, tag="pre", name="pre")
                copy_ps(pt[:], ps2[rl][:])
                return pt[:]
            ln_group4(rc, pre_fn, out_cb)

    # ======================= pipeline =======================
    # P1: dec1 (causal) on x_de
    embed_T_toD(hi['XdT'], xTd['xd'])
    attention(xTd['xd'], xTd['xd'], gw['dec_wv1'][:], gw['dec_wqk1'][:],
              [hi['dec1_A'][p] for p in range(2)], hi['dec1_t'], hi['dec1_cs'],
              True)
    resid_ln(lambda rt: embed_nat_ps(hi['XdT'], rt)[:],
             ln_out_to_TD(xTd['m'], also_nat_dram=mnD))

    # P2: encoder self-attn on x_en
    embed_T_toD(hi['XeT'], xTd['xe'])
    attention(xTd['xe'], xTd['xe'], gw['enc_wv'][:], gw['enc_wqk'][:],
              [hi['enc_A'][p] for p in range(2)], hi['enc_t'], hi['enc_cs'],
              False)
    resid_ln(lambda rt: embed_nat_ps(hi['XeT'], rt)[:], ln_out_to_TD(xTd['o1']))

    # P3: encoder FFN
    ffn(xTd['o1'], xTd['o1'], gw['enc_w1'][:], hi['enc_b1'], gw['enc_w2'][:],
        hi['enc_b2'], ln_out_to_TD(xTd['eo']))

    # P4: dec2 cross-attn
    attention(xTd['m'], xTd['eo'], gw['dec_wv2'][:], gw['dec_wqk2'][:],
              [hi['dec2_A'][p] for p in range(2)], hi['dec2_t'], hi['dec2_cs'],
              False)

    def m_reload(rt):
        t = work.tile([128, D], F32, tag="mload", name="mload", bufs=2)
        nc.sync.dma_start(t[:], mnD[rt * 128:(rt + 1) * 128, :])
        return t[:]
    resid_ln(m_reload, ln_out_to_TD(xTd['c']))

    # P5: decoder FFN
    ffn(xTd['c'], xTd['c'], gw['dec_w1'][:], hi['dec_b1'], gw['dec_w2'][:],
        hi['dec_b2'], ln_out_to_TD(xTd['of']))

    # P6: final projection + softmax (output ships as bf16)
    Wo = wpool.tile([128, 4 * 64], F32, tag="Wo", name="Wo")
    for dt in range(DT):
        nc.sync.dma_start(Wo[:, dt * 64:(dt + 1) * 64],
                          gw['W_out'][dt * 128:(dt + 1) * 128, :])
    Bo = small.tile([1, 64], F32, tag="Bo", name="Bo")
    nc.sync.dma_start(Bo[:], hi['B_out'][:])
    for rt in range(RT):
        ps = psB.tile([128, 64], F32, tag="psbq", name="psbo", bufs=1)
        for dt in range(DT):
            ol = work.tile([128, 128], F32, tag="rload", name="rload", bufs=4)
            nc.sync.dma_start(ol[:], xTd['of'][dt, :, rt * 128:(rt + 1) * 128])
            nc.tensor.matmul(ps[:], lhsT=ol[:], rhs=Wo[:, dt * 64:(dt + 1) * 64],
                             start=(dt == 0), stop=False)
        nc.tensor.matmul(ps[:], lhsT=ones1[:, 0:128], rhs=Bo[:],
                         start=False, stop=True)
        mx = small.tile([128, 1], F32, tag="mx", name="mx")
        nc.vector.tensor_reduce(out=mx[:], in_=ps[:], axis=AX.X, op=OP.max,
                                negate=True)
        ex = work.tile([128, 64], F32, tag="ex", name="ex")
        nc.scalar.activation(ex[:], ps[:], ACTF.Exp, bias=mx[:])
        zs = small.tile([128, 1], F32, tag="zs", name="zs")
        nc.vector.tensor_reduce(out=zs[:], in_=ex[:], axis=AX.X, op=OP.add)
        rz = small.tile([128, 1], F32, tag="rz", name="rz")
        nc.vector.reciprocal(rz[:], zs[:])
        oo = work.tile([128, 64], F32, tag="oo", name="oo")
        nc.vector.tensor_scalar(out=oo[:], in0=ex[:], scalar1=rz[:],
                                scalar2=None, op0=OP.mult)
        oo16 = work.tile([128, 64], BF16, tag="oo16", name="oo16")
        nc.vector.tensor_copy(oo16[:], oo[:])
        nc.sync.dma_start(out_ap[rt * 128:(rt + 1) * 128, :], oo16[:])


# ============================================================================
# 8-core SPMD wrapper with a cached PJRT dispatcher: kernel(**inputs) -> out
# ============================================================================
_CACHE = {}


def _get_program():
    if 'nc' not in _CACHE:
        nc = bacc.Bacc("TRN2", target_bir_lowering=False, debug=False)
        hi, out_ap = declare_io(nc)
        with tile.TileContext(nc, trace_sim=False) as tc:
            with ExitStack() as ctx:
                build(ctx, tc, hi, out_ap)
        nc.compile()
        _CACHE['nc'] = nc
    return _CACHE['nc']


def _get_dispatcher():
    """One cached jit(shard_map(...)) wrapper -- same execution path as
    bass_utils.run_bass_kernel_spmd under axon (bass2jax/_bass_exec_p via
    PJRT), but without rebuilding/retracing the wrapper on every call."""
    if 'disp' in _CACHE:
        return _CACHE['disp']
    import jax
    from jax.sharding import Mesh, PartitionSpec
    from jax.experimental.shard_map import shard_map
    from concourse import bass2jax

    nc = _get_program()
    bass2jax.install_neuronx_cc_hook()
    partition_name = (nc.partition_id_tensor.name
                      if nc.partition_id_tensor else None)
    in_names, out_names, out_avals, zero_tmpl = [], [], [], []
    for alloc in nc.m.functions[0].allocations:
        if not isinstance(alloc, mybir.MemoryLocationSet):
            continue
        name = alloc.memorylocations[0].name
        if alloc.kind == "ExternalInput":
            if name != partition_name:
                in_names.append(name)
        elif alloc.kind == "ExternalOutput":
            shape = tuple(alloc.tensor_shape)
            dtype = mybir.dt.np(alloc.dtype)
            out_avals.append(jax.core.ShapedArray(shape, dtype))
            zero_tmpl.append((shape, dtype))
            out_names.append(name)
    n_params = len(in_names)
    n_outs = len(out_avals)
    all_in_names = list(in_names) + list(out_names)
    if partition_name is not None:
        all_in_names.append(partition_name)
    donate = tuple(range(n_params, n_params + n_outs))

    def _body(*args):
        operands = list(args)
        if partition_name is not None:
            operands.append(bass2jax.partition_id_tensor())
        outs = bass2jax._bass_exec_p.bind(
            *operands, out_avals=tuple(out_avals),
            in_names=tuple(all_in_names), out_names=tuple(out_names),
            lowering_input_output_aliases=(), sim_require_finite=True,
            sim_require_nnan=True, nc=nc)
        return tuple(outs)

    devices = jax.devices()[:8]
    mesh = Mesh(np.asarray(devices), ("core",))
    sharded = jax.jit(
        shard_map(_body, mesh=mesh,
                  in_specs=(PartitionSpec("core"),) * (n_params + n_outs),
                  out_specs=(PartitionSpec("core"),) * n_outs,
                  check_rep=False),
        donate_argnums=donate, keep_unused=True)

    def dispatch(in_maps):
        concat_in = [
            np.concatenate([np.asarray(in_maps[c][nm]) for c in range(8)], 0)
            for nm in in_names]
        cz = [np.zeros((8 * s[0], *s[1:]), d) for (s, d) in zero_tmpl]
        outs = sharded(*concat_in, *cz)
        return [
            {nm: np.asarray(outs[i]).reshape(8, *out_avals[i].shape)[c]
             for i, nm in enumerate(out_names)}
            for c in range(8)]

    _CACHE['disp'] = dispatch
    return dispatch


def kernel(**inputs):
    dispatch = _get_dispatcher()
    in_maps = [host_inputs(inputs, core) for core in range(8)]
    res = dispatch(in_maps)
    outs = [np.asarray(res[c]['out'], np.float32) for c in range(8)]
    full = np.concatenate(outs, 0)          # [16384, 64] rows = (b, L)
    return full.reshape(64, 256, 64)


# revision 16
# speedup vs baseline: 6.8257x; 1.5434x over previous
"""Bass/Tile kernel for nn_DeepRelativeST on 8 NeuronCores (1/8 data-parallel
shard over the flat (b*L) row axis; 8 batches = 32 contiguous l-blocks per
core, so attention is core-local).

Per-core: R=2048 rows (8 batches x 256 pos), D=512, DFF=2048, H=8, dep=64,
Ll=32 local l values, 256 (l,h) softmax pairs split into two l-parity tiles:
tile p holds pair (h, l=2q+p) at partition h*16+q.

Key math (derived from reference.py):
  qs[l,h,j] = (x @ wq_headsum)[l*64+j, h]     (full Q GEMM never needed)
  ks likewise; V = x @ wv (full GEMM).
  abar[l,h,k,m] = rel[l,h,k,m-k+63] * (m<=k)  (host-gathered skew)
  r1 = sum_m abar*ks ; t = sum_m abar*m (HOST precomputed from rel)
  r2 = r1 + NEG*t ; cu = sc^2 * R1 * qs with R1 = sum_m colsum[m]*ks[m]
  (colsum = sum_k abar[.,k,.] HOST precomputed: keeps cu exact so abar can
   ship as fp8 -- r1's precision only affects softmax temperature-negligible
   terms; validated to 5.6e-5 max rel err on the host mirror.)
  logits[j,k] = cu[j]*r2[k] (+ causal NEG mask)
  p = softmax_k ; o = p @ V-block
  out row = l*64 + h*8 + j//8, col = (j%8)*64 + n   (torch raw-reshape scramble)

Transfer plan (the dispatch wall-clock is dominated by the ~35 MB/s axon
tunnel): abar ships as fp8e4 (1/4 bytes); all replicated weights ship as 1/8
row-shards and are AllGathered on-device (HBM Shared scratch); the causal
mask is built on device from a [1,4096] row; output returns as bf16.
"""
import numpy as np
from contextlib import ExitStack

import ml_dtypes

import concourse.bass as bass
import concourse.tile as tile
from concourse import bacc
from concourse import mybir

F32 = mybir.dt.float32
FP8 = mybir.dt.float8e4
F16 = mybir.dt.float16
BF16 = mybir.dt.bfloat16
AX = mybir.AxisListType
OP = mybir.AluOpType
ACTF = mybir.ActivationFunctionType

R, D, DFF, NH, DEP, LL = 2048, 512, 2048, 8, 64, 32
NEG, EPS, SC2 = -1e9, 1e-5, 1.0 / 64.0
RT, DT, FT = R // 128, D // 128, DFF // 128
NC8 = [[0, 1, 2, 3, 4, 5, 6, 7]]

# replicated weights: name -> full (rows, cols); shipped as [rows//8, cols]
# REPW32: fp32 (attention-selection critical -- qs/ks path must be exact).
# REPW16: fp16 on the wire, upcast to fp32 on device (FFN/out path; validated
# to ~2e-4 host-side).
REPW32 = {
    'W_in': (64, 512),
    'enc_wv': (512, 512), 'dec_wv1': (512, 512), 'dec_wv2': (512, 512),
    'enc_wqk': (512, 16), 'dec_wqk1': (512, 16), 'dec_wqk2': (512, 16),
    'I128': (128, 128),
}
REPW16 = {
    'enc_w1': (512, 2048), 'enc_w2': (2048, 512),
    'dec_w1': (512, 2048), 'dec_w2': (2048, 512),
    'W_out': (512, 64),
}
# A ships fp8, triangle-packed into 4 row-segments of 16 k's, each padded to
# width 16*(s+1): row k in segment s=k//16 keeps columns m=0..16(s+1)-1
# (superset of the valid m<=k). 2560 bytes/partition vs 4096 dense.
SEG_OFF = [0, 256, 768, 1536]
APK = 2560


def host_inputs(inp, core):
    f = lambda k: np.ascontiguousarray(np.asarray(inp[k], np.float32))
    bs = slice(core * 8, core * 8 + 8)
    ls = slice(core * 32, core * 32 + 32)
    Xe = f('X_en')[bs].reshape(R, 64)
    Xd = f('X_de')[bs].reshape(R, 64)

    def wqk_heads(wq, wk):
        a = wq.reshape(D, NH, DEP).sum(-1)
        b = wk.reshape(D, NH, DEP).sum(-1)
        return np.ascontiguousarray(np.concatenate([a, b], 1))  # [512,16]

    km = np.arange(64)
    kk, mm = np.meshgrid(km, km, indexing='ij')   # [k, m]

    def rel_arrange(rel):
        r = rel[ls]                                # [32,8,64,64] = [l,h,k,c]
        # abar[l,h,k,m] = r[l,h,k,m-k+63] if m<=k else 0
        c = mm - kk + 63
        valid = (mm <= kk)
        cs = np.clip(c, 0, 63)
        ab = np.take_along_axis(
            r.reshape(LL, NH, 64, 64), cs.reshape(1, 1, 64, 64), axis=3)
        ab = ab * valid.reshape(1, 1, 64, 64)
        t = (ab * mm.reshape(1, 1, 64, 64)).sum(-1)     # [l,h,k]   exact
        csum = ab.sum(-2)                               # [l,h,m]   exact
        abT = ab.transpose(1, 0, 2, 3)                  # [h,l,k,m]
        tT = t.transpose(1, 0, 2)                       # [h,l,k]
        cT = csum.transpose(1, 0, 2)                    # [h,l,m]
        A8 = np.zeros((2, 128, APK), ml_dtypes.float8_e4m3)
        Tt = np.empty((2, 128, 64), np.float32)
        Cs = np.empty((2, 128, 64), np.float32)
        for p in range(2):
            d = abT[:, p::2].reshape(128, 64, 64)          # [a, k, m]
            for s in range(4):
                ws = 16 * (s + 1)
                seg = d[:, 16 * s:16 * (s + 1), 0:ws].reshape(128, 16 * ws)
                A8[p][:, SEG_OFF[s]:SEG_OFF[s] + 16 * ws] = \
                    seg.astype(ml_dtypes.float8_e4m3)
            Tt[p] = tT[:, p::2].reshape(128, 64)
            Cs[p] = cT[:, p::2].reshape(128, 64)
        return A8, Tt, Cs

    A_e, t_e, c_e = rel_arrange(f('enc_rel'))
    A_d1, t_d1, c_d1 = rel_arrange(f('dec_rel1'))
    A_d2, t_d2, c_d2 = rel_arrange(f('dec_rel2'))
    caus_row = np.ascontiguousarray(
        np.triu(np.full((64, 64), NEG, np.float32), 1).reshape(1, 4096))

    out = {
        'XeT': np.ascontiguousarray(Xe.T), 'XdT': np.ascontiguousarray(Xd.T),
        'B_in': f('B_in').reshape(1, D),
        'enc_A': A_e, 'enc_t': t_e, 'enc_cs': c_e,
        'dec1_A': A_d1, 'dec1_t': t_d1, 'dec1_cs': c_d1,
        'dec2_A': A_d2, 'dec2_t': t_d2, 'dec2_cs': c_d2,
        'enc_b1': f('enc_b1').reshape(1, DFF), 'enc_b2': f('enc_b2').reshape(1, D),
        'dec_b1': f('dec_b1').reshape(1, DFF), 'dec_b2': f('dec_b2').reshape(1, D),
        'B_out': f('B_out').reshape(1, 64),
        'caus_row': caus_row,
    }
    fulls = {
        'W_in': f('W_in'),
        'enc_wv': f('enc_wv'), 'dec_wv1': f('dec_wv1'), 'dec_wv2': f('dec_wv2'),
        'enc_wqk': wqk_heads(f('enc_wq'), f('enc_wk')),
        'dec_wqk1': wqk_heads(f('dec_wq1'), f('dec_wk1')),
        'dec_wqk2': wqk_heads(f('dec_wq2'), f('dec_wk2')),
        'I128': np.eye(128, dtype=np.float32),
    }
    for nm, (r, c) in REPW32.items():
        sh = r // 8
        out[nm] = np.ascontiguousarray(fulls[nm][core * sh:(core + 1) * sh])
    for nm, (r, c) in REPW16.items():
        sh = r // 8
        out[nm] = np.ascontiguousarray(
            f(nm)[core * sh:(core + 1) * sh].astype(np.float16))
    return out


IN_SHAPES = {
    'XeT': ((64, R), F32), 'XdT': ((64, R), F32), 'B_in': ((1, D), F32),
    'enc_A': ((2, 128, APK), FP8), 'dec1_A': ((2, 128, APK), FP8),
    'dec2_A': ((2, 128, APK), FP8),
    'enc_t': ((2, 128, 64), F32), 'dec1_t': ((2, 128, 64), F32),
    'dec2_t': ((2, 128, 64), F32),
    'enc_cs': ((2, 128, 64), F32), 'dec1_cs': ((2, 128, 64), F32),
    'dec2_cs': ((2, 128, 64), F32),
    'enc_b1': ((1, DFF), F32), 'enc_b2': ((1, D), F32),
    'dec_b1': ((1, DFF), F32), 'dec_b2': ((1, D), F32),
    'B_out': ((1, 64), F32), 'caus_row': ((1, 4096), F32),
    **{nm: ((r // 8, c), F32) for nm, (r, c) in REPW32.items()},
    **{nm: ((r // 8, c), F16) for nm, (r, c) in REPW16.items()},
}


def declare_io(nc):
    hi = {k: nc.dram_tensor(k, list(s), dt, kind="ExternalInput").ap()
          for k, (s, dt) in IN_SHAPES.items()}
    out = nc.dram_tensor('out', [R, 64], F16, kind="ExternalOutput").ap()
    return hi, out


def build(ctx: ExitStack, tc: tile.TileContext, hi, out_ap, dbg=None):
    nc = tc.nc
    consts = ctx.enter_context(tc.tile_pool(name="consts", bufs=1))
    wpool = ctx.enter_context(tc.tile_pool(name="wpool", bufs=1))
    work = ctx.enter_context(tc.tile_pool(name="work", bufs=3))
    preQ = ctx.enter_context(tc.tile_pool(name="preQ", bufs=8))
    small = ctx.enter_context(tc.tile_pool(name="small", bufs=1))
    bigP = ctx.enter_context(tc.tile_pool(name="bigP", bufs=1))
    psA = ctx.enter_context(tc.tile_pool(name="psA", bufs=3, space="PSUM"))
    psB = ctx.enter_context(tc.tile_pool(name="psB", bufs=4, space="PSUM"))
    dram = ctx.enter_context(tc.tile_pool(name="dram", bufs=1, space="DRAM"))

    # ---------- gather replicated weights from 1/8 shards -------------------
    gw = {}
    for nm, (r, c) in REPW32.items():
        loc = dram.tile([r // 8, c], F32, tag=f"agl_{nm}", name=f"agl_{nm}")
        nc.sync.dma_start(loc[:], hi[nm][:])
        full = dram.tile([r, c], F32, addr_space="Shared",
                         tag=f"agf_{nm}", name=f"agf_{nm}")
        nc.gpsimd.collective_compute(
            "AllGather", OP.bypass, replica_groups=NC8,
            ins=[loc[:]], outs=[full[:]])
        gw[nm] = full
    for nm, (r, c) in REPW16.items():
        loc = dram.tile([r // 8, c], F16, tag=f"agl_{nm}", name=f"agl_{nm}")
        nc.sync.dma_start(loc[:], hi[nm][:])
        full16 = dram.tile([r, c], F16, addr_space="Shared",
                           tag=f"agh_{nm}", name=f"agh_{nm}")
        nc.gpsimd.collective_compute(
            "AllGather", OP.bypass, replica_groups=NC8,
            ins=[loc[:]], outs=[full16[:]])
        full = dram.tile([r, c], F32, tag=f"agf_{nm}", name=f"agf_{nm}")
        for r0 in range(0, r, 128):
            for c0 in range(0, c, 512):
                cw = min(512, c - c0)
                t16 = work.tile([128, 512], F16, tag="u16", name="u16", bufs=2)
                nc.sync.dma_start(t16[:, 0:cw],
                                  full16[r0:r0 + 128, c0:c0 + cw])
                t32 = work.tile([128, 512], F32, tag="xcT", name="u32")
                nc.vector.tensor_copy(t32[:, 0:cw], t16[:, 0:cw])
                nc.sync.dma_start(full[r0:r0 + 128, c0:c0 + cw], t32[:, 0:cw])
        gw[nm] = full

    I128 = consts.tile([128, 128], F32, tag="I128", name="I128")
    nc.sync.dma_start(I128[:], gw['I128'][:])
    ones1 = consts.tile([1, D], F32, tag="ones1", name="ones1")
    nc.vector.memset(ones1[:], 1.0)
    epsc = consts.tile([128, 1], F32, tag="epsc", name="epsc")
    nc.vector.memset(epsc[:], EPS)
    W_in = consts.tile([64, D], F32, tag="W_in", name="W_in")
    nc.sync.dma_start(W_in[:], gw['W_in'][:])
    B_in = consts.tile([1, D], F32, tag="B_in", name="B_in")
    nc.sync.dma_start(B_in[:], hi['B_in'][:])

    # causal mask [128, 4096] built on device from the [1,4096] row into
    # DRAM scratch (PE partition-broadcast), streamed back at use like the
    # baseline's shipped CAUS.
    causD = dram.tile([128, 4096], F32, tag="causD", name="causD")
    for q in range(8):
        cr = work.tile([1, 512], F32, tag="xin", name="crowc")
        nc.sync.dma_start(cr[:], hi['caus_row'][:, q * 512:(q + 1) * 512])
        ps = psA.tile([128, 512], F32, tag="psa", name="psa")
        nc.tensor.matmul(ps[:], lhsT=ones1[:, 0:128], rhs=cr[:],
                         start=True, stop=True)
        st = work.tile([128, 512], F32, tag="toD", name="toD", bufs=2)
        nc.scalar.copy(st[:], ps[:])
        nc.sync.dma_start(causD[:, q * 512:(q + 1) * 512], st[:])

    # DRAM scratch: transposed activations live here, streamed at use.
    xTd = {nm: dram.tile([DT, 128, R], F32, tag=f"xTd_{nm}", name=f"xTd_{nm}")
           for nm in ('xe', 'xd', 'm', 'o1', 'eo', 'c', 'of')}
    aD = dram.tile([R, D], F32, tag="aD", name="aD")
    vD = dram.tile([R, D], F32, tag="vD", name="vD")
    mnD = dram.tile([R, D], F32, tag="mnD", name="mnD")

    def copy_ps(dst, src):
        nc.scalar.copy(dst, src)

    # ---------- embed: x.T = (X@W_in+B).T streamed to DRAM ------------------
    def embed_T_toD(x_in_ap, dst):
        for ct in range(DT):
            for rc in range(4):
                xin = work.tile([64, 512], F32, tag="xin", name="xin")
                nc.sync.dma_start(xin[:], x_in_ap[:, rc * 512:(rc + 1) * 512])
                ps = psA.tile([128, 512], F32, tag="psa", name="psa")
                nc.tensor.matmul(ps[:], lhsT=W_in[:, ct * 128:(ct + 1) * 128],
                                 rhs=xin[:], start=True, stop=False)
                nc.tensor.matmul(ps[:], lhsT=B_in[:, ct * 128:(ct + 1) * 128],
                                 rhs=ones1[:, 0:512], start=False, stop=True)
                t = work.tile([128, 512], F32, tag="toD", name="toD", bufs=2)
                copy_ps(t[:], ps[:])
                nc.sync.dma_start(dst[ct, :, rc * 512:(rc + 1) * 512], t[:])

    def embed_nat_ps(x_in_ap, rt):
        xin = work.tile([64, 128], F32, tag="xin2", name="xin2")
        nc.sync.dma_start(xin[:], x_in_ap[:, rt * 128:(rt + 1) * 128])
        ps = psA.tile([128, 512], F32, tag="psa", name="psa")
        nc.tensor.matmul(ps[:], lhsT=xin[:], rhs=W_in[:], start=True, stop=False)
        nc.tensor.matmul(ps[:], lhsT=ones1[:, 0:128], rhs=B_in[:],
                         start=False, stop=True)
        return ps

    # ---------- layernorm over one group of 4 row-tiles ---------------------
    def ln_group4(g, pre_fn, out_cb):
        """pre_fn(rt) -> [128,512] AP (lazy); out_cb(rt, src, nmu, rstd)."""
        if True:
            sx = small.tile([128, 4], F32, tag="sx", name="sx", bufs=2)
            sx2 = small.tile([128, 4], F32, tag="sx2", name="sx2", bufs=2)
            pres = []
            for i in range(4):
                pa = pre_fn(g * 4 + i)
                pres.append(pa)
                scr = work.tile([128, D], F32, tag="lnscr", name="lnscr")
                nc.scalar.activation(scr[:], pa, ACTF.Copy,
                                     accum_out=sx[:, i:i + 1])
                nc.scalar.activation(scr[:], pa, ACTF.Square,
                                     accum_out=sx2[:, i:i + 1])
            negmu = small.tile([128, 4], F32, tag="negmu", name="negmu", bufs=2)
            nc.vector.tensor_scalar(out=negmu[:], in0=sx[:], scalar1=-1.0 / D,
                                    scalar2=None, op0=OP.mult)
            mu2 = small.tile([128, 4], F32, tag="mu2", name="mu2", bufs=2)
            nc.vector.tensor_tensor(out=mu2[:], in0=negmu[:], in1=negmu[:],
                                    op=OP.mult)
            var = small.tile([128, 4], F32, tag="var", name="var", bufs=2)
            nc.vector.scalar_tensor_tensor(out=var[:], in0=sx2[:],
                                           scalar=1.0 / D, in1=mu2[:],
                                           op0=OP.mult, op1=OP.subtract)
            std = small.tile([128, 4], F32, tag="std", name="std", bufs=2)
            nc.scalar.activation(std[:], var[:], ACTF.Sqrt, bias=epsc[:])
            rstd = small.tile([128, 4], F32, tag="rstd", name="rstd", bufs=2)
            nc.vector.reciprocal(rstd[:], std[:])
            for i in range(4):
                out_cb(g * 4 + i, pres[i], negmu[:, i:i + 1], rstd[:, i:i + 1])

    # ---------- attention ---------------------------------------------------
    def attention(xqTd, xkvTd, wv_ap, wqk_ap, A_ap, t_ap, cs_ap, causal):
        # V GEMM (x.T-stationary tiles streamed from DRAM) -> vD
        wv = wpool.tile([128, 4 * D], F32, tag="wv", name="wv")
        for dt in range(DT):
            nc.sync.dma_start(wv[:, dt * D:(dt + 1) * D],
                              wv_ap[dt * 128:(dt + 1) * 128, :])
        for rt in range(RT):
            ps = psA.tile([128, 512], F32, tag="psa", name="psa")
            for dt in range(DT):
                xl = work.tile([128, 128], F32, tag="xlT", name="xlT")
                nc.sync.dma_start(xl[:], xkvTd[dt, :, rt * 128:(rt + 1) * 128])
                nc.tensor.matmul(ps[:], lhsT=xl[:],
                                 rhs=wv[:, dt * D:(dt + 1) * D],
                                 start=(dt == 0), stop=(dt == DT - 1))
            vt = work.tile([128, D], F32, tag="Vtile", name="Vtile")
            copy_ps(vt[:], ps[:])
            nc.sync.dma_start(vD[rt * 128:(rt + 1) * 128, :], vt[:])

        # qs / ks GEMMs (W-stationary, M=8)
        wqk = wpool.tile([128, 4 * 16], F32, tag="wqk", name="wqk")
        for dt in range(DT):
            nc.sync.dma_start(wqk[:, dt * 16:(dt + 1) * 16],
                              wqk_ap[dt * 128:(dt + 1) * 128, :])
        qT = work.tile([8, R], F32, tag="qT", name="qT", bufs=1)
        kT = work.tile([8, R], F32, tag="kT", name="kT", bufs=1)
        for (dst, colofs, srcTd) in ((qT, 0, xqTd), (kT, 8, xkvTd)):
            for rc in range(4):
                ps = psB.tile([8, 512], F32, tag="psbq", name="psbq", bufs=1)
                for dt in range(DT):
                    xc = work.tile([128, 512], F32, tag="xcT", name="xcT")
                    nc.sync.dma_start(xc[:], srcTd[dt, :, rc * 512:(rc + 1) * 512])
                    nc.tensor.matmul(
                        ps[:], lhsT=wqk[:, dt * 16 + colofs: dt * 16 + colofs + 8],
                        rhs=xc[:], start=(dt == 0), stop=(dt == DT - 1))
                copy_ps(dst[:, rc * 512:(rc + 1) * 512], ps[:])

        qs_pp = small.tile([128, 2 * 64], F32, tag="qs_pp", name="qs_pp")
        ks_pp = small.tile([128, 2 * 64], F32, tag="ks_pp", name="ks_pp")
        qD = dram.tile([8, R], F32, tag="qD", name="qD")
        kD = dram.tile([8, R], F32, tag="kD", name="kD")
        for (src, bounce, dst) in ((qT, qD, qs_pp), (kT, kD, ks_pp)):
            nc.sync.dma_start(bounce[:], src[:])
            nc.sync.dma_start(
                dst[:], bounce[:].rearrange("h (q f) -> (h q) f", q=16))

        # r1 = sum_m abar*ks (abar arrives fp8, triangle-packed in 4 segments
        # of 16 k-rows padded to width 16(s+1); upcast then mult-reduce)
        r1 = small.tile([128, 2 * 64], F32, tag="r1", name="r1")
        for p in range(2):
            for s in range(4):
                ws = 16 * (s + 1)
                width = 16 * ws
                off = SEG_OFF[s]
                A8t = work.tile([128, 1024], FP8, tag="A8chunk", name="A8chunk",
                                bufs=1)
                nc.scalar.dma_start(A8t[:, 0:width], A_ap[p][:, off:off + width])
                A = work.tile([128, 1024], F32, tag="Achunk", name="Achunk", bufs=1)
                nc.vector.tensor_copy(A[:, 0:width], A8t[:, 0:width])
                A3 = A[:, 0:width].rearrange("a (k m) -> a k m", k=16)
                nc.gpsimd.tensor_tensor(
                    out=A3, in0=A3,
                    in1=ks_pp[:, p * 64:p * 64 + ws][:, None, :]
                        .broadcast_to([128, 16, ws]), op=OP.mult)
                nc.vector.tensor_reduce(
                    out=r1[:, p * 64 + s * 16: p * 64 + (s + 1) * 16],
                    in_=A3, axis=AX.X, op=OP.add)
        tH = small.tile([128, 2 * 64], F32, tag="tH", name="tH")
        nc.sync.dma_start(tH[:].rearrange("a (p k) -> a p k", p=2),
                          t_ap[:].rearrange("p a k -> a p k"))
        r2 = small.tile([128, 2 * 64], F32, tag="r2", name="r2")
        nc.vector.scalar_tensor_tensor(out=r2[:], in0=tH[:], scalar=NEG,
                                       in1=r1[:], op0=OP.mult, op1=OP.add)
        # R1 exact via host colsum: R1[p] = sum_m colsum[m]*ks[m]
        csH = small.tile([128, 2 * 64], F32, tag="csH", name="csH")
        nc.sync.dma_start(csH[:].rearrange("a (p k) -> a p k", p=2),
                          cs_ap[:].rearrange("p a k -> a p k"))
        csk = small.tile([128, 2 * 64], F32, tag="csk", name="csk")
        nc.vector.tensor_tensor(out=csk[:], in0=csH[:], in1=ks_pp[:], op=OP.mult)
        R1s = small.tile([128, 2], F32, tag="R1s", name="R1s")
        nc.vector.tensor_reduce(out=R1s[:],
                                in_=csk[:].rearrange("a (p k) -> a p k", p=2),
                                axis=AX.X, op=OP.add)
        nc.vector.tensor_scalar(out=R1s[:], in0=R1s[:], scalar1=SC2,
                                scalar2=None, op0=OP.mult)
        cu = small.tile([128, 2 * 64], F32, tag="cu", name="cu")
        for p in range(2):
            nc.vector.tensor_scalar(out=cu[:, p * 64:(p + 1) * 64],
                                    in0=qs_pp[:, p * 64:(p + 1) * 64],
                                    scalar1=R1s[:, p:p + 1], scalar2=None,
                                    op0=OP.mult)

        # M = rowmax of logits (rank-1 trick; scans for causal)
        M = small.tile([128, 2 * 64], F32, tag="Mm", name="Mm")
        t1 = small.tile([128, 64], F32, tag="Mt1", name="Mt1")
        t2 = small.tile([128, 64], F32, tag="Mt2", name="Mt2")
        if not causal:
            wmax = small.tile([128, 2], F32, tag="wmax", name="wmax")
            wmin = small.tile([128, 2], F32, tag="wmin", name="wmin")
            nc.vector.tensor_reduce(out=wmax[:],
                                    in_=r2[:].rearrange("a (p k) -> a p k", p=2),
                                    axis=AX.X, op=OP.max)
            nc.vector.tensor_reduce(out=wmin[:],
                                    in_=r2[:].rearrange("a (p k) -> a p k", p=2),
                                    axis=AX.X, op=OP.min)
            for p in range(2):
                sl = slice(p * 64, (p + 1) * 64)
                nc.vector.tensor_scalar(out=M[:, sl], in0=cu[:, sl],
                                        scalar1=wmax[:, p:p + 1], scalar2=None,
                                        op0=OP.mult)
                nc.vector.tensor_scalar(out=t1[:], in0=cu[:, sl],
                                        scalar1=wmin[:, p:p + 1], scalar2=None,
                                        op0=OP.mult)
                nc.vector.tensor_tensor(out=M[:, sl], in0=M[:, sl], in1=t1[:],
                                        op=OP.max)
        else:
            pm = small.tile([128, 128], F32, tag="pm", name="pm")
            pn = small.tile([128, 128], F32, tag="pn", name="pn")
            sm = small.tile([128, 128], F32, tag="sm", name="sm")
            sn = small.tile([128, 128], F32, tag="sn", name="sn")
            for p in range(2):
                sl = slice(p * 64, (p + 1) * 64)
                w_ = r2[:, sl]
                wr = r2[:, sl][:, ::-1]
                nc.vector.tensor_tensor_scan(out=pm[:, sl], data0=w_, data1=w_,
                                             initial=-3e38, op0=OP.max, op1=OP.bypass)
                nc.vector.tensor_tensor_scan(out=pn[:, sl], data0=w_, data1=w_,
                                             initial=3e38, op0=OP.min, op1=OP.bypass)
                nc.vector.tensor_tensor_scan(out=sm[:, sl][:, ::-1], data0=wr,
                                             data1=wr, initial=-3e38,
                                             op0=OP.max, op1=OP.bypass)
                nc.vector.tensor_tensor_scan(out=sn[:, sl][:, ::-1], data0=wr,
                                             data1=wr, initial=3e38,
                                             op0=OP.min, op1=OP.bypass)
            for p in range(2):
                sl = slice(p * 64, (p + 1) * 64)
                nc.vector.tensor_tensor(out=M[:, sl], in0=cu[:, sl],
                                        in1=pm[:, sl], op=OP.mult)
                nc.vector.tensor_tensor(out=t1[:], in0=cu[:, sl], in1=pn[:, sl],
                                        op=OP.mult)
                nc.vector.tensor_tensor(out=M[:, sl], in0=M[:, sl], in1=t1[:],
                                        op=OP.max)
                j63 = slice(p * 64, p * 64 + 63)
                cs = cu[:, j63]
                nc.vector.tensor_tensor(out=t1[:, 0:63], in0=cs,
                                        in1=sm[:, p * 64 + 1:(p + 1) * 64],
                                        op=OP.mult)
                nc.vector.tensor_tensor(out=t2[:, 0:63], in0=cs,
                                        in1=sn[:, p * 64 + 1:(p + 1) * 64],
                                        op=OP.mult)
                nc.vector.tensor_tensor(out=t1[:, 0:63], in0=t1[:, 0:63],
                                        in1=t2[:, 0:63], op=OP.max)
                nc.vector.tensor_scalar(out=t1[:, 0:63], in0=t1[:, 0:63],
                                        scalar1=NEG, scalar2=None, op0=OP.add)
                nc.vector.tensor_tensor(out=M[:, j63], in0=M[:, j63],
                                        in1=t1[:, 0:63], op=OP.max)

        # E chunks of 16 j: build/mask/-M/exp/Z/scale -> transpose to PT -> PV
        Zrec = small.tile([128, 2 * 64], F32, tag="Zrec", name="Zrec")
        for p in range(2):
            PT = bigP.tile([64, 64 * 128], F32, tag="PT", name="PT")
            PT4 = PT[:].rearrange("k (j pp) -> k j pp", j=64)
            for jc in range(4):
                jsl = slice(p * 64 + jc * 16, p * 64 + (jc + 1) * 16)
                E = work.tile([128, 1024], F32, tag="Echunk", name="Echunk", bufs=2)
                E3 = E[:].rearrange("a (j k) -> a j k", j=16)
                nc.vector.tensor_tensor(
                    out=E3, in0=cu[:, jsl][:, :, None].broadcast_to([128, 16, 64]),
                    in1=r2[:, p * 64:(p + 1) * 64][:, None, :]
                        .broadcast_to([128, 16, 64]), op=OP.mult)
                if causal:
                    CS = work.tile([128, 1024], F32, tag="CSchunk", name="CSchunk",
                                   bufs=2)
                    nc.scalar.dma_start(CS[:], causD[:, jc * 1024:(jc + 1) * 1024])
                    nc.gpsimd.tensor_tensor(out=E[:], in0=E[:], in1=CS[:], op=OP.add)
                nc.vector.tensor_tensor(
                    out=E3, in0=E3,
                    in1=M[:, jsl][:, :, None].broadcast_to([128, 16, 64]),
                    op=OP.subtract)
                nc.scalar.activation(E[:], E[:], ACTF.Exp)
                nc.vector.tensor_reduce(out=Zrec[:, jsl], in_=E3, axis=AX.X,
                                        op=OP.add)
                nc.vector.reciprocal(Zrec[:, jsl], Zrec[:, jsl])
                nc.gpsimd.tensor_tensor(
                    out=E3, in0=E3,
                    in1=Zrec[:, jsl][:, :, None].broadcast_to([128, 16, 64]),
                    op=OP.mult)
                for jb in range(0, 16, 4):
                    ps = psB.tile([64, 512], F32, tag="psb", name="psb")
                    for q in range(4):
                        nc.tensor.transpose(
                            ps[:, q * 128:(q + 1) * 128],
                            E[:, (jb + q) * 64:(jb + q + 1) * 64], I128[:])
                    copy_ps(PT[:, (jc * 16 + jb) * 128:(jc * 16 + jb + 4) * 128],
                            ps[:])

            # PV for this parity: half-banks [64, 512], pairs (h, q=b)
            for b in range(RT):
                vt = work.tile([64, D], F32, tag="Vload", name="Vload")
                nc.scalar.dma_start(vt[:], vD[(2 * b + p) * 64:(2 * b + p + 1) * 64, :])
                bank = psA.tile([64, 512], F32, tag="psa", name="psa")
                for h in range(NH):
                    pr = h * 16 + b
                    nc.tensor.matmul(
                        bank[:, h * 64:(h + 1) * 64],
                        lhsT=PT4[:, :, pr],
                        rhs=vt[:, h * 64:(h + 1) * 64],
                        start=True, stop=True)
                stag = work.tile([64, 512], F32, tag="stag", name="stag")
                copy_ps(stag[:], bank[:])
                for h in range(NH):
                    base = (2 * b + p) * 64 + h * 8
                    nc.sync.dma_start(
                        aD[base:base + 8, :],
                        stag[:, h * 64:(h + 1) * 64])

    # ---------- residual + LN from aD -------------------------------------
    def resid_ln(other_nat_cb, out_cb):
        def pre_fn(rt):
            at = work.tile([128, D], F32, tag="aload", name="aload")
            nc.sync.dma_start(at[:], aD[rt * 128:(rt + 1) * 128, :])
            pt = preQ.tile([128, D], F32, tag="pre", name="pre")
            nc.vector.tensor_tensor(out=pt[:], in0=at[:], in1=other_nat_cb(rt),
                                    op=OP.add)
            return pt[:]
        for g in range(RT // 4):
            ln_group4(g, pre_fn, out_cb)

    def ln_out_to_TD(dst_dram, also_nat_dram=None):
        """LN out_cb that immediately transposes each tile into dst_dram."""
        def cb(rt, src, negmu, rstd):
            ot = work.tile([128, D], F32, tag="lnout", name="lnout", bufs=4)
            nc.vector.tensor_scalar(out=ot[:], in0=src, scalar1=negmu,
                                    scalar2=rstd, op0=OP.add, op1=OP.mult)
            if also_nat_dram is not None:
                nc.sync.dma_start(also_nat_dram[rt * 128:(rt + 1) * 128, :], ot[:])
            ps = psB.tile([128, 512], F32, tag="psb", name="psb")
            for cb_ in range(4):
                nc.tensor.transpose(ps[:, cb_ * 128:(cb_ + 1) * 128],
                                    ot[:, cb_ * 128:(cb_ + 1) * 128], I128[:])
            t = work.tile([128, 512], F32, tag="toD", name="toD", bufs=2)
            copy_ps(t[:], ps[:])
            nc.sync.dma_start(
                dst_dram[:, :, rt * 128:(rt + 1) * 128].rearrange("c a r -> a c r"),
                t[:].rearrange("a (c r) -> a c r", c=4))
        return cb

    # ---------- FFN ---------------------------------------------------------
    def ffn(xTd, resTd, w1_ap, b1_ap, w2_ap, b2_ap, out_cb):
        b2 = small.tile([1, D], F32, tag="b2", name="b2")
        nc.sync.dma_start(b2[:], b2_ap[:])
        for rc in range(4):
            xcs = []
            for dt in range(DT):
                xc = work.tile([128, 512], F32, tag=f"xfc{dt}", name=f"xfc{dt}",
                               bufs=1)
                nc.sync.dma_start(xc[:], xTd[dt, :, rc * 512:(rc + 1) * 512])
                xcs.append(xc)
            ps2 = [psB.tile([128, 512], F32, tag="psb", name="psb")
                   for _ in range(4)]
            for ff in range(FT):
                w1f = work.tile([128, 512], F32, tag="w1f", name="w1f")
                nc.scalar.dma_start(
                    w1f[:].rearrange("a (d c) -> a d c", d=4),
                    w1_ap[:, ff * 128:(ff + 1) * 128]
                        .rearrange("(d a) c -> a d c", d=4))
                b1f = small.tile([1, 128], F32, tag="b1f", name="b1f", bufs=3)
                nc.sync.dma_start(b1f[:], b1_ap[:, ff * 128:(ff + 1) * 128])
                ps1 = psA.tile([128, 512], F32, tag="psa", name="psa")
                for dt in range(DT):
                    nc.tensor.matmul(ps1[:],
                                     lhsT=w1f[:, dt * 128:(dt + 1) * 128],
                                     rhs=xcs[dt][:], start=(dt == 0), stop=False)
                nc.tensor.matmul(ps1[:], lhsT=b1f[:], rhs=ones1[:, 0:512],
                                 start=False, stop=True)
                f1f = work.tile([128, 512], F32, tag="f1f", name="f1f")
                nc.scalar.activation(f1f[:], ps1[:], ACTF.Relu)
                w2f = work.tile([128, 512], F32, tag="w2f", name="w2f")
                nc.sync.dma_start(w2f[:], w2_ap[ff * 128:(ff + 1) * 128, :])
                for rl in range(4):
                    nc.tensor.matmul(ps2[rl][:],
                                     lhsT=f1f[:, rl * 128:(rl + 1) * 128],
                                     rhs=w2f[:], start=(ff == 0), stop=False)
            def pre_fn(rt):
                rl = rt % 4
                nc.tensor.matmul(ps2[rl][:], lhsT=ones1[:, 0:128], rhs=b2[:],
                                 start=False, stop=False)
                for ct in range(DT):
                    rtl = work.tile([128, 128], F32, tag="rload", name="rload",
                                    bufs=4)
                    nc.scalar.dma_start(rtl[:], resTd[ct, :, rt * 128:(rt + 1) * 128])
                    nc.tensor.matmul(ps2[rl][:, ct * 128:(ct + 1) * 128],
                                     lhsT=rtl[:], rhs=I128[:], start=False,
                                     stop=(ct == DT - 1))
                pt = preQ.tile([128, D], F32, tag="pre", name="pre")
                copy_ps(pt[:], ps2[rl][:])
                return pt[:]
            ln_group4(rc, pre_fn, out_cb)

    # ======================= pipeline =======================
    # P1: dec1 (causal) on x_de
    embed_T_toD(hi['XdT'], xTd['xd'])
    attention(xTd['xd'], xTd['xd'], gw['dec_wv1'][:], gw['dec_wqk1'][:],
              [hi['dec1_A'][p] for p in range(2)], hi['dec1_t'], hi['dec1_cs'],
              True)
    resid_ln(lambda rt: embed_nat_ps(hi['XdT'], rt)[:],
             ln_out_to_TD(xTd['m'], also_nat_dram=mnD))

    # P2: encoder self-attn on x_en
    embed_T_toD(hi['XeT'], xTd['xe'])
    attention(xTd['xe'], xTd['xe'], gw['enc_wv'][:], gw['enc_wqk'][:],
              [hi['enc_A'][p] for p in range(2)], hi['enc_t'], hi['enc_cs'],
              False)
    resid_ln(lambda rt: embed_nat_ps(hi['XeT'], rt)[:], ln_out_to_TD(xTd['o1']))

    # P3: encoder FFN
    ffn(xTd['o1'], xTd['o1'], gw['enc_w1'][:], hi['enc_b1'], gw['enc_w2'][:],
        hi['enc_b2'], ln_out_to_TD(xTd['eo']))

    # P4: dec2 cross-attn
    attention(xTd['m'], xTd['eo'], gw['dec_wv2'][:], gw['dec_wqk2'][:],
              [hi['dec2_A'][p] for p in range(2)], hi['dec2_t'], hi['dec2_cs'],
              False)

    def m_reload(rt):
        t = work.tile([128, D], F32, tag="mload", name="mload", bufs=2)
        nc.sync.dma_start(t[:], mnD[rt * 128:(rt + 1) * 128, :])
        return t[:]
    resid_ln(m_reload, ln_out_to_TD(xTd['c']))

    # P5: decoder FFN
    ffn(xTd['c'], xTd['c'], gw['dec_w1'][:], hi['dec_b1'], gw['dec_w2'][:],
        hi['dec_b2'], ln_out_to_TD(xTd['of']))

    # P6: final projection + softmax (output ships as bf16)
    Wo = wpool.tile([128, 4 * 64], F32, tag="Wo", name="Wo")
    for dt in range(DT):
        nc.sync.dma_start(Wo[:, dt * 64:(dt + 1) * 64],
                          gw['W_out'][dt * 128:(dt + 1) * 128, :])
    Bo = small.tile([1, 64], F32, tag="Bo", name="Bo")
    nc.sync.dma_start(Bo[:], hi['B_out'][:])
    for rt in range(RT):
        ps = psB.tile([128, 64], F32, tag="psbq", name="psbo", bufs=1)
        for dt in range(DT):
            ol = work.tile([128, 128], F32, tag="rload", name="rload", bufs=4)
            nc.sync.dma_start(ol[:], xTd['of'][dt, :, rt * 128:(rt + 1) * 128])
            nc.tensor.matmul(ps[:], lhsT=ol[:], rhs=Wo[:, dt * 64:(dt + 1) * 64],
                             start=(dt == 0), stop=False)
        nc.tensor.matmul(ps[:], lhsT=ones1[:, 0:128], rhs=Bo[:],
                         start=False, stop=True)
        mx = small.tile([128, 1], F32, tag="mx", name="mx")
        nc.vector.tensor_reduce(out=mx[:], in_=ps[:], axis=AX.X, op=OP.max,
                                negate=True)
        ex = work.tile([128, 64], F32, tag="ex", name="ex")
        nc.scalar.activation(ex[:], ps[:], ACTF.Exp, bias=mx[:])
        zs = small.tile([128, 1], F32, tag="zs", name="zs")
        nc.vector.tensor_reduce(out=zs[:], in_=ex[:], axis=AX.X, op=OP.add)
        rz = small.tile([128, 1], F32, tag="rz", name="rz")
        nc.vector.reciprocal(rz[:], zs[:])
        oo = work.tile([128, 64], F32, tag="oo", name="oo")
        nc.vector.tensor_scalar(out=oo[:], in0=ex[:], scalar1=rz[:],
                                scalar2=None, op0=OP.mult)
        oo16 = work.tile([128, 64], F16, tag="oo16", name="oo16")
        nc.vector.tensor_copy(oo16[:], oo[:])
        nc.sync.dma_start(out_ap[rt * 128:(rt + 1) * 128, :], oo16[:])


# ============================================================================
# 8-core SPMD wrapper with a cached PJRT dispatcher: kernel(**inputs) -> out
# ============================================================================
_CACHE = {}


def _get_program():
    if 'nc' not in _CACHE:
        nc = bacc.Bacc("TRN2", target_bir_lowering=False, debug=False)
        hi, out_ap = declare_io(nc)
        with tile.TileContext(nc, trace_sim=False) as tc:
            with ExitStack() as ctx:
                build(ctx, tc, hi, out_ap)
        nc.compile()
        _CACHE['nc'] = nc
    return _CACHE['nc']


def _get_dispatcher():
    """One cached jit(shard_map(...)) wrapper -- same execution path as
    bass_utils.run_bass_kernel_spmd under axon (bass2jax/_bass_exec_p via
    PJRT), but without rebuilding/retracing the wrapper on every call."""
    if 'disp' in _CACHE:
        return _CACHE['disp']
    import jax
    from jax.sharding import Mesh, PartitionSpec
    from jax.experimental.shard_map import shard_map
    from concourse import bass2jax

    nc = _get_program()
    bass2jax.install_neuronx_cc_hook()
    partition_name = (nc.partition_id_tensor.name
                      if nc.partition_id_tensor else None)
    in_names, out_names, out_avals, zero_tmpl = [], [], [], []
    for alloc in nc.m.functions[0].allocations:
        if not isinstance(alloc, mybir.MemoryLocationSet):
            continue
        name = alloc.memorylocations[0].name
        if alloc.kind == "ExternalInput":
            if name != partition_name:
                in_names.append(name)
        elif alloc.kind == "ExternalOutput":
            shape = tuple(alloc.tensor_shape)
            dtype = mybir.dt.np(alloc.dtype)
            out_avals.append(jax.core.ShapedArray(shape, dtype))
            zero_tmpl.append((shape, dtype))
            out_names.append(name)
    n_params = len(in_names)
    n_outs = len(out_avals)
    all_in_names = list(in_names) + list(out_names)
    if partition_name is not None:
        all_in_names.append(partition_name)
    donate = tuple(range(n_params, n_params + n_outs))

    def _body(*args):
        operands = list(args)
        if partition_name is not None:
            operands.append(bass2jax.partition_id_tensor())
        outs = bass2jax._bass_exec_p.bind(
            *operands, out_avals=tuple(out_avals),
            in_names=tuple(all_in_names), out_names=tuple(out_names),
            lowering_input_output_aliases=(), sim_require_finite=True,
            sim_require_nnan=True, nc=nc)
        return tuple(outs)

    devices = jax.devices()[:8]
    mesh = Mesh(np.asarray(devices), ("core",))
    sharded = jax.jit(
        shard_map(_body, mesh=mesh,
                  in_specs=(PartitionSpec("core"),) * (n_params + n_outs),
                  out_specs=(PartitionSpec("core"),) * n_outs,
                  check_rep=False),
        donate_argnums=donate, keep_unused=True)

    def dispatch(in_maps):
        concat_in = [
            np.concatenate([np.asarray(in_maps[c][nm]) for c in range(8)], 0)
            for nm in in_names]
        cz = [np.zeros((8 * s[0], *s[1:]), d) for (s, d) in zero_tmpl]
        outs = sharded(*concat_in, *cz)
        return [
            {nm: np.asarray(outs[i]).reshape(8, *out_avals[i].shape)[c]
             for i, nm in enumerate(out_names)}
            for c in range(8)]

    _CACHE['disp'] = dispatch
    return dispatch


def kernel(**inputs):
    dispatch = _get_dispatcher()
    in_maps = [host_inputs(inputs, core) for core in range(8)]
    res = dispatch(in_maps)
    outs = [np.asarray(res[c]['out'], np.float32) for c in range(8)]
    full = np.concatenate(outs, 0)          # [16384, 64] rows = (b, L)
    return full.reshape(64, 256, 64)


# revision 20
# speedup vs baseline: 7.5123x; 1.1006x over previous
"""Bass/Tile kernel for nn_DeepRelativeST on 8 NeuronCores (1/8 data-parallel
shard over the flat (b*L) row axis; 8 batches = 32 contiguous l-blocks per
core, so attention is core-local).

Per-core: R=2048 rows (8 batches x 256 pos), D=512, DFF=2048, H=8, dep=64,
Ll=32 local l values, 256 (l,h) softmax pairs split into two l-parity tiles:
tile p holds pair (h, l=2q+p) at partition h*16+q.

Key math (derived from reference.py):
  qs[l,h,j] = (x @ wq_headsum)[l*64+j, h]     (full Q GEMM never needed)
  ks likewise; V = x @ wv (full GEMM).
  abar[l,h,k,m] = rel[l,h,k,m-k+63] * (m<=k)  (host-gathered skew)
  r1 = sum_m abar*ks ; t = sum_m abar*m (HOST precomputed from rel)
  r2 = r1 + NEG*t ; cu = sc^2 * R1 * qs with R1 = sum_m colsum[m]*ks[m]
  (colsum = sum_k abar[.,k,.] HOST precomputed: keeps cu exact so abar can
   ship as fp8 -- r1's precision only affects softmax temperature-negligible
   terms; validated to 5.6e-5 max rel err on the host mirror.)
  logits[j,k] = cu[j]*r2[k] (+ causal NEG mask)
  p = softmax_k ; o = p @ V-block
  out row = l*64 + h*8 + j//8, col = (j%8)*64 + n   (torch raw-reshape scramble)

Transfer plan (the dispatch wall-clock is dominated by the ~35 MB/s axon
tunnel): abar ships as fp8e4 (1/4 bytes); all replicated weights ship as 1/8
row-shards and are AllGathered on-device (HBM Shared scratch); the causal
mask is built on device from a [1,4096] row; output returns as bf16.
"""
import numpy as np
from contextlib import ExitStack

import ml_dtypes

import concourse.bass as bass
import concourse.tile as tile
from concourse import bacc
from concourse import mybir

F32 = mybir.dt.float32
FP8 = mybir.dt.float8e4
F16 = mybir.dt.float16
BF16 = mybir.dt.bfloat16
AX = mybir.AxisListType
OP = mybir.AluOpType
ACTF = mybir.ActivationFunctionType

R, D, DFF, NH, DEP, LL = 2048, 512, 2048, 8, 64, 32
NEG, EPS, SC2 = -1e9, 1e-5, 1.0 / 64.0
RT, DT, FT = R // 128, D // 128, DFF // 128
NC8 = [[0, 1, 2, 3, 4, 5, 6, 7]]

# replicated weights: name -> full (rows, cols); shipped as [rows//8, cols]
# REPW32: fp32 (attention-selection critical -- qs/ks path must be exact).
# REPW16: fp16 on the wire, upcast to fp32 on device (FFN/out path; validated
# to ~2e-4 host-side).
REPW32 = {
    'W_in': (64, 512),
    'enc_wv': (512, 512), 'dec_wv1': (512, 512),
    'enc_wqk': (512, 16), 'dec_wqk1': (512, 16), 'dec_wqk2': (512, 16),
    'I128': (128, 128),
}
REPW16 = {
    'enc_w1': (512, 2048), 'enc_w2': (2048, 512),
    'dec_w1': (512, 2048), 'dec_w2': (2048, 512),
    'W_out': (512, 64), 'dec_wv2': (512, 512),
}
# A ships fp8, triangle-packed into 8 row-segments of 8 k's, each padded to
# width 8*(s+1): row k in segment s=k//8 keeps columns m=0..8(s+1)-1
# (superset of the valid m<=k). 2304 bytes/partition vs 4096 dense.
NSEG = 8
SEG_OFF = [32 * s * (s + 1) for s in range(NSEG)]
APK = 2304


def host_inputs(inp, core):
    f = lambda k: np.ascontiguousarray(np.asarray(inp[k], np.float32))
    bs = slice(core * 8, core * 8 + 8)
    ls = slice(core * 32, core * 32 + 32)
    Xe = f('X_en')[bs].reshape(R, 64)
    Xd = f('X_de')[bs].reshape(R, 64)

    def wqk_heads(wq, wk):
        a = wq.reshape(D, NH, DEP).sum(-1)
        b = wk.reshape(D, NH, DEP).sum(-1)
        return np.ascontiguousarray(np.concatenate([a, b], 1))  # [512,16]

    km = np.arange(64)
    kk, mm = np.meshgrid(km, km, indexing='ij')   # [k, m]

    def rel_arrange(rel):
        r = rel[ls]                                # [32,8,64,64] = [l,h,k,c]
        # abar[l,h,k,m] = r[l,h,k,m-k+63] if m<=k else 0
        c = mm - kk + 63
        valid = (mm <= kk)
        cs = np.clip(c, 0, 63)
        ab = np.take_along_axis(
            r.reshape(LL, NH, 64, 64), cs.reshape(1, 1, 64, 64), axis=3)
        ab = ab * valid.reshape(1, 1, 64, 64)
        t = (ab * mm.reshape(1, 1, 64, 64)).sum(-1)     # [l,h,k]   exact
        csum = ab.sum(-2)                               # [l,h,m]   exact
        abT = ab.transpose(1, 0, 2, 3)                  # [h,l,k,m]
        tT = t.transpose(1, 0, 2)                       # [h,l,k]
        cT = csum.transpose(1, 0, 2)                    # [h,l,m]
        A8 = np.zeros((2, 128, APK), ml_dtypes.float8_e4m3)
        Tt = np.empty((2, 128, 64), np.float32)
        Cs = np.empty((2, 128, 64), np.float32)
        for p in range(2):
            d = abT[:, p::2].reshape(128, 64, 64)          # [a, k, m]
            for s in range(NSEG):
                ws = 8 * (s + 1)
                seg = d[:, 8 * s:8 * (s + 1), 0:ws].reshape(128, 8 * ws)
                A8[p][:, SEG_OFF[s]:SEG_OFF[s] + 8 * ws] = \
                    seg.astype(ml_dtypes.float8_e4m3)
            Tt[p] = tT[:, p::2].reshape(128, 64)
            Cs[p] = cT[:, p::2].reshape(128, 64)
        return A8, Tt, Cs

    A_e, t_e, c_e = rel_arrange(f('enc_rel'))
    A_d1, t_d1, c_d1 = rel_arrange(f('dec_rel1'))
    A_d2, t_d2, c_d2 = rel_arrange(f('dec_rel2'))
    caus_row = np.ascontiguousarray(
        np.triu(np.full((64, 64), NEG, np.float32), 1).reshape(1, 4096))

    out = {
        'XeT': np.ascontiguousarray(Xe.T), 'XdT': np.ascontiguousarray(Xd.T),
        'B_in': f('B_in').reshape(1, D),
        'enc_A': A_e, 'enc_t': t_e, 'enc_cs': c_e,
        'dec1_A': A_d1, 'dec1_t': t_d1, 'dec1_cs': c_d1,
        'dec2_A': A_d2, 'dec2_t': t_d2, 'dec2_cs': c_d2,
        'enc_b1': f('enc_b1').reshape(1, DFF), 'enc_b2': f('enc_b2').reshape(1, D),
        'dec_b1': f('dec_b1').reshape(1, DFF), 'dec_b2': f('dec_b2').reshape(1, D),
        'B_out': f('B_out').reshape(1, 64),
        'caus_row': caus_row,
    }
    fulls = {
        'W_in': f('W_in'),
        'enc_wv': f('enc_wv'), 'dec_wv1': f('dec_wv1'), 'dec_wv2': f('dec_wv2'),
        'enc_wqk': wqk_heads(f('enc_wq'), f('enc_wk')),
        'dec_wqk1': wqk_heads(f('dec_wq1'), f('dec_wk1')),
        'dec_wqk2': wqk_heads(f('dec_wq2'), f('dec_wk2')),
        'I128': np.eye(128, dtype=np.float32),
    }
    for nm, (r, c) in REPW32.items():
        sh = r // 8
        out[nm] = np.ascontiguousarray(fulls[nm][core * sh:(core + 1) * sh])
    for nm, (r, c) in REPW16.items():
        sh = r // 8
        out[nm] = np.ascontiguousarray(
            f(nm)[core * sh:(core + 1) * sh].astype(np.float16))
    return out


IN_SHAPES = {
    'XeT': ((64, R), F32), 'XdT': ((64, R), F32), 'B_in': ((1, D), F32),
    'enc_A': ((2, 128, APK), FP8), 'dec1_A': ((2, 128, APK), FP8),
    'dec2_A': ((2, 128, APK), FP8),
    'enc_t': ((2, 128, 64), F32), 'dec1_t': ((2, 128, 64), F32),
    'dec2_t': ((2, 128, 64), F32),
    'enc_cs': ((2, 128, 64), F32), 'dec1_cs': ((2, 128, 64), F32),
    'dec2_cs': ((2, 128, 64), F32),
    'enc_b1': ((1, DFF), F32), 'enc_b2': ((1, D), F32),
    'dec_b1': ((1, DFF), F32), 'dec_b2': ((1, D), F32),
    'B_out': ((1, 64), F32), 'caus_row': ((1, 4096), F32),
    **{nm: ((r // 8, c), F32) for nm, (r, c) in REPW32.items()},
    **{nm: ((r // 8, c), F16) for nm, (r, c) in REPW16.items()},
}


def declare_io(nc):
    hi = {k: nc.dram_tensor(k, list(s), dt, kind="ExternalInput").ap()
          for k, (s, dt) in IN_SHAPES.items()}
    out = nc.dram_tensor('out', [R, 64], F16, kind="ExternalOutput").ap()
    return hi, out


def build(ctx: ExitStack, tc: tile.TileContext, hi, out_ap, dbg=None):
    nc = tc.nc
    consts = ctx.enter_context(tc.tile_pool(name="consts", bufs=1))
    wpool = ctx.enter_context(tc.tile_pool(name="wpool", bufs=1))
    work = ctx.enter_context(tc.tile_pool(name="work", bufs=3))
    preQ = ctx.enter_context(tc.tile_pool(name="preQ", bufs=8))
    small = ctx.enter_context(tc.tile_pool(name="small", bufs=1))
    bigP = ctx.enter_context(tc.tile_pool(name="bigP", bufs=1))
    psA = ctx.enter_context(tc.tile_pool(name="psA", bufs=3, space="PSUM"))
    psB = ctx.enter_context(tc.tile_pool(name="psB", bufs=4, space="PSUM"))
    dram = ctx.enter_context(tc.tile_pool(name="dram", bufs=1, space="DRAM"))

    # ---------- gather replicated weights from 1/8 shards -------------------
    gw = {}
    for nm, (r, c) in REPW32.items():
        loc = dram.tile([r // 8, c], F32, tag=f"agl_{nm}", name=f"agl_{nm}")
        nc.sync.dma_start(loc[:], hi[nm][:])
        full = dram.tile([r, c], F32, addr_space="Shared",
                         tag=f"agf_{nm}", name=f"agf_{nm}")
        nc.gpsimd.collective_compute(
            "AllGather", OP.bypass, replica_groups=NC8,
            ins=[loc[:]], outs=[full[:]])
        gw[nm] = full
    for nm, (r, c) in REPW16.items():
        loc = dram.tile([r // 8, c], F16, tag=f"agl_{nm}", name=f"agl_{nm}")
        nc.sync.dma_start(loc[:], hi[nm][:])
        full16 = dram.tile([r, c], F16, addr_space="Shared",
                           tag=f"agh_{nm}", name=f"agh_{nm}")
        nc.gpsimd.collective_compute(
            "AllGather", OP.bypass, replica_groups=NC8,
            ins=[loc[:]], outs=[full16[:]])
        full = dram.tile([r, c], F32, tag=f"agf_{nm}", name=f"agf_{nm}")
        for r0 in range(0, r, 128):
            for c0 in range(0, c, 512):
                cw = min(512, c - c0)
                t16 = work.tile([128, 512], F16, tag="u16", name="u16", bufs=2)
                nc.sync.dma_start(t16[:, 0:cw],
                                  full16[r0:r0 + 128, c0:c0 + cw])
                t32 = work.tile([128, 512], F32, tag="xcT", name="u32")
                nc.vector.tensor_copy(t32[:, 0:cw], t16[:, 0:cw])
                nc.sync.dma_start(full[r0:r0 + 128, c0:c0 + cw], t32[:, 0:cw])
        gw[nm] = full

    I128 = consts.tile([128, 128], F32, tag="I128", name="I128")
    nc.sync.dma_start(I128[:], gw['I128'][:])
    ones1 = consts.tile([1, D], F32, tag="ones1", name="ones1")
    nc.vector.memset(ones1[:], 1.0)
    epsc = consts.tile([128, 1], F32, tag="epsc", name="epsc")
    nc.vector.memset(epsc[:], EPS)
    W_in = consts.tile([64, D], F32, tag="W_in", name="W_in")
    nc.sync.dma_start(W_in[:], gw['W_in'][:])
    B_in = consts.tile([1, D], F32, tag="B_in", name="B_in")
    nc.sync.dma_start(B_in[:], hi['B_in'][:])

    # causal mask [128, 4096] built on device from the [1,4096] row into
    # DRAM scratch (PE partition-broadcast), streamed back at use like the
    # baseline's shipped CAUS.
    causD = dram.tile([128, 4096], F32, tag="causD", name="causD")
    for q in range(8):
        cr = work.tile([1, 512], F32, tag="xin", name="crowc")
        nc.sync.dma_start(cr[:], hi['caus_row'][:, q * 512:(q + 1) * 512])
        ps = psA.tile([128, 512], F32, tag="psa", name="psa")
        nc.tensor.matmul(ps[:], lhsT=ones1[:, 0:128], rhs=cr[:],
                         start=True, stop=True)
        st = work.tile([128, 512], F32, tag="toD", name="toD", bufs=2)
        nc.scalar.copy(st[:], ps[:])
        nc.sync.dma_start(causD[:, q * 512:(q + 1) * 512], st[:])

    # DRAM scratch: transposed activations live here, streamed at use.
    xTd = {nm: dram.tile([DT, 128, R], F32, tag=f"xTd_{nm}", name=f"xTd_{nm}")
           for nm in ('xe', 'xd', 'm', 'o1', 'eo', 'c', 'of')}
    aD = dram.tile([R, D], F32, tag="aD", name="aD")
    vD = dram.tile([R, D], F32, tag="vD", name="vD")
    mnD = dram.tile([R, D], F32, tag="mnD", name="mnD")

    def copy_ps(dst, src):
        nc.scalar.copy(dst, src)

    # ---------- embed: x.T = (X@W_in+B).T streamed to DRAM ------------------
    def embed_T_toD(x_in_ap, dst):
        for ct in range(DT):
            for rc in range(4):
                xin = work.tile([64, 512], F32, tag="xin", name="xin")
                nc.sync.dma_start(xin[:], x_in_ap[:, rc * 512:(rc + 1) * 512])
                ps = psA.tile([128, 512], F32, tag="psa", name="psa")
                nc.tensor.matmul(ps[:], lhsT=W_in[:, ct * 128:(ct + 1) * 128],
                                 rhs=xin[:], start=True, stop=False)
                nc.tensor.matmul(ps[:], lhsT=B_in[:, ct * 128:(ct + 1) * 128],
                                 rhs=ones1[:, 0:512], start=False, stop=True)
                t = work.tile([128, 512], F32, tag="toD", name="toD", bufs=2)
                copy_ps(t[:], ps[:])
                nc.sync.dma_start(dst[ct, :, rc * 512:(rc + 1) * 512], t[:])

    def embed_nat_ps(x_in_ap, rt):
        xin = work.tile([64, 128], F32, tag="xin2", name="xin2")
        nc.sync.dma_start(xin[:], x_in_ap[:, rt * 128:(rt + 1) * 128])
        ps = psA.tile([128, 512], F32, tag="psa", name="psa")
        nc.tensor.matmul(ps[:], lhsT=xin[:], rhs=W_in[:], start=True, stop=False)
        nc.tensor.matmul(ps[:], lhsT=ones1[:, 0:128], rhs=B_in[:],
                         start=False, stop=True)
        return ps

    # ---------- layernorm over one group of 4 row-tiles ---------------------
    def ln_group4(g, pre_fn, out_cb):
        """pre_fn(rt) -> [128,512] AP (lazy); out_cb(rt, src, nmu, rstd)."""
        if True:
            sx = small.tile([128, 4], F32, tag="sx", name="sx", bufs=2)
            sx2 = small.tile([128, 4], F32, tag="sx2", name="sx2", bufs=2)
            pres = []
            for i in range(4):
                pa = pre_fn(g * 4 + i)
                pres.append(pa)
                scr = work.tile([128, D], F32, tag="lnscr", name="lnscr")
                nc.scalar.activation(scr[:], pa, ACTF.Copy,
                                     accum_out=sx[:, i:i + 1])
                nc.scalar.activation(scr[:], pa, ACTF.Square,
                                     accum_out=sx2[:, i:i + 1])
            negmu = small.tile([128, 4], F32, tag="negmu", name="negmu", bufs=2)
            nc.vector.tensor_scalar(out=negmu[:], in0=sx[:], scalar1=-1.0 / D,
                                    scalar2=None, op0=OP.mult)
            mu2 = small.tile([128, 4], F32, tag="mu2", name="mu2", bufs=2)
            nc.vector.tensor_tensor(out=mu2[:], in0=negmu[:], in1=negmu[:],
                                    op=OP.mult)
            var = small.tile([128, 4], F32, tag="var", name="var", bufs=2)
            nc.vector.scalar_tensor_tensor(out=var[:], in0=sx2[:],
                                           scalar=1.0 / D, in1=mu2[:],
                                           op0=OP.mult, op1=OP.subtract)
            std = small.tile([128, 4], F32, tag="std", name="std", bufs=2)
            nc.scalar.activation(std[:], var[:], ACTF.Sqrt, bias=epsc[:])
            rstd = small.tile([128, 4], F32, tag="rstd", name="rstd", bufs=2)
            nc.vector.reciprocal(rstd[:], std[:])
            for i in range(4):
                out_cb(g * 4 + i, pres[i], negmu[:, i:i + 1], rstd[:, i:i + 1])

    # ---------- attention ---------------------------------------------------
    def attention(xqTd, xkvTd, wv_ap, wqk_ap, A_ap, t_ap, cs_ap, causal):
        # V GEMM (x.T-stationary tiles streamed from DRAM) -> vD
        wv = wpool.tile([128, 4 * D], F32, tag="wv", name="wv")
        for dt in range(DT):
            nc.sync.dma_start(wv[:, dt * D:(dt + 1) * D],
                              wv_ap[dt * 128:(dt + 1) * 128, :])
        for rt in range(RT):
            ps = psA.tile([128, 512], F32, tag="psa", name="psa")
            for dt in range(DT):
                xl = work.tile([128, 128], F32, tag="xlT", name="xlT")
                nc.sync.dma_start(xl[:], xkvTd[dt, :, rt * 128:(rt + 1) * 128])
                nc.tensor.matmul(ps[:], lhsT=xl[:],
                                 rhs=wv[:, dt * D:(dt + 1) * D],
                                 start=(dt == 0), stop=(dt == DT - 1))
            vt = work.tile([128, D], F32, tag="Vtile", name="Vtile")
            copy_ps(vt[:], ps[:])
            nc.sync.dma_start(vD[rt * 128:(rt + 1) * 128, :], vt[:])

        # qs / ks GEMMs (W-stationary, M=8)
        wqk = wpool.tile([128, 4 * 16], F32, tag="wqk", name="wqk")
        for dt in range(DT):
            nc.sync.dma_start(wqk[:, dt * 16:(dt + 1) * 16],
                              wqk_ap[dt * 128:(dt + 1) * 128, :])
        qT = work.tile([8, R], F32, tag="qT", name="qT", bufs=1)
        kT = work.tile([8, R], F32, tag="kT", name="kT", bufs=1)
        for (dst, colofs, srcTd) in ((qT, 0, xqTd), (kT, 8, xkvTd)):
            for rc in range(4):
                ps = psB.tile([8, 512], F32, tag="psbq", name="psbq", bufs=1)
                for dt in range(DT):
                    xc = work.tile([128, 512], F32, tag="xcT", name="xcT")
                    nc.sync.dma_start(xc[:], srcTd[dt, :, rc * 512:(rc + 1) * 512])
                    nc.tensor.matmul(
                        ps[:], lhsT=wqk[:, dt * 16 + colofs: dt * 16 + colofs + 8],
                        rhs=xc[:], start=(dt == 0), stop=(dt == DT - 1))
                copy_ps(dst[:, rc * 512:(rc + 1) * 512], ps[:])

        qs_pp = small.tile([128, 2 * 64], F32, tag="qs_pp", name="qs_pp")
        ks_pp = small.tile([128, 2 * 64], F32, tag="ks_pp", name="ks_pp")
        qD = dram.tile([8, R], F32, tag="qD", name="qD")
        kD = dram.tile([8, R], F32, tag="kD", name="kD")
        for (src, bounce, dst) in ((qT, qD, qs_pp), (kT, kD, ks_pp)):
            nc.sync.dma_start(bounce[:], src[:])
            nc.sync.dma_start(
                dst[:], bounce[:].rearrange("h (q f) -> (h q) f", q=16))

        # r1 = sum_m abar*ks (abar arrives fp8, triangle-packed in 8 segments
        # of 8 k-rows padded to width 8(s+1); upcast then mult-reduce)
        r1 = small.tile([128, 2 * 64], F32, tag="r1", name="r1")
        for p in range(2):
            for s in range(NSEG):
                ws = 8 * (s + 1)
                width = 8 * ws
                off = SEG_OFF[s]
                A8t = work.tile([128, 512], FP8, tag="A8chunk", name="A8chunk",
                                bufs=1)
                nc.scalar.dma_start(A8t[:, 0:width], A_ap[p][:, off:off + width])
                A = work.tile([128, 512], F32, tag="Achunk", name="Achunk", bufs=1)
                nc.vector.tensor_copy(A[:, 0:width], A8t[:, 0:width])
                A3 = A[:, 0:width].rearrange("a (k m) -> a k m", k=8)
                nc.gpsimd.tensor_tensor(
                    out=A3, in0=A3,
                    in1=ks_pp[:, p * 64:p * 64 + ws][:, None, :]
                        .broadcast_to([128, 8, ws]), op=OP.mult)
                nc.vector.tensor_reduce(
                    out=r1[:, p * 64 + s * 8: p * 64 + (s + 1) * 8],
                    in_=A3, axis=AX.X, op=OP.add)
        tH = small.tile([128, 2 * 64], F32, tag="tH", name="tH")
        nc.sync.dma_start(tH[:].rearrange("a (p k) -> a p k", p=2),
                          t_ap[:].rearrange("p a k -> a p k"))
        r2 = small.tile([128, 2 * 64], F32, tag="r2", name="r2")
        nc.vector.scalar_tensor_tensor(out=r2[:], in0=tH[:], scalar=NEG,
                                       in1=r1[:], op0=OP.mult, op1=OP.add)
        # R1 exact via host colsum: R1[p] = sum_m colsum[m]*ks[m]
        csH = small.tile([128, 2 * 64], F32, tag="csH", name="csH")
        nc.sync.dma_start(csH[:].rearrange("a (p k) -> a p k", p=2),
                          cs_ap[:].rearrange("p a k -> a p k"))
        csk = small.tile([128, 2 * 64], F32, tag="csk", name="csk")
        nc.vector.tensor_tensor(out=csk[:], in0=csH[:], in1=ks_pp[:], op=OP.mult)
        R1s = small.tile([128, 2], F32, tag="R1s", name="R1s")
        nc.vector.tensor_reduce(out=R1s[:],
                                in_=csk[:].rearrange("a (p k) -> a p k", p=2),
                                axis=AX.X, op=OP.add)
        nc.vector.tensor_scalar(out=R1s[:], in0=R1s[:], scalar1=SC2,
                                scalar2=None, op0=OP.mult)
        cu = small.tile([128, 2 * 64], F32, tag="cu", name="cu")
        for p in range(2):
            nc.vector.tensor_scalar(out=cu[:, p * 64:(p + 1) * 64],
                                    in0=qs_pp[:, p * 64:(p + 1) * 64],
                                    scalar1=R1s[:, p:p + 1], scalar2=None,
                                    op0=OP.mult)

        # M = rowmax of logits (rank-1 trick; scans for causal)
        M = small.tile([128, 2 * 64], F32, tag="Mm", name="Mm")
        t1 = small.tile([128, 64], F32, tag="Mt1", name="Mt1")
        t2 = small.tile([128, 64], F32, tag="Mt2", name="Mt2")
        if not causal:
            wmax = small.tile([128, 2], F32, tag="wmax", name="wmax")
            wmin = small.tile([128, 2], F32, tag="wmin", name="wmin")
            nc.vector.tensor_reduce(out=wmax[:],
                                    in_=r2[:].rearrange("a (p k) -> a p k", p=2),
                                    axis=AX.X, op=OP.max)
            nc.vector.tensor_reduce(out=wmin[:],
                                    in_=r2[:].rearrange("a (p k) -> a p k", p=2),
                                    axis=AX.X, op=OP.min)
            for p in range(2):
                sl = slice(p * 64, (p + 1) * 64)
                nc.vector.tensor_scalar(out=M[:, sl], in0=cu[:, sl],
                                        scalar1=wmax[:, p:p + 1], scalar2=None,
                                        op0=OP.mult)
                nc.vector.tensor_scalar(out=t1[:], in0=cu[:, sl],
                                        scalar1=wmin[:, p:p + 1], scalar2=None,
                                        op0=OP.mult)
                nc.vector.tensor_tensor(out=M[:, sl], in0=M[:, sl], in1=t1[:],
                                        op=OP.max)
        else:
            pm = small.tile([128, 128], F32, tag="pm", name="pm")
            pn = small.tile([128, 128], F32, tag="pn", name="pn")
            sm = small.tile([128, 128], F32, tag="sm", name="sm")
            sn = small.tile([128, 128], F32, tag="sn", name="sn")
            for p in range(2):
                sl = slice(p * 64, (p + 1) * 64)
                w_ = r2[:, sl]
                wr = r2[:, sl][:, ::-1]
                nc.vector.tensor_tensor_scan(out=pm[:, sl], data0=w_, data1=w_,
                                             initial=-3e38, op0=OP.max, op1=OP.bypass)
                nc.vector.tensor_tensor_scan(out=pn[:, sl], data0=w_, data1=w_,
                                             initial=3e38, op0=OP.min, op1=OP.bypass)
                nc.vector.tensor_tensor_scan(out=sm[:, sl][:, ::-1], data0=wr,
                                             data1=wr, initial=-3e38,
                                             op0=OP.max, op1=OP.bypass)
                nc.vector.tensor_tensor_scan(out=sn[:, sl][:, ::-1], data0=wr,
                                             data1=wr, initial=3e38,
                                             op0=OP.min, op1=OP.bypass)
            for p in range(2):
                sl = slice(p * 64, (p + 1) * 64)
                nc.vector.tensor_tensor(out=M[:, sl], in0=cu[:, sl],
                                        in1=pm[:, sl], op=OP.mult)
                nc.vector.tensor_tensor(out=t1[:], in0=cu[:, sl], in1=pn[:, sl],
                                        op=OP.mult)
                nc.vector.tensor_tensor(out=M[:, sl], in0=M[:, sl], in1=t1[:],
                                        op=OP.max)
                j63 = slice(p * 64, p * 64 + 63)
                cs = cu[:, j63]
                nc.vector.tensor_tensor(out=t1[:, 0:63], in0=cs,
                                        in1=sm[:, p * 64 + 1:(p + 1) * 64],
                                        op=OP.mult)
                nc.vector.tensor_tensor(out=t2[:, 0:63], in0=cs,
                                        in1=sn[:, p * 64 + 1:(p + 1) * 64],
                                        op=OP.mult)
                nc.vector.tensor_tensor(out=t1[:, 0:63], in0=t1[:, 0:63],
                                        in1=t2[:, 0:63], op=OP.max)
                nc.vector.tensor_scalar(out=t1[:, 0:63], in0=t1[:, 0:63],
                                        scalar1=NEG, scalar2=None, op0=OP.add)
                nc.vector.tensor_tensor(out=M[:, j63], in0=M[:, j63],
                                        in1=t1[:, 0:63], op=OP.max)

        # E chunks of 16 j: build/mask/-M/exp/Z/scale -> transpose to PT -> PV
        Zrec = small.tile([128, 2 * 64], F32, tag="Zrec", name="Zrec")
        for p in range(2):
            PT = bigP.tile([64, 64 * 128], F32, tag="PT", name="PT")
            PT4 = PT[:].rearrange("k (j pp) -> k j pp", j=64)
            for jc in range(4):
                jsl = slice(p * 64 + jc * 16, p * 64 + (jc + 1) * 16)
                E = work.tile([128, 1024], F32, tag="Echunk", name="Echunk", bufs=2)
                E3 = E[:].rearrange("a (j k) -> a j k", j=16)
                nc.vector.tensor_tensor(
                    out=E3, in0=cu[:, jsl][:, :, None].broadcast_to([128, 16, 64]),
                    in1=r2[:, p * 64:(p + 1) * 64][:, None, :]
                        .broadcast_to([128, 16, 64]), op=OP.mult)
                if causal:
                    CS = work.tile([128, 1024], F32, tag="CSchunk", name="CSchunk",
                                   bufs=2)
                    nc.scalar.dma_start(CS[:], causD[:, jc * 1024:(jc + 1) * 1024])
                    nc.gpsimd.tensor_tensor(out=E[:], in0=E[:], in1=CS[:], op=OP.add)
                nc.vector.tensor_tensor(
                    out=E3, in0=E3,
                    in1=M[:, jsl][:, :, None].broadcast_to([128, 16, 64]),
                    op=OP.subtract)
                nc.scalar.activation(E[:], E[:], ACTF.Exp)
                nc.vector.tensor_reduce(out=Zrec[:, jsl], in_=E3, axis=AX.X,
                                        op=OP.add)
                nc.vector.reciprocal(Zrec[:, jsl], Zrec[:, jsl])
                nc.gpsimd.tensor_tensor(
                    out=E3, in0=E3,
                    in1=Zrec[:, jsl][:, :, None].broadcast_to([128, 16, 64]),
                    op=OP.mult)
                for jb in range(0, 16, 4):
                    ps = psB.tile([64, 512], F32, tag="psb", name="psb")
                    for q in range(4):
                        nc.tensor.transpose(
                            ps[:, q * 128:(q + 1) * 128],
                            E[:, (jb + q) * 64:(jb + q + 1) * 64], I128[:])
                    copy_ps(PT[:, (jc * 16 + jb) * 128:(jc * 16 + jb + 4) * 128],
                            ps[:])

            # PV for this parity: half-banks [64, 512], pairs (h, q=b)
            for b in range(RT):
                vt = work.tile([64, D], F32, tag="Vload", name="Vload")
                nc.scalar.dma_start(vt[:], vD[(2 * b + p) * 64:(2 * b + p + 1) * 64, :])
                bank = psA.tile([64, 512], F32, tag="psa", name="psa")
                for h in range(NH):
                    pr = h * 16 + b
                    nc.tensor.matmul(
                        bank[:, h * 64:(h + 1) * 64],
                        lhsT=PT4[:, :, pr],
                        rhs=vt[:, h * 64:(h + 1) * 64],
                        start=True, stop=True)
                stag = work.tile([64, 512], F32, tag="stag", name="stag")
                copy_ps(stag[:], bank[:])
                for h in range(NH):
                    base = (2 * b + p) * 64 + h * 8
                    nc.sync.dma_start(
                        aD[base:base + 8, :],
                        stag[:, h * 64:(h + 1) * 64])

    # ---------- residual + LN from aD -------------------------------------
    def resid_ln(other_nat_cb, out_cb):
        def pre_fn(rt):
            at = work.tile([128, D], F32, tag="aload", name="aload")
            nc.sync.dma_start(at[:], aD[rt * 128:(rt + 1) * 128, :])
            pt = preQ.tile([128, D], F32, tag="pre", name="pre")
            nc.vector.tensor_tensor(out=pt[:], in0=at[:], in1=other_nat_cb(rt),
                                    op=OP.add)
            return pt[:]
        for g in range(RT // 4):
            ln_group4(g, pre_fn, out_cb)

    def ln_out_to_TD(dst_dram, also_nat_dram=None):
        """LN out_cb that immediately transposes each tile into dst_dram."""
        def cb(rt, src, negmu, rstd):
            ot = work.tile([128, D], F32, tag="lnout", name="lnout", bufs=4)
            nc.vector.tensor_scalar(out=ot[:], in0=src, scalar1=negmu,
                                    scalar2=rstd, op0=OP.add, op1=OP.mult)
            if also_nat_dram is not None:
                nc.sync.dma_start(also_nat_dram[rt * 128:(rt + 1) * 128, :], ot[:])
            ps = psB.tile([128, 512], F32, tag="psb", name="psb")
            for cb_ in range(4):
                nc.tensor.transpose(ps[:, cb_ * 128:(cb_ + 1) * 128],
                                    ot[:, cb_ * 128:(cb_ + 1) * 128], I128[:])
            t = work.tile([128, 512], F32, tag="toD", name="toD", bufs=2)
            copy_ps(t[:], ps[:])
            nc.sync.dma_start(
                dst_dram[:, :, rt * 128:(rt + 1) * 128].rearrange("c a r -> a c r"),
                t[:].rearrange("a (c r) -> a c r", c=4))
        return cb

    # ---------- FFN ---------------------------------------------------------
    def ffn(xTd, resTd, w1_ap, b1_ap, w2_ap, b2_ap, out_cb):
        b2 = small.tile([1, D], F32, tag="b2", name="b2")
        nc.sync.dma_start(b2[:], b2_ap[:])
        for rc in range(4):
            xcs = []
            for dt in range(DT):
                xc = work.tile([128, 512], F32, tag=f"xfc{dt}", name=f"xfc{dt}",
                               bufs=1)
                nc.sync.dma_start(xc[:], xTd[dt, :, rc * 512:(rc + 1) * 512])
                xcs.append(xc)
            ps2 = [psB.tile([128, 512], F32, tag="psb", name="psb")
                   for _ in range(4)]
            for ff in range(FT):
                w1f = work.tile([128, 512], F32, tag="w1f", name="w1f")
                nc.scalar.dma_start(
                    w1f[:].rearrange("a (d c) -> a d c", d=4),
                    w1_ap[:, ff * 128:(ff + 1) * 128]
                        .rearrange("(d a) c -> a d c", d=4))
                b1f = small.tile([1, 128], F32, tag="b1f", name="b1f", bufs=3)
                nc.sync.dma_start(b1f[:], b1_ap[:, ff * 128:(ff + 1) * 128])
                ps1 = psA.tile([128, 512], F32, tag="psa", name="psa")
                for dt in range(DT):
                    nc.tensor.matmul(ps1[:],
                                     lhsT=w1f[:, dt * 128:(dt + 1) * 128],
                                     rhs=xcs[dt][:], start=(dt == 0), stop=False)
                nc.tensor.matmul(ps1[:], lhsT=b1f[:], rhs=ones1[:, 0:512],
                                 start=False, stop=True)
                f1f = work.tile([128, 512], F32, tag="f1f", name="f1f")
                nc.scalar.activation(f1f[:], ps1[:], ACTF.Relu)
                w2f = work.tile([128, 512], F32, tag="w2f", name="w2f")
                nc.sync.dma_start(w2f[:], w2_ap[ff * 128:(ff + 1) * 128, :])
                for rl in range(4):
                    nc.tensor.matmul(ps2[rl][:],
                                     lhsT=f1f[:, rl * 128:(rl + 1) * 128],
                                     rhs=w2f[:], start=(ff == 0), stop=False)
            def pre_fn(rt):
                rl = rt % 4
                nc.tensor.matmul(ps2[rl][:], lhsT=ones1[:, 0:128], rhs=b2[:],
                                 start=False, stop=False)
                for ct in range(DT):
                    rtl = work.tile([128, 128], F32, tag="rload", name="rload",
                                    bufs=4)
                    nc.scalar.dma_start(rtl[:], resTd[ct, :, rt * 128:(rt + 1) * 128])
                    nc.tensor.matmul(ps2[rl][:, ct * 128:(ct + 1) * 128],
                                     lhsT=rtl[:], rhs=I128[:], start=False,
                                     stop=(ct == DT - 1))
                pt = preQ.tile([128, D], F32, tag="pre", name="pre")
                copy_ps(pt[:], ps2[rl][:])
                return pt[:]
            ln_group4(rc, pre_fn, out_cb)

    # ======================= pipeline =======================
    # P1: dec1 (causal) on x_de
    embed_T_toD(hi['XdT'], xTd['xd'])
    attention(xTd['xd'], xTd['xd'], gw['dec_wv1'][:], gw['dec_wqk1'][:],
              [hi['dec1_A'][p] for p in range(2)], hi['dec1_t'], hi['dec1_cs'],
              True)
    resid_ln(lambda rt: embed_nat_ps(hi['XdT'], rt)[:],
             ln_out_to_TD(xTd['m'], also_nat_dram=mnD))

    # P2: encoder self-attn on x_en
    embed_T_toD(hi['XeT'], xTd['xe'])
    attention(xTd['xe'], xTd['xe'], gw['enc_wv'][:], gw['enc_wqk'][:],
              [hi['enc_A'][p] for p in range(2)], hi['enc_t'], hi['enc_cs'],
              False)
    resid_ln(lambda rt: embed_nat_ps(hi['XeT'], rt)[:], ln_out_to_TD(xTd['o1']))

    # P3: encoder FFN
    ffn(xTd['o1'], xTd['o1'], gw['enc_w1'][:], hi['enc_b1'], gw['enc_w2'][:],
        hi['enc_b2'], ln_out_to_TD(xTd['eo']))

    # P4: dec2 cross-attn
    attention(xTd['m'], xTd['eo'], gw['dec_wv2'][:], gw['dec_wqk2'][:],
              [hi['dec2_A'][p] for p in range(2)], hi['dec2_t'], hi['dec2_cs'],
              False)

    def m_reload(rt):
        t = work.tile([128, D], F32, tag="mload", name="mload", bufs=2)
        nc.sync.dma_start(t[:], mnD[rt * 128:(rt + 1) * 128, :])
        return t[:]
    resid_ln(m_reload, ln_out_to_TD(xTd['c']))

    # P5: decoder FFN
    ffn(xTd['c'], xTd['c'], gw['dec_w1'][:], hi['dec_b1'], gw['dec_w2'][:],
        hi['dec_b2'], ln_out_to_TD(xTd['of']))

    # P6: final projection + softmax (output ships as bf16)
    Wo = wpool.tile([128, 4 * 64], F32, tag="Wo", name="Wo")
    for dt in range(DT):
        nc.sync.dma_start(Wo[:, dt * 64:(dt + 1) * 64],
                          gw['W_out'][dt * 128:(dt + 1) * 128, :])
    Bo = small.tile([1, 64], F32, tag="Bo", name="Bo")
    nc.sync.dma_start(Bo[:], hi['B_out'][:])
    for rt in range(RT):
        ps = psB.tile([128, 64], F32, tag="psbq", name="psbo", bufs=1)
        for dt in range(DT):
            ol = work.tile([128, 128], F32, tag="rload", name="rload", bufs=4)
            nc.sync.dma_start(ol[:], xTd['of'][dt, :, rt * 128:(rt + 1) * 128])
            nc.tensor.matmul(ps[:], lhsT=ol[:], rhs=Wo[:, dt * 64:(dt + 1) * 64],
                             start=(dt == 0), stop=False)
        nc.tensor.matmul(ps[:], lhsT=ones1[:, 0:128], rhs=Bo[:],
                         start=False, stop=True)
        mx = small.tile([128, 1], F32, tag="mx", name="mx")
        nc.vector.tensor_reduce(out=mx[:], in_=ps[:], axis=AX.X, op=OP.max,
                                negate=True)
        ex = work.tile([128, 64], F32, tag="ex", name="ex")
        nc.scalar.activation(ex[:], ps[:], ACTF.Exp, bias=mx[:])
        zs = small.tile([128, 1], F32, tag="zs", name="zs")
        nc.vector.tensor_reduce(out=zs[:], in_=ex[:], axis=AX.X, op=OP.add)
        rz = small.tile([128, 1], F32, tag="rz", name="rz")
        nc.vector.reciprocal(rz[:], zs[:])
        oo = work.tile([128, 64], F32, tag="oo", name="oo")
        nc.vector.tensor_scalar(out=oo[:], in0=ex[:], scalar1=rz[:],
                                scalar2=None, op0=OP.mult)
        oo16 = work.tile([128, 64], F16, tag="oo16", name="oo16")
        nc.vector.tensor_copy(oo16[:], oo[:])
        nc.sync.dma_start(out_ap[rt * 128:(rt + 1) * 128, :], oo16[:])


# ============================================================================
# 8-core SPMD wrapper with a cached PJRT dispatcher: kernel(**inputs) -> out
# ============================================================================
_CACHE = {}


def _get_program():
    if 'nc' not in _CACHE:
        nc = bacc.Bacc("TRN2", target_bir_lowering=False, debug=False)
        hi, out_ap = declare_io(nc)
        with tile.TileContext(nc, trace_sim=False) as tc:
            with ExitStack() as ctx:
                build(ctx, tc, hi, out_ap)
        nc.compile()
        _CACHE['nc'] = nc
    return _CACHE['nc']


def _get_dispatcher():
    """One cached jit(shard_map(...)) wrapper -- same execution path as
    bass_utils.run_bass_kernel_spmd under axon (bass2jax/_bass_exec_p via
    PJRT), but without rebuilding/retracing the wrapper on every call."""
    if 'disp' in _CACHE:
        return _CACHE['disp']
    import jax
    from jax.sharding import Mesh, PartitionSpec
    from jax.experimental.shard_map import shard_map
    from concourse import bass2jax

    nc = _get_program()
    bass2jax.install_neuronx_cc_hook()
    partition_name = (nc.partition_id_tensor.name
                      if nc.partition_id_tensor else None)
    in_names, out_names, out_avals, zero_tmpl = [], [], [], []
    for alloc in nc.m.functions[0].allocations:
        if not isinstance(alloc, mybir.MemoryLocationSet):
            continue
        name = alloc.memorylocations[0].name
        if alloc.kind == "ExternalInput":
            if name != partition_name:
                in_names.append(name)
        elif alloc.kind == "ExternalOutput":
            shape = tuple(alloc.tensor_shape)
            dtype = mybir.dt.np(alloc.dtype)
            out_avals.append(jax.core.ShapedArray(shape, dtype))
            zero_tmpl.append((shape, dtype))
            out_names.append(name)
    n_params = len(in_names)
    n_outs = len(out_avals)
    all_in_names = list(in_names) + list(out_names)
    if partition_name is not None:
        all_in_names.append(partition_name)
    donate = tuple(range(n_params, n_params + n_outs))

    def _body(*args):
        operands = list(args)
        if partition_name is not None:
            operands.append(bass2jax.partition_id_tensor())
        outs = bass2jax._bass_exec_p.bind(
            *operands, out_avals=tuple(out_avals),
            in_names=tuple(all_in_names), out_names=tuple(out_names),
            lowering_input_output_aliases=(), sim_require_finite=True,
            sim_require_nnan=True, nc=nc)
        return tuple(outs)

    devices = jax.devices()[:8]
    mesh = Mesh(np.asarray(devices), ("core",))
    sharded = jax.jit(
        shard_map(_body, mesh=mesh,
                  in_specs=(PartitionSpec("core"),) * (n_params + n_outs),
                  out_specs=(PartitionSpec("core"),) * n_outs,
                  check_rep=False),
        donate_argnums=donate, keep_unused=True)

    # donated output buffers are allocated+zeroed ON DEVICE (no tunnel bytes)
    import jax.numpy as jnp
    from jax.sharding import NamedSharding
    zsh = NamedSharding(mesh, PartitionSpec("core"))
    zfn = jax.jit(
        lambda: tuple(jnp.zeros((8 * s[0], *s[1:]), d) for (s, d) in zero_tmpl),
        out_shardings=(zsh,) * n_outs)

    def dispatch(in_maps):
        concat_in = [
            np.concatenate([np.asarray(in_maps[c][nm]) for c in range(8)], 0)
            for nm in in_names]
        cz = zfn()
        outs = sharded(*concat_in, *cz)
        return [
            {nm: np.asarray(outs[i]).reshape(8, *out_avals[i].shape)[c]
             for i, nm in enumerate(out_names)}
            for c in range(8)]

    _CACHE['disp'] = dispatch
    return dispatch


def kernel(**inputs):
    dispatch = _get_dispatcher()
    in_maps = [host_inputs(inputs, core) for core in range(8)]
    res = dispatch(in_maps)
    outs = [np.asarray(res[c]['out'], np.float32) for c in range(8)]
    full = np.concatenate(outs, 0)          # [16384, 64] rows = (b, L)
    return full.reshape(64, 256, 64)


# revision 31
# speedup vs baseline: 8.1604x; 1.0863x over previous
"""Bass/Tile kernel for nn_DeepRelativeST on 8 NeuronCores (1/8 data-parallel
shard over the flat (b*L) row axis; 8 batches = 32 contiguous l-blocks per
core, so attention is core-local).

Per-core: R=2048 rows (8 batches x 256 pos), D=512, DFF=2048, H=8, dep=64,
Ll=32 local l values, 256 (l,h) softmax pairs split into two l-parity tiles:
tile p holds pair (h, l=2q+p) at partition h*16+q.

Key math (derived from reference.py):
  qs[l,h,j] = (x @ wq_headsum)[l*64+j, h]     (full Q GEMM never needed)
  ks likewise; V = x @ wv (full GEMM).
  abar[l,h,k,m] = rel[l,h,k,m-k+63] * (m<=k)  (host-gathered skew)
  r1 = sum_m abar*ks ; t = sum_m abar*m (HOST precomputed from rel)
  r2 = r1 + NEG*t ; cu = sc^2 * R1 * qs with R1 = sum_m colsum[m]*ks[m]
  (colsum = sum_k abar[.,k,.] HOST precomputed: keeps cu exact so abar can
   ship as fp8 -- r1's precision only affects softmax temperature-negligible
   terms; validated to 5.6e-5 max rel err on the host mirror.)
  logits[j,k] = cu[j]*r2[k] (+ causal NEG mask)
  p = softmax_k ; o = p @ V-block
  out row = l*64 + h*8 + j//8, col = (j%8)*64 + n   (torch raw-reshape scramble)

Transfer plan (the dispatch wall-clock is dominated by the ~35 MB/s axon
tunnel): abar ships as fp8e4 (1/4 bytes); all replicated weights ship as 1/8
row-shards and are AllGathered on-device (HBM Shared scratch); the causal
mask is built on device from a [1,4096] row; output returns as bf16.
"""
import numpy as np
from contextlib import ExitStack

import ml_dtypes

import concourse.bass as bass
import concourse.tile as tile
from concourse import bacc
from concourse import mybir

F32 = mybir.dt.float32
FP8 = mybir.dt.float8e4
F16 = mybir.dt.float16
BF16 = mybir.dt.bfloat16
U8 = mybir.dt.uint8
AX = mybir.AxisListType
OP = mybir.AluOpType
ACTF = mybir.ActivationFunctionType

R, D, DFF, NH, DEP, LL = 2048, 512, 2048, 8, 64, 32
NEG, EPS, SC2 = -1e9, 1e-5, 1.0 / 64.0
RT, DT, FT = R // 128, D // 128, DFF // 128
NC8 = [[0, 1, 2, 3, 4, 5, 6, 7]]

# replicated weights: name -> full (rows, cols); shipped as [rows//8, cols]
# REPW32: fp32 (attention-selection critical -- qs/ks path must be exact).
# REPW16: fp16 on the wire, upcast to fp32 on device (FFN/out path; validated
# to ~2e-4 host-side).
REPW32 = {
    'W_in': (64, 512),
    'enc_wv': (512, 512), 'dec_wv1': (512, 512),
    'enc_wqk': (512, 16), 'dec_wqk1': (512, 16), 'dec_wqk2': (512, 16),
    'I128': (128, 128),
}
REPW16 = {
    'enc_w1': (512, 2048), 'enc_w2': (2048, 512),
    'dec_w1': (512, 2048), 'dec_w2': (2048, 512),
    'W_out': (512, 64), 'dec_wv2': (512, 512),
}
# A ships as int4 codes (two per byte), triangle-packed into 8 row-segments
# of 8 k's, each padded to width 8*(s+1): row k in segment s=k//8 keeps
# columns m=0..8(s+1)-1 (superset of the valid m<=k). Within a segment, the
# hi nibble holds rows 8s..8s+3, the lo nibble rows 8s+4..8s+7.
# value = (code - 8) * scale[l,h], scale shipped fp32 per (l,h).
# 1152 bytes/partition vs 4096 dense fp32=16384. r1's precision headroom is
# enormous (selection is set by exact t/colsum sidecars): int4 measured
# 9.2e-4 end-to-end on the host mirror.
NSEG = 8
SEG_OFF = [16 * s * (s + 1) for s in range(NSEG)]
APK = 1152


def host_inputs(inp, core):
    f = lambda k: np.ascontiguousarray(np.asarray(inp[k], np.float32))
    bs = slice(core * 8, core * 8 + 8)
    ls = slice(core * 32, core * 32 + 32)
    Xe = f('X_en')[bs].reshape(R, 64)
    Xd = f('X_de')[bs].reshape(R, 64)

    def wqk_heads(wq, wk):
        a = wq.reshape(D, NH, DEP).sum(-1)
        b = wk.reshape(D, NH, DEP).sum(-1)
        return np.ascontiguousarray(np.concatenate([a, b], 1))  # [512,16]

    km = np.arange(64)
    kk, mm = np.meshgrid(km, km, indexing='ij')   # [k, m]

    def rel_arrange(rel):
        r = rel[ls]                                # [32,8,64,64] = [l,h,k,c]
        # abar[l,h,k,m] = r[l,h,k,m-k+63] if m<=k else 0
        c = mm - kk + 63
        valid = (mm <= kk)
        cs = np.clip(c, 0, 63)
        ab = np.take_along_axis(
            r.reshape(LL, NH, 64, 64), cs.reshape(1, 1, 64, 64), axis=3)
        ab = ab * valid.reshape(1, 1, 64, 64)
        t = (ab * mm.reshape(1, 1, 64, 64)).sum(-1)     # [l,h,k]   exact
        csum = ab.sum(-2)                               # [l,h,m]   exact
        abT = ab.transpose(1, 0, 2, 3)                  # [h,l,k,m]
        tT = t.transpose(1, 0, 2)                       # [h,l,k]
        cT = csum.transpose(1, 0, 2)                    # [h,l,m]
        A4 = np.zeros((2, 128, APK), np.uint8)
        Sc = np.empty((2, 128, 1), np.float32)
        Tt = np.empty((2, 128, 64), np.float32)
        Cs = np.empty((2, 128, 64), np.float32)
        for p in range(2):
            d = abT[:, p::2].reshape(128, 64, 64)          # [a, k, m]
            scale = np.maximum(np.abs(d).max((1, 2)), 1e-30) / 7.0
            codes = (np.clip(np.round(d / scale[:, None, None]), -8, 7)
                     + 8).astype(np.uint8)
            for s in range(NSEG):
                ws = 8 * (s + 1)
                blk = codes[:, 8 * s:8 * (s + 1), 0:ws].reshape(128, 8 * ws)
                half = 4 * ws
                A4[p][:, SEG_OFF[s]:SEG_OFF[s] + half] = \
                    (blk[:, :half] << 4) | blk[:, half:]
            Sc[p] = scale.reshape(128, 1)
            Tt[p] = tT[:, p::2].reshape(128, 64)
            Cs[p] = cT[:, p::2].reshape(128, 64)
        return A4, Sc, Tt, Cs

    A_e, s_e, t_e, c_e = rel_arrange(f('enc_rel'))
    A_d1, s_d1, t_d1, c_d1 = rel_arrange(f('dec_rel1'))
    A_d2, s_d2, t_d2, c_d2 = rel_arrange(f('dec_rel2'))
    caus_row = np.ascontiguousarray(
        np.triu(np.full((64, 64), NEG, np.float32), 1).reshape(1, 4096))

    out = {
        'XeT': np.ascontiguousarray(Xe.T), 'XdT': np.ascontiguousarray(Xd.T),
        'B_in': f('B_in').reshape(1, D),
        'enc_A': A_e, 'enc_scl': s_e, 'enc_t': t_e, 'enc_cs': c_e,
        'dec1_A': A_d1, 'dec1_scl': s_d1, 'dec1_t': t_d1, 'dec1_cs': c_d1,
        'dec2_A': A_d2, 'dec2_scl': s_d2, 'dec2_t': t_d2, 'dec2_cs': c_d2,
        'enc_b1': f('enc_b1').reshape(1, DFF), 'enc_b2': f('enc_b2').reshape(1, D),
        'dec_b1': f('dec_b1').reshape(1, DFF), 'dec_b2': f('dec_b2').reshape(1, D),
        'B_out': f('B_out').reshape(1, 64),
        'caus_row': caus_row,
    }
    fulls = {
        'W_in': f('W_in'),
        'enc_wv': f('enc_wv'), 'dec_wv1': f('dec_wv1'), 'dec_wv2': f('dec_wv2'),
        'enc_wqk': wqk_heads(f('enc_wq'), f('enc_wk')),
        'dec_wqk1': wqk_heads(f('dec_wq1'), f('dec_wk1')),
        'dec_wqk2': wqk_heads(f('dec_wq2'), f('dec_wk2')),
        'I128': np.eye(128, dtype=np.float32),
    }
    for nm, (r, c) in REPW32.items():
        sh = r // 8
        out[nm] = np.ascontiguousarray(fulls[nm][core * sh:(core + 1) * sh])
    for nm, (r, c) in REPW16.items():
        sh = r // 8
        out[nm] = np.ascontiguousarray(
            f(nm)[core * sh:(core + 1) * sh].astype(np.float16))
    return out


IN_SHAPES = {
    'XeT': ((64, R), F32), 'XdT': ((64, R), F32), 'B_in': ((1, D), F32),
    'enc_A': ((2, 128, APK), U8), 'dec1_A': ((2, 128, APK), U8),
    'dec2_A': ((2, 128, APK), U8),
    'enc_scl': ((2, 128, 1), F32), 'dec1_scl': ((2, 128, 1), F32),
    'dec2_scl': ((2, 128, 1), F32),
    'enc_t': ((2, 128, 64), F32), 'dec1_t': ((2, 128, 64), F32),
    'dec2_t': ((2, 128, 64), F32),
    'enc_cs': ((2, 128, 64), F32), 'dec1_cs': ((2, 128, 64), F32),
    'dec2_cs': ((2, 128, 64), F32),
    'enc_b1': ((1, DFF), F32), 'enc_b2': ((1, D), F32),
    'dec_b1': ((1, DFF), F32), 'dec_b2': ((1, D), F32),
    'B_out': ((1, 64), F32), 'caus_row': ((1, 4096), F32),
    **{nm: ((r // 8, c), F32) for nm, (r, c) in REPW32.items()},
    **{nm: ((r // 8, c), F16) for nm, (r, c) in REPW16.items()},
}


def declare_io(nc):
    hi = {k: nc.dram_tensor(k, list(s), dt, kind="ExternalInput").ap()
          for k, (s, dt) in IN_SHAPES.items()}
    out = nc.dram_tensor('out', [R, 64], F16, kind="ExternalOutput").ap()
    return hi, out


def build(ctx: ExitStack, tc: tile.TileContext, hi, out_ap, dbg=None):
    nc = tc.nc
    consts = ctx.enter_context(tc.tile_pool(name="consts", bufs=1))
    wpool = ctx.enter_context(tc.tile_pool(name="wpool", bufs=1))
    work = ctx.enter_context(tc.tile_pool(name="work", bufs=3))
    preQ = ctx.enter_context(tc.tile_pool(name="preQ", bufs=8))
    small = ctx.enter_context(tc.tile_pool(name="small", bufs=1))
    bigP = ctx.enter_context(tc.tile_pool(name="bigP", bufs=1))
    psA = ctx.enter_context(tc.tile_pool(name="psA", bufs=3, space="PSUM"))
    psB = ctx.enter_context(tc.tile_pool(name="psB", bufs=4, space="PSUM"))
    dram = ctx.enter_context(tc.tile_pool(name="dram", bufs=1, space="DRAM"))

    # ---------- gather replicated weights from 1/8 shards -------------------
    gw = {}
    for nm, (r, c) in REPW32.items():
        loc = dram.tile([r // 8, c], F32, tag=f"agl_{nm}", name=f"agl_{nm}")
        nc.sync.dma_start(loc[:], hi[nm][:])
        full = dram.tile([r, c], F32, addr_space="Shared",
                         tag=f"agf_{nm}", name=f"agf_{nm}")
        nc.gpsimd.collective_compute(
            "AllGather", OP.bypass, replica_groups=NC8,
            ins=[loc[:]], outs=[full[:]])
        gw[nm] = full
    for nm, (r, c) in REPW16.items():
        loc = dram.tile([r // 8, c], F16, tag=f"agl_{nm}", name=f"agl_{nm}")
        nc.sync.dma_start(loc[:], hi[nm][:])
        full16 = dram.tile([r, c], F16, addr_space="Shared",
                           tag=f"agh_{nm}", name=f"agh_{nm}")
        nc.gpsimd.collective_compute(
            "AllGather", OP.bypass, replica_groups=NC8,
            ins=[loc[:]], outs=[full16[:]])
        full = dram.tile([r, c], F32, tag=f"agf_{nm}", name=f"agf_{nm}")
        for r0 in range(0, r, 128):
            for c0 in range(0, c, 512):
                cw = min(512, c - c0)
                t16 = work.tile([128, 512], F16, tag="u16", name="u16", bufs=2)
                nc.sync.dma_start(t16[:, 0:cw],
                                  full16[r0:r0 + 128, c0:c0 + cw])
                t32 = work.tile([128, 512], F32, tag="xcT", name="u32")
                nc.vector.tensor_copy(t32[:, 0:cw], t16[:, 0:cw])
                nc.sync.dma_start(full[r0:r0 + 128, c0:c0 + cw], t32[:, 0:cw])
        gw[nm] = full

    I128 = consts.tile([128, 128], F32, tag="I128", name="I128")
    nc.sync.dma_start(I128[:], gw['I128'][:])
    ones1 = consts.tile([1, D], F32, tag="ones1", name="ones1")
    nc.vector.memset(ones1[:], 1.0)
    epsc = consts.tile([128, 1], F32, tag="epsc", name="epsc")
    nc.vector.memset(epsc[:], EPS)
    W_in = consts.tile([64, D], F32, tag="W_in", name="W_in")
    nc.sync.dma_start(W_in[:], gw['W_in'][:])
    B_in = consts.tile([1, D], F32, tag="B_in", name="B_in")
    nc.sync.dma_start(B_in[:], hi['B_in'][:])

    # causal mask [128, 4096] built on device from the [1,4096] row into
    # DRAM scratch (PE partition-broadcast), streamed back at use like the
    # baseline's shipped CAUS.
    causD = dram.tile([128, 4096], F32, tag="causD", name="causD")
    for q in range(8):
        cr = work.tile([1, 512], F32, tag="xin", name="crowc")
        nc.sync.dma_start(cr[:], hi['caus_row'][:, q * 512:(q + 1) * 512])
        ps = psA.tile([128, 512], F32, tag="psa", name="psa")
        nc.tensor.matmul(ps[:], lhsT=ones1[:, 0:128], rhs=cr[:],
                         start=True, stop=True)
        st = work.tile([128, 512], F32, tag="toD", name="toD", bufs=2)
        nc.scalar.copy(st[:], ps[:])
        nc.sync.dma_start(causD[:, q * 512:(q + 1) * 512], st[:])

    # DRAM scratch: transposed activations live here, streamed at use.
    xTd = {nm: dram.tile([DT, 128, R], F32, tag=f"xTd_{nm}", name=f"xTd_{nm}")
           for nm in ('xe', 'xd', 'm', 'o1', 'eo', 'c', 'of')}
    aD = dram.tile([R, D], F32, tag="aD", name="aD")
    vD = dram.tile([R, D], F32, tag="vD", name="vD")
    mnD = dram.tile([R, D], F32, tag="mnD", name="mnD")

    def copy_ps(dst, src):
        nc.scalar.copy(dst, src)

    # ---------- embed: x.T = (X@W_in+B).T streamed to DRAM ------------------
    def embed_T_toD(x_in_ap, dst):
        for ct in range(DT):
            for rc in range(4):
                xin = work.tile([64, 512], F32, tag="xin", name="xin")
                nc.sync.dma_start(xin[:], x_in_ap[:, rc * 512:(rc + 1) * 512])
                ps = psA.tile([128, 512], F32, tag="psa", name="psa")
                nc.tensor.matmul(ps[:], lhsT=W_in[:, ct * 128:(ct + 1) * 128],
                                 rhs=xin[:], start=True, stop=False)
                nc.tensor.matmul(ps[:], lhsT=B_in[:, ct * 128:(ct + 1) * 128],
                                 rhs=ones1[:, 0:512], start=False, stop=True)
                t = work.tile([128, 512], F32, tag="toD", name="toD", bufs=2)
                copy_ps(t[:], ps[:])
                nc.sync.dma_start(dst[ct, :, rc * 512:(rc + 1) * 512], t[:])

    def embed_nat_ps(x_in_ap, rt):
        xin = work.tile([64, 128], F32, tag="xin2", name="xin2")
        nc.sync.dma_start(xin[:], x_in_ap[:, rt * 128:(rt + 1) * 128])
        ps = psA.tile([128, 512], F32, tag="psa", name="psa")
        nc.tensor.matmul(ps[:], lhsT=xin[:], rhs=W_in[:], start=True, stop=False)
        nc.tensor.matmul(ps[:], lhsT=ones1[:, 0:128], rhs=B_in[:],
                         start=False, stop=True)
        return ps

    # ---------- layernorm over one group of 4 row-tiles ---------------------
    def ln_group4(g, pre_fn, out_cb):
        """pre_fn(rt) -> [128,512] AP (lazy); out_cb(rt, src, nmu, rstd)."""
        if True:
            sx = small.tile([128, 4], F32, tag="sx", name="sx", bufs=2)
            sx2 = small.tile([128, 4], F32, tag="sx2", name="sx2", bufs=2)
            pres = []
            for i in range(4):
                pa = pre_fn(g * 4 + i)
                pres.append(pa)
                scr = work.tile([128, D], F32, tag="lnscr", name="lnscr")
                nc.scalar.activation(scr[:], pa, ACTF.Copy,
                                     accum_out=sx[:, i:i + 1])
                nc.scalar.activation(scr[:], pa, ACTF.Square,
                                     accum_out=sx2[:, i:i + 1])
            negmu = small.tile([128, 4], F32, tag="negmu", name="negmu", bufs=2)
            nc.vector.tensor_scalar(out=negmu[:], in0=sx[:], scalar1=-1.0 / D,
                                    scalar2=None, op0=OP.mult)
            mu2 = small.tile([128, 4], F32, tag="mu2", name="mu2", bufs=2)
            nc.vector.tensor_tensor(out=mu2[:], in0=negmu[:], in1=negmu[:],
                                    op=OP.mult)
            var = small.tile([128, 4], F32, tag="var", name="var", bufs=2)
            nc.vector.scalar_tensor_tensor(out=var[:], in0=sx2[:],
                                           scalar=1.0 / D, in1=mu2[:],
                                           op0=OP.mult, op1=OP.subtract)
            std = small.tile([128, 4], F32, tag="std", name="std", bufs=2)
            nc.scalar.activation(std[:], var[:], ACTF.Sqrt, bias=epsc[:])
            rstd = small.tile([128, 4], F32, tag="rstd", name="rstd", bufs=2)
            nc.vector.reciprocal(rstd[:], std[:])
            for i in range(4):
                out_cb(g * 4 + i, pres[i], negmu[:, i:i + 1], rstd[:, i:i + 1])

    # ---------- attention ---------------------------------------------------
    def attention(xqTd, xkvTd, wv_ap, wqk_ap, A_ap, scl_ap, t_ap, cs_ap, causal):
        # V GEMM (x.T-stationary tiles streamed from DRAM) -> vD
        wv = wpool.tile([128, 4 * D], F32, tag="wv", name="wv")
        for dt in range(DT):
            nc.sync.dma_start(wv[:, dt * D:(dt + 1) * D],
                              wv_ap[dt * 128:(dt + 1) * 128, :])
        for rt in range(RT):
            ps = psA.tile([128, 512], F32, tag="psa", name="psa")
            for dt in range(DT):
                xl = work.tile([128, 128], F32, tag="xlT", name="xlT")
                nc.sync.dma_start(xl[:], xkvTd[dt, :, rt * 128:(rt + 1) * 128])
                nc.tensor.matmul(ps[:], lhsT=xl[:],
                                 rhs=wv[:, dt * D:(dt + 1) * D],
                                 start=(dt == 0), stop=(dt == DT - 1))
            vt = work.tile([128, D], F32, tag="Vtile", name="Vtile")
            copy_ps(vt[:], ps[:])
            nc.sync.dma_start(vD[rt * 128:(rt + 1) * 128, :], vt[:])

        # qs / ks GEMMs (W-stationary, M=8)
        wqk = wpool.tile([128, 4 * 16], F32, tag="wqk", name="wqk")
        for dt in range(DT):
            nc.sync.dma_start(wqk[:, dt * 16:(dt + 1) * 16],
                              wqk_ap[dt * 128:(dt + 1) * 128, :])
        qT = work.tile([8, R], F32, tag="qT", name="qT", bufs=1)
        kT = work.tile([8, R], F32, tag="kT", name="kT", bufs=1)
        for (dst, colofs, srcTd) in ((qT, 0, xqTd), (kT, 8, xkvTd)):
            for rc in range(4):
                ps = psB.tile([8, 512], F32, tag="psbq", name="psbq", bufs=1)
                for dt in range(DT):
                    xc = work.tile([128, 512], F32, tag="xcT", name="xcT")
                    nc.sync.dma_start(xc[:], srcTd[dt, :, rc * 512:(rc + 1) * 512])
                    nc.tensor.matmul(
                        ps[:], lhsT=wqk[:, dt * 16 + colofs: dt * 16 + colofs + 8],
                        rhs=xc[:], start=(dt == 0), stop=(dt == DT - 1))
                copy_ps(dst[:, rc * 512:(rc + 1) * 512], ps[:])

        qs_pp = small.tile([128, 2 * 64], F32, tag="qs_pp", name="qs_pp")
        ks_pp = small.tile([128, 2 * 64], F32, tag="ks_pp", name="ks_pp")
        qD = dram.tile([8, R], F32, tag="qD", name="qD")
        kD = dram.tile([8, R], F32, tag="kD", name="kD")
        for (src, bounce, dst) in ((qT, qD, qs_pp), (kT, kD, ks_pp)):
            nc.sync.dma_start(bounce[:], src[:])
            nc.sync.dma_start(
                dst[:], bounce[:].rearrange("h (q f) -> (h q) f", q=16))

        # r1 = sum_m abar*ks. abar arrives as int4 nibble pairs, triangle-
        # packed in 8 segments of 8 k-rows padded to width 8(s+1); unpack
        # (shift/mask -> u8->f32 -> fused (x-8)*scale) then mult-reduce.
        scl_pp = small.tile([128, 2], F32, tag="scl_pp", name="scl_pp")
        nc.sync.dma_start(scl_pp[:].rearrange("a (p k) -> a p k", p=2),
                          scl_ap[:].rearrange("p a k -> a p k"))
        r1 = small.tile([128, 2 * 64], F32, tag="r1", name="r1")
        for p in range(2):
            for s in range(NSEG):
                ws = 8 * (s + 1)
                width = 8 * ws
                half = 4 * ws
                off = SEG_OFF[s]
                A4t = work.tile([128, 256], U8, tag="A8chunk", name="A8chunk",
                                bufs=1)
                nc.scalar.dma_start(A4t[:, 0:half], A_ap[p][:, off:off + half])
                hiu = work.tile([128, 256], U8, tag="hiu", name="hiu", bufs=1)
                nc.vector.tensor_scalar(out=hiu[:, 0:half], in0=A4t[:, 0:half],
                                        scalar1=4, scalar2=None,
                                        op0=OP.logical_shift_right)
                lou = work.tile([128, 256], U8, tag="lou", name="lou", bufs=1)
                nc.vector.tensor_scalar(out=lou[:, 0:half], in0=A4t[:, 0:half],
                                        scalar1=15, scalar2=None,
                                        op0=OP.bitwise_and)
                A = work.tile([128, 512], F32, tag="Achunk", name="Achunk", bufs=1)
                nc.vector.tensor_copy(A[:, 0:half], hiu[:, 0:half])
                nc.vector.tensor_copy(A[:, half:width], lou[:, 0:half])
                nc.vector.tensor_scalar(out=A[:, 0:width], in0=A[:, 0:width],
                                        scalar1=8.0, scalar2=scl_pp[:, p:p + 1],
                                        op0=OP.subtract, op1=OP.mult)
                A3 = A[:, 0:width].rearrange("a (k m) -> a k m", k=8)
                nc.gpsimd.tensor_tensor(
                    out=A3, in0=A3,
                    in1=ks_pp[:, p * 64:p * 64 + ws][:, None, :]
                        .broadcast_to([128, 8, ws]), op=OP.mult)
                nc.vector.tensor_reduce(
                    out=r1[:, p * 64 + s * 8: p * 64 + (s + 1) * 8],
                    in_=A3, axis=AX.X, op=OP.add)
        tH = small.tile([128, 2 * 64], F32, tag="tH", name="tH")
        nc.sync.dma_start(tH[:].rearrange("a (p k) -> a p k", p=2),
                          t_ap[:].rearrange("p a k -> a p k"))
        r2 = small.tile([128, 2 * 64], F32, tag="r2", name="r2")
        nc.vector.scalar_tensor_tensor(out=r2[:], in0=tH[:], scalar=NEG,
                                       in1=r1[:], op0=OP.mult, op1=OP.add)
        # R1 exact via host colsum: R1[p] = sum_m colsum[m]*ks[m]
        csH = small.tile([128, 2 * 64], F32, tag="csH", name="csH")
        nc.sync.dma_start(csH[:].rearrange("a (p k) -> a p k", p=2),
                          cs_ap[:].rearrange("p a k -> a p k"))
        csk = small.tile([128, 2 * 64], F32, tag="csk", name="csk")
        nc.vector.tensor_tensor(out=csk[:], in0=csH[:], in1=ks_pp[:], op=OP.mult)
        R1s = small.tile([128, 2], F32, tag="R1s", name="R1s")
        nc.vector.tensor_reduce(out=R1s[:],
                                in_=csk[:].rearrange("a (p k) -> a p k", p=2),
                                axis=AX.X, op=OP.add)
        nc.vector.tensor_scalar(out=R1s[:], in0=R1s[:], scalar1=SC2,
                                scalar2=None, op0=OP.mult)
        cu = small.tile([128, 2 * 64], F32, tag="cu", name="cu")
        for p in range(2):
            nc.vector.tensor_scalar(out=cu[:, p * 64:(p + 1) * 64],
                                    in0=qs_pp[:, p * 64:(p + 1) * 64],
                                    scalar1=R1s[:, p:p + 1], scalar2=None,
                                    op0=OP.mult)

        # M = rowmax of logits (rank-1 trick; scans for causal)
        M = small.tile([128, 2 * 64], F32, tag="Mm", name="Mm")
        t1 = small.tile([128, 64], F32, tag="Mt1", name="Mt1")
        t2 = small.tile([128, 64], F32, tag="Mt2", name="Mt2")
        if not causal:
            wmax = small.tile([128, 2], F32, tag="wmax", name="wmax")
            wmin = small.tile([128, 2], F32, tag="wmin", name="wmin")
            nc.vector.tensor_reduce(out=wmax[:],
                                    in_=r2[:].rearrange("a (p k) -> a p k", p=2),
                                    axis=AX.X, op=OP.max)
            nc.vector.tensor_reduce(out=wmin[:],
                                    in_=r2[:].rearrange("a (p k) -> a p k", p=2),
                                    axis=AX.X, op=OP.min)
            for p in range(2):
                sl = slice(p * 64, (p + 1) * 64)
                nc.vector.tensor_scalar(out=M[:, sl], in0=cu[:, sl],
                                        scalar1=wmax[:, p:p + 1], scalar2=None,
                                        op0=OP.mult)
                nc.vector.tensor_scalar(out=t1[:], in0=cu[:, sl],
                                        scalar1=wmin[:, p:p + 1], scalar2=None,
                                        op0=OP.mult)
                nc.vector.tensor_tensor(out=M[:, sl], in0=M[:, sl], in1=t1[:],
                                        op=OP.max)
        else:
            pm = small.tile([128, 128], F32, tag="pm", name="pm")
            pn = small.tile([128, 128], F32, tag="pn", name="pn")
            sm = small.tile([128, 128], F32, tag="sm", name="sm")
            sn = small.tile([128, 128], F32, tag="sn", name="sn")
            for p in range(2):
                sl = slice(p * 64, (p + 1) * 64)
                w_ = r2[:, sl]
                wr = r2[:, sl][:, ::-1]
                nc.vector.tensor_tensor_scan(out=pm[:, sl], data0=w_, data1=w_,
                                             initial=-3e38, op0=OP.max, op1=OP.bypass)
                nc.vector.tensor_tensor_scan(out=pn[:, sl], data0=w_, data1=w_,
                                             initial=3e38, op0=OP.min, op1=OP.bypass)
                nc.vector.tensor_tensor_scan(out=sm[:, sl][:, ::-1], data0=wr,
                                             data1=wr, initial=-3e38,
                                             op0=OP.max, op1=OP.bypass)
                nc.vector.tensor_tensor_scan(out=sn[:, sl][:, ::-1], data0=wr,
                                             data1=wr, initial=3e38,
                                             op0=OP.min, op1=OP.bypass)
            for p in range(2):
                sl = slice(p * 64, (p + 1) * 64)
                nc.vector.tensor_tensor(out=M[:, sl], in0=cu[:, sl],
                                        in1=pm[:, sl], op=OP.mult)
                nc.vector.tensor_tensor(out=t1[:], in0=cu[:, sl], in1=pn[:, sl],
                                        op=OP.mult)
                nc.vector.tensor_tensor(out=M[:, sl], in0=M[:, sl], in1=t1[:],
                                        op=OP.max)
                j63 = slice(p * 64, p * 64 + 63)
                cs = cu[:, j63]
                nc.vector.tensor_tensor(out=t1[:, 0:63], in0=cs,
                                        in1=sm[:, p * 64 + 1:(p + 1) * 64],
                                        op=OP.mult)
                nc.vector.tensor_tensor(out=t2[:, 0:63], in0=cs,
                                        in1=sn[:, p * 64 + 1:(p + 1) * 64],
                                        op=OP.mult)
                nc.vector.tensor_tensor(out=t1[:, 0:63], in0=t1[:, 0:63],
                                        in1=t2[:, 0:63], op=OP.max)
                nc.vector.tensor_scalar(out=t1[:, 0:63], in0=t1[:, 0:63],
                                        scalar1=NEG, scalar2=None, op0=OP.add)
                nc.vector.tensor_tensor(out=M[:, j63], in0=M[:, j63],
                                        in1=t1[:, 0:63], op=OP.max)

        # E chunks of 16 j: build/mask/-M/exp/Z/scale -> transpose to PT -> PV
        Zrec = small.tile([128, 2 * 64], F32, tag="Zrec", name="Zrec")
        for p in range(2):
            PT = bigP.tile([64, 64 * 128], F32, tag="PT", name="PT")
            PT4 = PT[:].rearrange("k (j pp) -> k j pp", j=64)
            for jc in range(4):
                jsl = slice(p * 64 + jc * 16, p * 64 + (jc + 1) * 16)
                E = work.tile([128, 1024], F32, tag="Echunk", name="Echunk", bufs=2)
                E3 = E[:].rearrange("a (j k) -> a j k", j=16)
                nc.vector.tensor_tensor(
                    out=E3, in0=cu[:, jsl][:, :, None].broadcast_to([128, 16, 64]),
                    in1=r2[:, p * 64:(p + 1) * 64][:, None, :]
                        .broadcast_to([128, 16, 64]), op=OP.mult)
                if causal:
                    CS = work.tile([128, 1024], F32, tag="CSchunk", name="CSchunk",
                                   bufs=2)
                    nc.scalar.dma_start(CS[:], causD[:, jc * 1024:(jc + 1) * 1024])
                    nc.gpsimd.tensor_tensor(out=E[:], in0=E[:], in1=CS[:], op=OP.add)
                nc.vector.tensor_tensor(
                    out=E3, in0=E3,
                    in1=M[:, jsl][:, :, None].broadcast_to([128, 16, 64]),
                    op=OP.subtract)
                nc.scalar.activation(E[:], E[:], ACTF.Exp)
                nc.vector.tensor_reduce(out=Zrec[:, jsl], in_=E3, axis=AX.X,
                                        op=OP.add)
                nc.vector.reciprocal(Zrec[:, jsl], Zrec[:, jsl])
                nc.gpsimd.tensor_tensor(
                    out=E3, in0=E3,
                    in1=Zrec[:, jsl][:, :, None].broadcast_to([128, 16, 64]),
                    op=OP.mult)
                for jb in range(0, 16, 4):
                    ps = psB.tile([64, 512], F32, tag="psb", name="psb")
                    for q in range(4):
                        nc.tensor.transpose(
                            ps[:, q * 128:(q + 1) * 128],
                            E[:, (jb + q) * 64:(jb + q + 1) * 64], I128[:])
                    copy_ps(PT[:, (jc * 16 + jb) * 128:(jc * 16 + jb + 4) * 128],
                            ps[:])

            # PV for this parity: half-banks [64, 512], pairs (h, q=b)
            for b in range(RT):
                vt = work.tile([64, D], F32, tag="Vload", name="Vload")
                nc.scalar.dma_start(vt[:], vD[(2 * b + p) * 64:(2 * b + p + 1) * 64, :])
                bank = psA.tile([64, 512], F32, tag="psa", name="psa")
                for h in range(NH):
                    pr = h * 16 + b
                    nc.tensor.matmul(
                        bank[:, h * 64:(h + 1) * 64],
                        lhsT=PT4[:, :, pr],
                        rhs=vt[:, h * 64:(h + 1) * 64],
                        start=True, stop=True)
                stag = work.tile([64, 512], F32, tag="stag", name="stag")
                copy_ps(stag[:], bank[:])
                for h in range(NH):
                    base = (2 * b + p) * 64 + h * 8
                    nc.sync.dma_start(
                        aD[base:base + 8, :],
                        stag[:, h * 64:(h + 1) * 64])

    # ---------- residual + LN from aD -------------------------------------
    def resid_ln(other_nat_cb, out_cb):
        def pre_fn(rt):
            at = work.tile([128, D], F32, tag="aload", name="aload")
            nc.sync.dma_start(at[:], aD[rt * 128:(rt + 1) * 128, :])
            pt = preQ.tile([128, D], F32, tag="pre", name="pre")
            nc.vector.tensor_tensor(out=pt[:], in0=at[:], in1=other_nat_cb(rt),
                                    op=OP.add)
            return pt[:]
        for g in range(RT // 4):
            ln_group4(g, pre_fn, out_cb)

    def ln_out_to_TD(dst_dram, also_nat_dram=None):
        """LN out_cb that immediately transposes each tile into dst_dram."""
        def cb(rt, src, negmu, rstd):
            ot = work.tile([128, D], F32, tag="lnout", name="lnout", bufs=4)
            nc.vector.tensor_scalar(out=ot[:], in0=src, scalar1=negmu,
                                    scalar2=rstd, op0=OP.add, op1=OP.mult)
            if also_nat_dram is not None:
                nc.sync.dma_start(also_nat_dram[rt * 128:(rt + 1) * 128, :], ot[:])
            ps = psB.tile([128, 512], F32, tag="psb", name="psb")
            for cb_ in range(4):
                nc.tensor.transpose(ps[:, cb_ * 128:(cb_ + 1) * 128],
                                    ot[:, cb_ * 128:(cb_ + 1) * 128], I128[:])
            t = work.tile([128, 512], F32, tag="toD", name="toD", bufs=2)
            copy_ps(t[:], ps[:])
            nc.sync.dma_start(
                dst_dram[:, :, rt * 128:(rt + 1) * 128].rearrange("c a r -> a c r"),
                t[:].rearrange("a (c r) -> a c r", c=4))
        return cb

    # ---------- FFN ---------------------------------------------------------
    def ffn(xTd, resTd, w1_ap, b1_ap, w2_ap, b2_ap, out_cb):
        b2 = small.tile([1, D], F32, tag="b2", name="b2")
        nc.sync.dma_start(b2[:], b2_ap[:])
        for rc in range(4):
            xcs = []
            for dt in range(DT):
                xc = work.tile([128, 512], F32, tag=f"xfc{dt}", name=f"xfc{dt}",
                               bufs=1)
                nc.sync.dma_start(xc[:], xTd[dt, :, rc * 512:(rc + 1) * 512])
                xcs.append(xc)
            ps2 = [psB.tile([128, 512], F32, tag="psb", name="psb")
                   for _ in range(4)]
            for ff in range(FT):
                w1f = work.tile([128, 512], F32, tag="w1f", name="w1f")
                nc.scalar.dma_start(
                    w1f[:].rearrange("a (d c) -> a d c", d=4),
                    w1_ap[:, ff * 128:(ff + 1) * 128]
                        .rearrange("(d a) c -> a d c", d=4))
                b1f = small.tile([1, 128], F32, tag="b1f", name="b1f", bufs=3)
                nc.sync.dma_start(b1f[:], b1_ap[:, ff * 128:(ff + 1) * 128])
                ps1 = psA.tile([128, 512], F32, tag="psa", name="psa")
                for dt in range(DT):
                    nc.tensor.matmul(ps1[:],
                                     lhsT=w1f[:, dt * 128:(dt + 1) * 128],
                                     rhs=xcs[dt][:], start=(dt == 0), stop=False)
                nc.tensor.matmul(ps1[:], lhsT=b1f[:], rhs=ones1[:, 0:512],
                                 start=False, stop=True)
                f1f = work.tile([128, 512], F32, tag="f1f", name="f1f")
                nc.scalar.activation(f1f[:], ps1[:], ACTF.Relu)
                w2f = work.tile([128, 512], F32, tag="w2f", name="w2f")
                nc.sync.dma_start(w2f[:], w2_ap[ff * 128:(ff + 1) * 128, :])
                for rl in range(4):
                    nc.tensor.matmul(ps2[rl][:],
                                     lhsT=f1f[:, rl * 128:(rl + 1) * 128],
                                     rhs=w2f[:], start=(ff == 0), stop=False)
            def pre_fn(rt):
                rl = rt % 4
                nc.tensor.matmul(ps2[rl][:], lhsT=ones1[:, 0:128], rhs=b2[:],
                                 start=False, stop=False)
                for ct in range(DT):
                    rtl = work.tile([128, 128], F32, tag="rload", name="rload",
                                    bufs=4)
                    nc.scalar.dma_start(rtl[:], resTd[ct, :, rt * 128:(rt + 1) * 128])
                    nc.tensor.matmul(ps2[rl][:, ct * 128:(ct + 1) * 128],
                                     lhsT=rtl[:], rhs=I128[:], start=False,
                                     stop=(ct == DT - 1))
                pt = preQ.tile([128, D], F32, tag="pre", name="pre")
                copy_ps(pt[:], ps2[rl][:])
                return pt[:]
            ln_group4(rc, pre_fn, out_cb)

    # ======================= pipeline =======================
    # P1: dec1 (causal) on x_de
    embed_T_toD(hi['XdT'], xTd['xd'])
    attention(xTd['xd'], xTd['xd'], gw['dec_wv1'][:], gw['dec_wqk1'][:],
              [hi['dec1_A'][p] for p in range(2)], hi['dec1_scl'],
              hi['dec1_t'], hi['dec1_cs'], True)
    resid_ln(lambda rt: embed_nat_ps(hi['XdT'], rt)[:],
             ln_out_to_TD(xTd['m'], also_nat_dram=mnD))

    # P2: encoder self-attn on x_en
    embed_T_toD(hi['XeT'], xTd['xe'])
    attention(xTd['xe'], xTd['xe'], gw['enc_wv'][:], gw['enc_wqk'][:],
              [hi['enc_A'][p] for p in range(2)], hi['enc_scl'],
              hi['enc_t'], hi['enc_cs'], False)
    resid_ln(lambda rt: embed_nat_ps(hi['XeT'], rt)[:], ln_out_to_TD(xTd['o1']))

    # P3: encoder FFN
    ffn(xTd['o1'], xTd['o1'], gw['enc_w1'][:], hi['enc_b1'], gw['enc_w2'][:],
        hi['enc_b2'], ln_out_to_TD(xTd['eo']))

    # P4: dec2 cross-attn
    attention(xTd['m'], xTd['eo'], gw['dec_wv2'][:], gw['dec_wqk2'][:],
              [hi['dec2_A'][p] for p in range(2)], hi['dec2_scl'],
              hi['dec2_t'], hi['dec2_cs'], False)

    def m_reload(rt):
        t = work.tile([128, D], F32, tag="mload", name="mload", bufs=2)
        nc.sync.dma_start(t[:], mnD[rt * 128:(rt + 1) * 128, :])
        return t[:]
    resid_ln(m_reload, ln_out_to_TD(xTd['c']))

    # P5: decoder FFN
    ffn(xTd['c'], xTd['c'], gw['dec_w1'][:], hi['dec_b1'], gw['dec_w2'][:],
        hi['dec_b2'], ln_out_to_TD(xTd['of']))

    # P6: final projection + softmax (output ships as bf16)
    Wo = wpool.tile([128, 4 * 64], F32, tag="Wo", name="Wo")
    for dt in range(DT):
        nc.sync.dma_start(Wo[:, dt * 64:(dt + 1) * 64],
                          gw['W_out'][dt * 128:(dt + 1) * 128, :])
    Bo = small.tile([1, 64], F32, tag="Bo", name="Bo")
    nc.sync.dma_start(Bo[:], hi['B_out'][:])
    for rt in range(RT):
        ps = psB.tile([128, 64], F32, tag="psbq", name="psbo", bufs=1)
        for dt in range(DT):
            ol = work.tile([128, 128], F32, tag="rload", name="rload", bufs=4)
            nc.sync.dma_start(ol[:], xTd['of'][dt, :, rt * 128:(rt + 1) * 128])
            nc.tensor.matmul(ps[:], lhsT=ol[:], rhs=Wo[:, dt * 64:(dt + 1) * 64],
                             start=(dt == 0), stop=False)
        nc.tensor.matmul(ps[:], lhsT=ones1[:, 0:128], rhs=Bo[:],
                         start=False, stop=True)
        mx = small.tile([128, 1], F32, tag="mx", name="mx")
        nc.vector.tensor_reduce(out=mx[:], in_=ps[:], axis=AX.X, op=OP.max,
                                negate=True)
        ex = work.tile([128, 64], F32, tag="ex", name="ex")
        nc.scalar.activation(ex[:], ps[:], ACTF.Exp, bias=mx[:])
        zs = small.tile([128, 1], F32, tag="zs", name="zs")
        nc.vector.tensor_reduce(out=zs[:], in_=ex[:], axis=AX.X, op=OP.add)
        rz = small.tile([128, 1], F32, tag="rz", name="rz")
        nc.vector.reciprocal(rz[:], zs[:])
        oo = work.tile([128, 64], F32, tag="oo", name="oo")
        nc.vector.tensor_scalar(out=oo[:], in0=ex[:], scalar1=rz[:],
                                scalar2=None, op0=OP.mult)
        oo16 = work.tile([128, 64], F16, tag="oo16", name="oo16")
        nc.vector.tensor_copy(oo16[:], oo[:])
        nc.sync.dma_start(out_ap[rt * 128:(rt + 1) * 128, :], oo16[:])


# ============================================================================
# 8-core SPMD wrapper with a cached PJRT dispatcher: kernel(**inputs) -> out
# ============================================================================
_CACHE = {}


def _get_program():
    if 'nc' not in _CACHE:
        nc = bacc.Bacc("TRN2", target_bir_lowering=False, debug=False)
        hi, out_ap = declare_io(nc)
        with tile.TileContext(nc, trace_sim=False) as tc:
            with ExitStack() as ctx:
                build(ctx, tc, hi, out_ap)
        nc.compile()
        _CACHE['nc'] = nc
    return _CACHE['nc']


def _get_dispatcher():
    """One cached jit(shard_map(...)) wrapper -- same execution path as
    bass_utils.run_bass_kernel_spmd under axon (bass2jax/_bass_exec_p via
    PJRT), but without rebuilding/retracing the wrapper on every call."""
    if 'disp' in _CACHE:
        return _CACHE['disp']
    import jax
    from jax.sharding import Mesh, PartitionSpec
    from jax.experimental.shard_map import shard_map
    from concourse import bass2jax

    nc = _get_program()
    bass2jax.install_neuronx_cc_hook()
    partition_name = (nc.partition_id_tensor.name
                      if nc.partition_id_tensor else None)
    in_names, out_names, out_avals, zero_tmpl = [], [], [], []
    for alloc in nc.m.functions[0].allocations:
        if not isinstance(alloc, mybir.MemoryLocationSet):
            continue
        name = alloc.memorylocations[0].name
        if alloc.kind == "ExternalInput":
            if name != partition_name:
                in_names.append(name)
        elif alloc.kind == "ExternalOutput":
            shape = tuple(alloc.tensor_shape)
            dtype = mybir.dt.np(alloc.dtype)
            out_avals.append(jax.core.ShapedArray(shape, dtype))
            zero_tmpl.append((shape, dtype))
            out_names.append(name)
    n_params = len(in_names)
    n_outs = len(out_avals)
    all_in_names = list(in_names) + list(out_names)
    if partition_name is not None:
        all_in_names.append(partition_name)
    donate = tuple(range(n_params, n_params + n_outs))

    def _body(*args):
        operands = list(args)
        if partition_name is not None:
            operands.append(bass2jax.partition_id_tensor())
        outs = bass2jax._bass_exec_p.bind(
            *operands, out_avals=tuple(out_avals),
            in_names=tuple(all_in_names), out_names=tuple(out_names),
            lowering_input_output_aliases=(), sim_require_finite=True,
            sim_require_nnan=True, nc=nc)
        return tuple(outs)

    devices = jax.devices()[:8]
    mesh = Mesh(np.asarray(devices), ("core",))
    sharded = jax.jit(
        shard_map(_body, mesh=mesh,
                  in_specs=(PartitionSpec("core"),) * (n_params + n_outs),
                  out_specs=(PartitionSpec("core"),) * n_outs,
                  check_rep=False),
        donate_argnums=donate, keep_unused=True)

    # donated output buffers are allocated+zeroed ON DEVICE (no tunnel bytes)
    import jax.numpy as jnp
    from jax.sharding import NamedSharding
    zsh = NamedSharding(mesh, PartitionSpec("core"))
    zfn = jax.jit(
        lambda: tuple(jnp.zeros((8 * s[0], *s[1:]), d) for (s, d) in zero_tmpl),
        out_shardings=(zsh,) * n_outs)

    def dispatch(in_maps):
        concat_in = [
            np.concatenate([np.asarray(in_maps[c][nm]) for c in range(8)], 0)
            for nm in in_names]
        cz = zfn()
        outs = sharded(*concat_in, *cz)
        return [
            {nm: np.asarray(outs[i]).reshape(8, *out_avals[i].shape)[c]
             for i, nm in enumerate(out_names)}
            for c in range(8)]

    _CACHE['disp'] = dispatch
    return dispatch


def kernel(**inputs):
    dispatch = _get_dispatcher()
    in_maps = [host_inputs(inputs, core) for core in range(8)]
    res = dispatch(in_maps)
    outs = [np.asarray(res[c]['out'], np.float32) for c in range(8)]
    full = np.concatenate(outs, 0)          # [16384, 64] rows = (b, L)
    return full.reshape(64, 256, 64)


# revision 32
# speedup vs baseline: 8.7348x; 1.0704x over previous
"""Bass/Tile kernel for nn_DeepRelativeST on 8 NeuronCores (1/8 data-parallel
shard over the flat (b*L) row axis; 8 batches = 32 contiguous l-blocks per
core, so attention is core-local).

Per-core: R=2048 rows (8 batches x 256 pos), D=512, DFF=2048, H=8, dep=64,
Ll=32 local l values, 256 (l,h) softmax pairs split into two l-parity tiles:
tile p holds pair (h, l=2q+p) at partition h*16+q.

Key math (derived from reference.py):
  qs[l,h,j] = (x @ wq_headsum)[l*64+j, h]     (full Q GEMM never needed)
  ks likewise; V = x @ wv (full GEMM).
  abar[l,h,k,m] = rel[l,h,k,m-k+63] * (m<=k)  (host-gathered skew)
  r1 = sum_m abar*ks ; t = sum_m abar*m (HOST precomputed from rel)
  r2 = r1 + NEG*t ; cu = sc^2 * R1 * qs with R1 = sum_m colsum[m]*ks[m]
  (colsum = sum_k abar[.,k,.] HOST precomputed: keeps cu exact so abar can
   ship at very low precision -- r1's error only shifts logits that softmax
   renders irrelevant; int4 abar validated to 9.2e-4 max rel err end-to-end.)
  logits[j,k] = cu[j]*r2[k] (+ causal NEG mask)
  p = softmax_k ; o = p @ V-block
  out row = l*64 + h*8 + j//8, col = (j%8)*64 + n   (torch raw-reshape scramble)

Transfer plan (the dispatch wall-clock is dominated by the ~30 MB/s serial
axon tunnel, so bytes-on-wire is everything): abar ships as triangle-packed
int4 codes + per-(l,h) fp32 scales, with exact fp32 t/colsum sidecars; X and
the q/k-path weights stay fp32 (attention selection is chaotic -- any
quantization there flips softmax winners); FFN weights + dec_wv2 + W_out
ship fp16 and are upcast on device; all replicated weights ship as 1/8
row-shards and are AllGathered on-device (HBM Shared scratch); the causal
mask is built on device from a [1,4096] row; the donated output buffers are
zeroed on device; output returns as fp16 and is upcast on host.
"""
import numpy as np
from contextlib import ExitStack

import ml_dtypes

import concourse.bass as bass
import concourse.tile as tile
from concourse import bacc
from concourse import mybir

F32 = mybir.dt.float32
FP8 = mybir.dt.float8e4
F16 = mybir.dt.float16
BF16 = mybir.dt.bfloat16
U8 = mybir.dt.uint8
AX = mybir.AxisListType
OP = mybir.AluOpType
ACTF = mybir.ActivationFunctionType

R, D, DFF, NH, DEP, LL = 2048, 512, 2048, 8, 64, 32
NEG, EPS, SC2 = -1e9, 1e-5, 1.0 / 64.0
RT, DT, FT = R // 128, D // 128, DFF // 128
NC8 = [[0, 1, 2, 3, 4, 5, 6, 7]]

# replicated weights: name -> full (rows, cols); shipped as [rows//8, cols]
# REPW32: fp32 (attention-selection critical -- qs/ks path must be exact).
# REPW16: fp16 on the wire, upcast to fp32 on device (FFN/out path; validated
# to ~2e-4 host-side).
REPW32 = {
    'W_in': (64, 512),
    'enc_wv': (512, 512), 'dec_wv1': (512, 512),
    'enc_wqk': (512, 16), 'dec_wqk1': (512, 16), 'dec_wqk2': (512, 16),
    'I128': (128, 128),
}
REPW16 = {
    'enc_w1': (512, 2048), 'enc_w2': (2048, 512),
    'dec_w1': (512, 2048), 'dec_w2': (2048, 512),
    'W_out': (512, 64), 'dec_wv2': (512, 512),
}
# A ships as int4 codes (two per byte), triangle-packed into 8 row-segments
# of 8 k's, each padded to width 8*(s+1): row k in segment s=k//8 keeps
# columns m=0..8(s+1)-1 (superset of the valid m<=k). Within a segment, the
# hi nibble holds rows 8s..8s+3, the lo nibble rows 8s+4..8s+7.
# value = (code - 8) * scale[l,h], scale shipped fp32 per (l,h).
# 1152 bytes/partition vs 4096 dense fp32=16384. r1's precision headroom is
# enormous (selection is set by exact t/colsum sidecars): int4 measured
# 9.2e-4 end-to-end on the host mirror.
NSEG = 8
SEG_OFF = [16 * s * (s + 1) for s in range(NSEG)]
APK = 1152


def host_inputs(inp, core):
    f = lambda k: np.ascontiguousarray(np.asarray(inp[k], np.float32))
    bs = slice(core * 8, core * 8 + 8)
    ls = slice(core * 32, core * 32 + 32)
    Xe = f('X_en')[bs].reshape(R, 64)
    Xd = f('X_de')[bs].reshape(R, 64)

    def wqk_heads(wq, wk):
        a = wq.reshape(D, NH, DEP).sum(-1)
        b = wk.reshape(D, NH, DEP).sum(-1)
        return np.ascontiguousarray(np.concatenate([a, b], 1))  # [512,16]

    km = np.arange(64)
    kk, mm = np.meshgrid(km, km, indexing='ij')   # [k, m]

    def rel_arrange(rel):
        r = rel[ls]                                # [32,8,64,64] = [l,h,k,c]
        # abar[l,h,k,m] = r[l,h,k,m-k+63] if m<=k else 0
        c = mm - kk + 63
        valid = (mm <= kk)
        cs = np.clip(c, 0, 63)
        ab = np.take_along_axis(
            r.reshape(LL, NH, 64, 64), cs.reshape(1, 1, 64, 64), axis=3)
        ab = ab * valid.reshape(1, 1, 64, 64)
        t = (ab * mm.reshape(1, 1, 64, 64)).sum(-1)     # [l,h,k]   exact
        csum = ab.sum(-2)                               # [l,h,m]   exact
        abT = ab.transpose(1, 0, 2, 3)                  # [h,l,k,m]
        tT = t.transpose(1, 0, 2)                       # [h,l,k]
        cT = csum.transpose(1, 0, 2)                    # [h,l,m]
        A4 = np.zeros((2, 128, APK), np.uint8)
        Sc = np.empty((2, 128, 1), np.float32)
        Tt = np.empty((2, 128, 64), np.float32)
        Cs = np.empty((2, 128, 64), np.float32)
        for p in range(2):
            d = abT[:, p::2].reshape(128, 64, 64)          # [a, k, m]
            scale = np.maximum(np.abs(d).max((1, 2)), 1e-30) / 7.0
            codes = (np.clip(np.round(d / scale[:, None, None]), -8, 7)
                     + 8).astype(np.uint8)
            for s in range(NSEG):
                ws = 8 * (s + 1)
                blk = codes[:, 8 * s:8 * (s + 1), 0:ws].reshape(128, 8 * ws)
                half = 4 * ws
                A4[p][:, SEG_OFF[s]:SEG_OFF[s] + half] = \
                    (blk[:, :half] << 4) | blk[:, half:]
            Sc[p] = scale.reshape(128, 1)
            Tt[p] = tT[:, p::2].reshape(128, 64)
            Cs[p] = cT[:, p::2].reshape(128, 64)
        return A4, Sc, Tt, Cs

    A_e, s_e, t_e, c_e = rel_arrange(f('enc_rel'))
    A_d1, s_d1, t_d1, c_d1 = rel_arrange(f('dec_rel1'))
    A_d2, s_d2, t_d2, c_d2 = rel_arrange(f('dec_rel2'))
    caus_row = np.ascontiguousarray(
        np.triu(np.full((64, 64), NEG, np.float32), 1).reshape(1, 4096))

    out = {
        'XeT': np.ascontiguousarray(Xe.T), 'XdT': np.ascontiguousarray(Xd.T),
        'B_in': f('B_in').reshape(1, D),
        'enc_A': A_e, 'enc_scl': s_e, 'enc_t': t_e, 'enc_cs': c_e,
        'dec1_A': A_d1, 'dec1_scl': s_d1, 'dec1_t': t_d1, 'dec1_cs': c_d1,
        'dec2_A': A_d2, 'dec2_scl': s_d2, 'dec2_t': t_d2, 'dec2_cs': c_d2,
        'enc_b1': f('enc_b1').reshape(1, DFF), 'enc_b2': f('enc_b2').reshape(1, D),
        'dec_b1': f('dec_b1').reshape(1, DFF), 'dec_b2': f('dec_b2').reshape(1, D),
        'B_out': f('B_out').reshape(1, 64),
        'caus_row': caus_row,
    }
    fulls = {
        'W_in': f('W_in'),
        'enc_wv': f('enc_wv'), 'dec_wv1': f('dec_wv1'), 'dec_wv2': f('dec_wv2'),
        'enc_wqk': wqk_heads(f('enc_wq'), f('enc_wk')),
        'dec_wqk1': wqk_heads(f('dec_wq1'), f('dec_wk1')),
        'dec_wqk2': wqk_heads(f('dec_wq2'), f('dec_wk2')),
        'I128': np.eye(128, dtype=np.float32),
    }
    for nm, (r, c) in REPW32.items():
        sh = r // 8
        out[nm] = np.ascontiguousarray(fulls[nm][core * sh:(core + 1) * sh])
    for nm, (r, c) in REPW16.items():
        sh = r // 8
        out[nm] = np.ascontiguousarray(
            f(nm)[core * sh:(core + 1) * sh].astype(np.float16))
    return out


IN_SHAPES = {
    'XeT': ((64, R), F32), 'XdT': ((64, R), F32), 'B_in': ((1, D), F32),
    'enc_A': ((2, 128, APK), U8), 'dec1_A': ((2, 128, APK), U8),
    'dec2_A': ((2, 128, APK), U8),
    'enc_scl': ((2, 128, 1), F32), 'dec1_scl': ((2, 128, 1), F32),
    'dec2_scl': ((2, 128, 1), F32),
    'enc_t': ((2, 128, 64), F32), 'dec1_t': ((2, 128, 64), F32),
    'dec2_t': ((2, 128, 64), F32),
    'enc_cs': ((2, 128, 64), F32), 'dec1_cs': ((2, 128, 64), F32),
    'dec2_cs': ((2, 128, 64), F32),
    'enc_b1': ((1, DFF), F32), 'enc_b2': ((1, D), F32),
    'dec_b1': ((1, DFF), F32), 'dec_b2': ((1, D), F32),
    'B_out': ((1, 64), F32), 'caus_row': ((1, 4096), F32),
    **{nm: ((r // 8, c), F32) for nm, (r, c) in REPW32.items()},
    **{nm: ((r // 8, c), F16) for nm, (r, c) in REPW16.items()},
}


def declare_io(nc):
    hi = {k: nc.dram_tensor(k, list(s), dt, kind="ExternalInput").ap()
          for k, (s, dt) in IN_SHAPES.items()}
    out = nc.dram_tensor('out', [R, 64], F16, kind="ExternalOutput").ap()
    return hi, out


def build(ctx: ExitStack, tc: tile.TileContext, hi, out_ap, dbg=None):
    nc = tc.nc
    consts = ctx.enter_context(tc.tile_pool(name="consts", bufs=1))
    wpool = ctx.enter_context(tc.tile_pool(name="wpool", bufs=1))
    work = ctx.enter_context(tc.tile_pool(name="work", bufs=3))
    preQ = ctx.enter_context(tc.tile_pool(name="preQ", bufs=8))
    small = ctx.enter_context(tc.tile_pool(name="small", bufs=1))
    bigP = ctx.enter_context(tc.tile_pool(name="bigP", bufs=1))
    psA = ctx.enter_context(tc.tile_pool(name="psA", bufs=3, space="PSUM"))
    psB = ctx.enter_context(tc.tile_pool(name="psB", bufs=4, space="PSUM"))
    dram = ctx.enter_context(tc.tile_pool(name="dram", bufs=1, space="DRAM"))

    # ---------- gather replicated weights from 1/8 shards -------------------
    gw = {}
    for nm, (r, c) in REPW32.items():
        loc = dram.tile([r // 8, c], F32, tag=f"agl_{nm}", name=f"agl_{nm}")
        nc.sync.dma_start(loc[:], hi[nm][:])
        full = dram.tile([r, c], F32, addr_space="Shared",
                         tag=f"agf_{nm}", name=f"agf_{nm}")
        nc.gpsimd.collective_compute(
            "AllGather", OP.bypass, replica_groups=NC8,
            ins=[loc[:]], outs=[full[:]])
        gw[nm] = full
    for nm, (r, c) in REPW16.items():
        loc = dram.tile([r // 8, c], F16, tag=f"agl_{nm}", name=f"agl_{nm}")
        nc.sync.dma_start(loc[:], hi[nm][:])
        full16 = dram.tile([r, c], F16, addr_space="Shared",
                           tag=f"agh_{nm}", name=f"agh_{nm}")
        nc.gpsimd.collective_compute(
            "AllGather", OP.bypass, replica_groups=NC8,
            ins=[loc[:]], outs=[full16[:]])
        full = dram.tile([r, c], F32, tag=f"agf_{nm}", name=f"agf_{nm}")
        for r0 in range(0, r, 128):
            for c0 in range(0, c, 512):
                cw = min(512, c - c0)
                t16 = work.tile([128, 512], F16, tag="u16", name="u16", bufs=2)
                nc.sync.dma_start(t16[:, 0:cw],
                                  full16[r0:r0 + 128, c0:c0 + cw])
                t32 = work.tile([128, 512], F32, tag="xcT", name="u32")
                nc.vector.tensor_copy(t32[:, 0:cw], t16[:, 0:cw])
                nc.sync.dma_start(full[r0:r0 + 128, c0:c0 + cw], t32[:, 0:cw])
        gw[nm] = full

    I128 = consts.tile([128, 128], F32, tag="I128", name="I128")
    nc.sync.dma_start(I128[:], gw['I128'][:])
    ones1 = consts.tile([1, D], F32, tag="ones1", name="ones1")
    nc.vector.memset(ones1[:], 1.0)
    epsc = consts.tile([128, 1], F32, tag="epsc", name="epsc")
    nc.vector.memset(epsc[:], EPS)
    W_in = consts.tile([64, D], F32, tag="W_in", name="W_in")
    nc.sync.dma_start(W_in[:], gw['W_in'][:])
    B_in = consts.tile([1, D], F32, tag="B_in", name="B_in")
    nc.sync.dma_start(B_in[:], hi['B_in'][:])

    # causal mask [128, 4096] built on device from the [1,4096] row into
    # DRAM scratch (PE partition-broadcast), streamed back at use like the
    # baseline's shipped CAUS.
    causD = dram.tile([128, 4096], F32, tag="causD", name="causD")
    for q in range(8):
        cr = work.tile([1, 512], F32, tag="xin", name="crowc")
        nc.sync.dma_start(cr[:], hi['caus_row'][:, q * 512:(q + 1) * 512])
        ps = psA.tile([128, 512], F32, tag="psa", name="psa")
        nc.tensor.matmul(ps[:], lhsT=ones1[:, 0:128], rhs=cr[:],
                         start=True, stop=True)
        st = work.tile([128, 512], F32, tag="toD", name="toD", bufs=2)
        nc.scalar.copy(st[:], ps[:])
        nc.sync.dma_start(causD[:, q * 512:(q + 1) * 512], st[:])

    # DRAM scratch: transposed activations live here, streamed at use.
    xTd = {nm: dram.tile([DT, 128, R], F32, tag=f"xTd_{nm}", name=f"xTd_{nm}")
           for nm in ('xe', 'xd', 'm', 'o1', 'eo', 'c', 'of')}
    aD = dram.tile([R, D], F32, tag="aD", name="aD")
    vD = dram.tile([R, D], F32, tag="vD", name="vD")
    mnD = dram.tile([R, D], F32, tag="mnD", name="mnD")

    def copy_ps(dst, src):
        nc.scalar.copy(dst, src)

    # ---------- embed: x.T = (X@W_in+B).T streamed to DRAM ------------------
    def embed_T_toD(x_in_ap, dst):
        for ct in range(DT):
            for rc in range(4):
                xin = work.tile([64, 512], F32, tag="xin", name="xin")
                nc.sync.dma_start(xin[:], x_in_ap[:, rc * 512:(rc + 1) * 512])
                ps = psA.tile([128, 512], F32, tag="psa", name="psa")
                nc.tensor.matmul(ps[:], lhsT=W_in[:, ct * 128:(ct + 1) * 128],
                                 rhs=xin[:], start=True, stop=False)
                nc.tensor.matmul(ps[:], lhsT=B_in[:, ct * 128:(ct + 1) * 128],
                                 rhs=ones1[:, 0:512], start=False, stop=True)
                t = work.tile([128, 512], F32, tag="toD", name="toD", bufs=2)
                copy_ps(t[:], ps[:])
                nc.sync.dma_start(dst[ct, :, rc * 512:(rc + 1) * 512], t[:])

    def embed_nat_ps(x_in_ap, rt):
        xin = work.tile([64, 128], F32, tag="xin2", name="xin2")
        nc.sync.dma_start(xin[:], x_in_ap[:, rt * 128:(rt + 1) * 128])
        ps = psA.tile([128, 512], F32, tag="psa", name="psa")
        nc.tensor.matmul(ps[:], lhsT=xin[:], rhs=W_in[:], start=True, stop=False)
        nc.tensor.matmul(ps[:], lhsT=ones1[:, 0:128], rhs=B_in[:],
                         start=False, stop=True)
        return ps

    # ---------- layernorm over one group of 4 row-tiles ---------------------
    def ln_group4(g, pre_fn, out_cb):
        """pre_fn(rt) -> [128,512] AP (lazy); out_cb(rt, src, nmu, rstd)."""
        if True:
            sx = small.tile([128, 4], F32, tag="sx", name="sx", bufs=2)
            sx2 = small.tile([128, 4], F32, tag="sx2", name="sx2", bufs=2)
            pres = []
            for i in range(4):
                pa = pre_fn(g * 4 + i)
                pres.append(pa)
                scr = work.tile([128, D], F32, tag="lnscr", name="lnscr")
                nc.scalar.activation(scr[:], pa, ACTF.Copy,
                                     accum_out=sx[:, i:i + 1])
                nc.scalar.activation(scr[:], pa, ACTF.Square,
                                     accum_out=sx2[:, i:i + 1])
            negmu = small.tile([128, 4], F32, tag="negmu", name="negmu", bufs=2)
            nc.vector.tensor_scalar(out=negmu[:], in0=sx[:], scalar1=-1.0 / D,
                                    scalar2=None, op0=OP.mult)
            mu2 = small.tile([128, 4], F32, tag="mu2", name="mu2", bufs=2)
            nc.vector.tensor_tensor(out=mu2[:], in0=negmu[:], in1=negmu[:],
                                    op=OP.mult)
            var = small.tile([128, 4], F32, tag="var", name="var", bufs=2)
            nc.vector.scalar_tensor_tensor(out=var[:], in0=sx2[:],
                                           scalar=1.0 / D, in1=mu2[:],
                                           op0=OP.mult, op1=OP.subtract)
            std = small.tile([128, 4], F32, tag="std", name="std", bufs=2)
            nc.scalar.activation(std[:], var[:], ACTF.Sqrt, bias=epsc[:])
            rstd = small.tile([128, 4], F32, tag="rstd", name="rstd", bufs=2)
            nc.vector.reciprocal(rstd[:], std[:])
            for i in range(4):
                out_cb(g * 4 + i, pres[i], negmu[:, i:i + 1], rstd[:, i:i + 1])

    # ---------- attention ---------------------------------------------------
    def attention(xqTd, xkvTd, wv_ap, wqk_ap, A_ap, scl_ap, t_ap, cs_ap, causal):
        # V GEMM (x.T-stationary tiles streamed from DRAM) -> vD
        wv = wpool.tile([128, 4 * D], F32, tag="wv", name="wv")
        for dt in range(DT):
            nc.sync.dma_start(wv[:, dt * D:(dt + 1) * D],
                              wv_ap[dt * 128:(dt + 1) * 128, :])
        for rt in range(RT):
            ps = psA.tile([128, 512], F32, tag="psa", name="psa")
            for dt in range(DT):
                xl = work.tile([128, 128], F32, tag="xlT", name="xlT")
                nc.sync.dma_start(xl[:], xkvTd[dt, :, rt * 128:(rt + 1) * 128])
                nc.tensor.matmul(ps[:], lhsT=xl[:],
                                 rhs=wv[:, dt * D:(dt + 1) * D],
                                 start=(dt == 0), stop=(dt == DT - 1))
            vt = work.tile([128, D], F32, tag="Vtile", name="Vtile")
            copy_ps(vt[:], ps[:])
            nc.sync.dma_start(vD[rt * 128:(rt + 1) * 128, :], vt[:])

        # qs / ks GEMMs (W-stationary, M=8)
        wqk = wpool.tile([128, 4 * 16], F32, tag="wqk", name="wqk")
        for dt in range(DT):
            nc.sync.dma_start(wqk[:, dt * 16:(dt + 1) * 16],
                              wqk_ap[dt * 128:(dt + 1) * 128, :])
        qT = work.tile([8, R], F32, tag="qT", name="qT", bufs=1)
        kT = work.tile([8, R], F32, tag="kT", name="kT", bufs=1)
        for (dst, colofs, srcTd) in ((qT, 0, xqTd), (kT, 8, xkvTd)):
            for rc in range(4):
                ps = psB.tile([8, 512], F32, tag="psbq", name="psbq", bufs=1)
                for dt in range(DT):
                    xc = work.tile([128, 512], F32, tag="xcT", name="xcT")
                    nc.sync.dma_start(xc[:], srcTd[dt, :, rc * 512:(rc + 1) * 512])
                    nc.tensor.matmul(
                        ps[:], lhsT=wqk[:, dt * 16 + colofs: dt * 16 + colofs + 8],
                        rhs=xc[:], start=(dt == 0), stop=(dt == DT - 1))
                copy_ps(dst[:, rc * 512:(rc + 1) * 512], ps[:])

        qs_pp = small.tile([128, 2 * 64], F32, tag="qs_pp", name="qs_pp")
        ks_pp = small.tile([128, 2 * 64], F32, tag="ks_pp", name="ks_pp")
        qD = dram.tile([8, R], F32, tag="qD", name="qD")
        kD = dram.tile([8, R], F32, tag="kD", name="kD")
        for (src, bounce, dst) in ((qT, qD, qs_pp), (kT, kD, ks_pp)):
            nc.sync.dma_start(bounce[:], src[:])
            nc.sync.dma_start(
                dst[:], bounce[:].rearrange("h (q f) -> (h q) f", q=16))

        # r1 = sum_m abar*ks. abar arrives as int4 nibble pairs, triangle-
        # packed in 8 segments of 8 k-rows padded to width 8(s+1); unpack
        # (shift/mask -> u8->f32 -> fused (x-8)*scale) then mult-reduce.
        scl_pp = small.tile([128, 2], F32, tag="scl_pp", name="scl_pp")
        nc.sync.dma_start(scl_pp[:].rearrange("a (p k) -> a p k", p=2),
                          scl_ap[:].rearrange("p a k -> a p k"))
        r1 = small.tile([128, 2 * 64], F32, tag="r1", name="r1")
        for p in range(2):
            for s in range(NSEG):
                ws = 8 * (s + 1)
                width = 8 * ws
                half = 4 * ws
                off = SEG_OFF[s]
                A4t = work.tile([128, 256], U8, tag="A8chunk", name="A8chunk",
                                bufs=1)
                nc.scalar.dma_start(A4t[:, 0:half], A_ap[p][:, off:off + half])
                hiu = work.tile([128, 256], U8, tag="hiu", name="hiu", bufs=1)
                nc.vector.tensor_scalar(out=hiu[:, 0:half], in0=A4t[:, 0:half],
                                        scalar1=4, scalar2=None,
                                        op0=OP.logical_shift_right)
                lou = work.tile([128, 256], U8, tag="lou", name="lou", bufs=1)
                nc.vector.tensor_scalar(out=lou[:, 0:half], in0=A4t[:, 0:half],
                                        scalar1=15, scalar2=None,
                                        op0=OP.bitwise_and)
                A = work.tile([128, 512], F32, tag="Achunk", name="Achunk", bufs=1)
                nc.vector.tensor_copy(A[:, 0:half], hiu[:, 0:half])
                nc.vector.tensor_copy(A[:, half:width], lou[:, 0:half])
                nc.vector.tensor_scalar(out=A[:, 0:width], in0=A[:, 0:width],
                                        scalar1=8.0, scalar2=scl_pp[:, p:p + 1],
                                        op0=OP.subtract, op1=OP.mult)
                A3 = A[:, 0:width].rearrange("a (k m) -> a k m", k=8)
                nc.gpsimd.tensor_tensor(
                    out=A3, in0=A3,
                    in1=ks_pp[:, p * 64:p * 64 + ws][:, None, :]
                        .broadcast_to([128, 8, ws]), op=OP.mult)
                nc.vector.tensor_reduce(
                    out=r1[:, p * 64 + s * 8: p * 64 + (s + 1) * 8],
                    in_=A3, axis=AX.X, op=OP.add)
        tH = small.tile([128, 2 * 64], F32, tag="tH", name="tH")
        nc.sync.dma_start(tH[:].rearrange("a (p k) -> a p k", p=2),
                          t_ap[:].rearrange("p a k -> a p k"))
        r2 = small.tile([128, 2 * 64], F32, tag="r2", name="r2")
        nc.vector.scalar_tensor_tensor(out=r2[:], in0=tH[:], scalar=NEG,
                                       in1=r1[:], op0=OP.mult, op1=OP.add)
        # R1 exact via host colsum: R1[p] = sum_m colsum[m]*ks[m]
        csH = small.tile([128, 2 * 64], F32, tag="csH", name="csH")
        nc.sync.dma_start(csH[:].rearrange("a (p k) -> a p k", p=2),
                          cs_ap[:].rearrange("p a k -> a p k"))
        csk = small.tile([128, 2 * 64], F32, tag="csk", name="csk")
        nc.vector.tensor_tensor(out=csk[:], in0=csH[:], in1=ks_pp[:], op=OP.mult)
        R1s = small.tile([128, 2], F32, tag="R1s", name="R1s")
        nc.vector.tensor_reduce(out=R1s[:],
                                in_=csk[:].rearrange("a (p k) -> a p k", p=2),
                                axis=AX.X, op=OP.add)
        nc.vector.tensor_scalar(out=R1s[:], in0=R1s[:], scalar1=SC2,
                                scalar2=None, op0=OP.mult)
        cu = small.tile([128, 2 * 64], F32, tag="cu", name="cu")
        for p in range(2):
            nc.vector.tensor_scalar(out=cu[:, p * 64:(p + 1) * 64],
                                    in0=qs_pp[:, p * 64:(p + 1) * 64],
                                    scalar1=R1s[:, p:p + 1], scalar2=None,
                                    op0=OP.mult)

        # M = rowmax of logits (rank-1 trick; scans for causal)
        M = small.tile([128, 2 * 64], F32, tag="Mm", name="Mm")
        t1 = small.tile([128, 64], F32, tag="Mt1", name="Mt1")
        t2 = small.tile([128, 64], F32, tag="Mt2", name="Mt2")
        if not causal:
            wmax = small.tile([128, 2], F32, tag="wmax", name="wmax")
            wmin = small.tile([128, 2], F32, tag="wmin", name="wmin")
            nc.vector.tensor_reduce(out=wmax[:],
                                    in_=r2[:].rearrange("a (p k) -> a p k", p=2),
                                    axis=AX.X, op=OP.max)
            nc.vector.tensor_reduce(out=wmin[:],
                                    in_=r2[:].rearrange("a (p k) -> a p k", p=2),
                                    axis=AX.X, op=OP.min)
            for p in range(2):
                sl = slice(p * 64, (p + 1) * 64)
                nc.vector.tensor_scalar(out=M[:, sl], in0=cu[:, sl],
                                        scalar1=wmax[:, p:p + 1], scalar2=None,
                                        op0=OP.mult)
                nc.vector.tensor_scalar(out=t1[:], in0=cu[:, sl],
                                        scalar1=wmin[:, p:p + 1], scalar2=None,
                                        op0=OP.mult)
                nc.vector.tensor_tensor(out=M[:, sl], in0=M[:, sl], in1=t1[:],
                                        op=OP.max)
        else:
            pm = small.tile([128, 128], F32, tag="pm", name="pm")
            pn = small.tile([128, 128], F32, tag="pn", name="pn")
            sm = small.tile([128, 128], F32, tag="sm", name="sm")
            sn = small.tile([128, 128], F32, tag="sn", name="sn")
            for p in range(2):
                sl = slice(p * 64, (p + 1) * 64)
                w_ = r2[:, sl]
                wr = r2[:, sl][:, ::-1]
                nc.vector.tensor_tensor_scan(out=pm[:, sl], data0=w_, data1=w_,
                                             initial=-3e38, op0=OP.max, op1=OP.bypass)
                nc.vector.tensor_tensor_scan(out=pn[:, sl], data0=w_, data1=w_,
                                             initial=3e38, op0=OP.min, op1=OP.bypass)
                nc.vector.tensor_tensor_scan(out=sm[:, sl][:, ::-1], data0=wr,
                                             data1=wr, initial=-3e38,
                                             op0=OP.max, op1=OP.bypass)
                nc.vector.tensor_tensor_scan(out=sn[:, sl][:, ::-1], data0=wr,
                                             data1=wr, initial=3e38,
                                             op0=OP.min, op1=OP.bypass)
            for p in range(2):
                sl = slice(p * 64, (p + 1) * 64)
                nc.vector.tensor_tensor(out=M[:, sl], in0=cu[:, sl],
                                        in1=pm[:, sl], op=OP.mult)
                nc.vector.tensor_tensor(out=t1[:], in0=cu[:, sl], in1=pn[:, sl],
                                        op=OP.mult)
                nc.vector.tensor_tensor(out=M[:, sl], in0=M[:, sl], in1=t1[:],
                                        op=OP.max)
                j63 = slice(p * 64, p * 64 + 63)
                cs = cu[:, j63]
                nc.vector.tensor_tensor(out=t1[:, 0:63], in0=cs,
                                        in1=sm[:, p * 64 + 1:(p + 1) * 64],
                                        op=OP.mult)
                nc.vector.tensor_tensor(out=t2[:, 0:63], in0=cs,
                                        in1=sn[:, p * 64 + 1:(p + 1) * 64],
                                        op=OP.mult)
                nc.vector.tensor_tensor(out=t1[:, 0:63], in0=t1[:, 0:63],
                                        in1=t2[:, 0:63], op=OP.max)
                nc.vector.tensor_scalar(out=t1[:, 0:63], in0=t1[:, 0:63],
                                        scalar1=NEG, scalar2=None, op0=OP.add)
                nc.vector.tensor_tensor(out=M[:, j63], in0=M[:, j63],
                                        in1=t1[:, 0:63], op=OP.max)

        # E chunks of 16 j: build/mask/-M/exp/Z/scale -> transpose to PT -> PV
        Zrec = small.tile([128, 2 * 64], F32, tag="Zrec", name="Zrec")
        for p in range(2):
            PT = bigP.tile([64, 64 * 128], F32, tag="PT", name="PT")
            PT4 = PT[:].rearrange("k (j pp) -> k j pp", j=64)
            for jc in range(4):
                jsl = slice(p * 64 + jc * 16, p * 64 + (jc + 1) * 16)
                E = work.tile([128, 1024], F32, tag="Echunk", name="Echunk", bufs=2)
                E3 = E[:].rearrange("a (j k) -> a j k", j=16)
                nc.vector.tensor_tensor(
                    out=E3, in0=cu[:, jsl][:, :, None].broadcast_to([128, 16, 64]),
                    in1=r2[:, p * 64:(p + 1) * 64][:, None, :]
                        .broadcast_to([128, 16, 64]), op=OP.mult)
                if causal:
                    CS = work.tile([128, 1024], F32, tag="CSchunk", name="CSchunk",
                                   bufs=2)
                    nc.scalar.dma_start(CS[:], causD[:, jc * 1024:(jc + 1) * 1024])
                    nc.gpsimd.tensor_tensor(out=E[:], in0=E[:], in1=CS[:], op=OP.add)
                nc.vector.tensor_tensor(
                    out=E3, in0=E3,
                    in1=M[:, jsl][:, :, None].broadcast_to([128, 16, 64]),
                    op=OP.subtract)
                nc.scalar.activation(E[:], E[:], ACTF.Exp)
                nc.vector.tensor_reduce(out=Zrec[:, jsl], in_=E3, axis=AX.X,
                                        op=OP.add)
                nc.vector.reciprocal(Zrec[:, jsl], Zrec[:, jsl])
                nc.gpsimd.tensor_tensor(
                    out=E3, in0=E3,
                    in1=Zrec[:, jsl][:, :, None].broadcast_to([128, 16, 64]),
                    op=OP.mult)
                for jb in range(0, 16, 4):
                    ps = psB.tile([64, 512], F32, tag="psb", name="psb")
                    for q in range(4):
                        nc.tensor.transpose(
                            ps[:, q * 128:(q + 1) * 128],
                            E[:, (jb + q) * 64:(jb + q + 1) * 64], I128[:])
                    copy_ps(PT[:, (jc * 16 + jb) * 128:(jc * 16 + jb + 4) * 128],
                            ps[:])

            # PV for this parity: half-banks [64, 512], pairs (h, q=b)
            for b in range(RT):
                vt = work.tile([64, D], F32, tag="Vload", name="Vload")
                nc.scalar.dma_start(vt[:], vD[(2 * b + p) * 64:(2 * b + p + 1) * 64, :])
                bank = psA.tile([64, 512], F32, tag="psa", name="psa")
                for h in range(NH):
                    pr = h * 16 + b
                    nc.tensor.matmul(
                        bank[:, h * 64:(h + 1) * 64],
                        lhsT=PT4[:, :, pr],
                        rhs=vt[:, h * 64:(h + 1) * 64],
                        start=True, stop=True)
                stag = work.tile([64, 512], F32, tag="stag", name="stag")
                copy_ps(stag[:], bank[:])
                for h in range(NH):
                    base = (2 * b + p) * 64 + h * 8
                    nc.sync.dma_start(
                        aD[base:base + 8, :],
                        stag[:, h * 64:(h + 1) * 64])

    # ---------- residual + LN from aD -------------------------------------
    def resid_ln(other_nat_cb, out_cb):
        def pre_fn(rt):
            at = work.tile([128, D], F32, tag="aload", name="aload")
            nc.sync.dma_start(at[:], aD[rt * 128:(rt + 1) * 128, :])
            pt = preQ.tile([128, D], F32, tag="pre", name="pre")
            nc.vector.tensor_tensor(out=pt[:], in0=at[:], in1=other_nat_cb(rt),
                                    op=OP.add)
            return pt[:]
        for g in range(RT // 4):
            ln_group4(g, pre_fn, out_cb)

    def ln_out_to_TD(dst_dram, also_nat_dram=None):
        """LN out_cb that immediately transposes each tile into dst_dram."""
        def cb(rt, src, negmu, rstd):
            ot = work.tile([128, D], F32, tag="lnout", name="lnout", bufs=4)
            nc.vector.tensor_scalar(out=ot[:], in0=src, scalar1=negmu,
                                    scalar2=rstd, op0=OP.add, op1=OP.mult)
            if also_nat_dram is not None:
                nc.sync.dma_start(also_nat_dram[rt * 128:(rt + 1) * 128, :], ot[:])
            ps = psB.tile([128, 512], F32, tag="psb", name="psb")
            for cb_ in range(4):
                nc.tensor.transpose(ps[:, cb_ * 128:(cb_ + 1) * 128],
                                    ot[:, cb_ * 128:(cb_ + 1) * 128], I128[:])
            t = work.tile([128, 512], F32, tag="toD", name="toD", bufs=2)
            copy_ps(t[:], ps[:])
            nc.sync.dma_start(
                dst_dram[:, :, rt * 128:(rt + 1) * 128].rearrange("c a r -> a c r"),
                t[:].rearrange("a (c r) -> a c r", c=4))
        return cb

    # ---------- FFN ---------------------------------------------------------
    def ffn(xTd, resTd, w1_ap, b1_ap, w2_ap, b2_ap, out_cb):
        b2 = small.tile([1, D], F32, tag="b2", name="b2")
        nc.sync.dma_start(b2[:], b2_ap[:])
        for rc in range(4):
            xcs = []
            for dt in range(DT):
                xc = work.tile([128, 512], F32, tag=f"xfc{dt}", name=f"xfc{dt}",
                               bufs=1)
                nc.sync.dma_start(xc[:], xTd[dt, :, rc * 512:(rc + 1) * 512])
                xcs.append(xc)
            ps2 = [psB.tile([128, 512], F32, tag="psb", name="psb")
                   for _ in range(4)]
            for ff in range(FT):
                w1f = work.tile([128, 512], F32, tag="w1f", name="w1f")
                nc.scalar.dma_start(
                    w1f[:].rearrange("a (d c) -> a d c", d=4),
                    w1_ap[:, ff * 128:(ff + 1) * 128]
                        .rearrange("(d a) c -> a d c", d=4))
                b1f = small.tile([1, 128], F32, tag="b1f", name="b1f", bufs=3)
                nc.sync.dma_start(b1f[:], b1_ap[:, ff * 128:(ff + 1) * 128])
                ps1 = psA.tile([128, 512], F32, tag="psa", name="psa")
                for dt in range(DT):
                    nc.tensor.matmul(ps1[:],
                                     lhsT=w1f[:, dt * 128:(dt + 1) * 128],
                                     rhs=xcs[dt][:], start=(dt == 0), stop=False)
                nc.tensor.matmul(ps1[:], lhsT=b1f[:], rhs=ones1[:, 0:512],
                                 start=False, stop=True)
                f1f = work.tile([128, 512], F32, tag="f1f", name="f1f")
                nc.scalar.activation(f1f[:], ps1[:], ACTF.Relu)
                w2f = work.tile([128, 512], F32, tag="w2f", name="w2f")
                nc.sync.dma_start(w2f[:], w2_ap[ff * 128:(ff + 1) * 128, :])
                for rl in range(4):
                    nc.tensor.matmul(ps2[rl][:],
                                     lhsT=f1f[:, rl * 128:(rl + 1) * 128],
                                     rhs=w2f[:], start=(ff == 0), stop=False)
            def pre_fn(rt):
                rl = rt % 4
                nc.tensor.matmul(ps2[rl][:], lhsT=ones1[:, 0:128], rhs=b2[:],
                                 start=False, stop=False)
                for ct in range(DT):
                    rtl = work.tile([128, 128], F32, tag="rload", name="rload",
                                    bufs=4)
                    nc.scalar.dma_start(rtl[:], resTd[ct, :, rt * 128:(rt + 1) * 128])
                    nc.tensor.matmul(ps2[rl][:, ct * 128:(ct + 1) * 128],
                                     lhsT=rtl[:], rhs=I128[:], start=False,
                                     stop=(ct == DT - 1))
                pt = preQ.tile([128, D], F32, tag="pre", name="pre")
                copy_ps(pt[:], ps2[rl][:])
                return pt[:]
            ln_group4(rc, pre_fn, out_cb)

    # ======================= pipeline =======================
    # P1: dec1 (causal) on x_de
    embed_T_toD(hi['XdT'], xTd['xd'])
    attention(xTd['xd'], xTd['xd'], gw['dec_wv1'][:], gw['dec_wqk1'][:],
              [hi['dec1_A'][p] for p in range(2)], hi['dec1_scl'],
              hi['dec1_t'], hi['dec1_cs'], True)
    resid_ln(lambda rt: embed_nat_ps(hi['XdT'], rt)[:],
             ln_out_to_TD(xTd['m'], also_nat_dram=mnD))

    # P2: encoder self-attn on x_en
    embed_T_toD(hi['XeT'], xTd['xe'])
    attention(xTd['xe'], xTd['xe'], gw['enc_wv'][:], gw['enc_wqk'][:],
              [hi['enc_A'][p] for p in range(2)], hi['enc_scl'],
              hi['enc_t'], hi['enc_cs'], False)
    resid_ln(lambda rt: embed_nat_ps(hi['XeT'], rt)[:], ln_out_to_TD(xTd['o1']))

    # P3: encoder FFN
    ffn(xTd['o1'], xTd['o1'], gw['enc_w1'][:], hi['enc_b1'], gw['enc_w2'][:],
        hi['enc_b2'], ln_out_to_TD(xTd['eo']))

    # P4: dec2 cross-attn
    attention(xTd['m'], xTd['eo'], gw['dec_wv2'][:], gw['dec_wqk2'][:],
              [hi['dec2_A'][p] for p in range(2)], hi['dec2_scl'],
              hi['dec2_t'], hi['dec2_cs'], False)

    def m_reload(rt):
        t = work.tile([128, D], F32, tag="mload", name="mload", bufs=2)
        nc.sync.dma_start(t[:], mnD[rt * 128:(rt + 1) * 128, :])
        return t[:]
    resid_ln(m_reload, ln_out_to_TD(xTd['c']))

    # P5: decoder FFN
    ffn(xTd['c'], xTd['c'], gw['dec_w1'][:], hi['dec_b1'], gw['dec_w2'][:],
        hi['dec_b2'], ln_out_to_TD(xTd['of']))

    # P6: final projection + softmax (output ships as bf16)
    Wo = wpool.tile([128, 4 * 64], F32, tag="Wo", name="Wo")
    for dt in range(DT):
        nc.sync.dma_start(Wo[:, dt * 64:(dt + 1) * 64],
                          gw['W_out'][dt * 128:(dt + 1) * 128, :])
    Bo = small.tile([1, 64], F32, tag="Bo", name="Bo")
    nc.sync.dma_start(Bo[:], hi['B_out'][:])
    for rt in range(RT):
        ps = psB.tile([128, 64], F32, tag="psbq", name="psbo", bufs=1)
        for dt in range(DT):
            ol = work.tile([128, 128], F32, tag="rload", name="rload", bufs=4)
            nc.sync.dma_start(ol[:], xTd['of'][dt, :, rt * 128:(rt + 1) * 128])
            nc.tensor.matmul(ps[:], lhsT=ol[:], rhs=Wo[:, dt * 64:(dt + 1) * 64],
                             start=(dt == 0), stop=False)
        nc.tensor.matmul(ps[:], lhsT=ones1[:, 0:128], rhs=Bo[:],
                         start=False, stop=True)
        mx = small.tile([128, 1], F32, tag="mx", name="mx")
        nc.vector.tensor_reduce(out=mx[:], in_=ps[:], axis=AX.X, op=OP.max,
                                negate=True)
        ex = work.tile([128, 64], F32, tag="ex", name="ex")
        nc.scalar.activation(ex[:], ps[:], ACTF.Exp, bias=mx[:])
        zs = small.tile([128, 1], F32, tag="zs", name="zs")
        nc.vector.tensor_reduce(out=zs[:], in_=ex[:], axis=AX.X, op=OP.add)
        rz = small.tile([128, 1], F32, tag="rz", name="rz")
        nc.vector.reciprocal(rz[:], zs[:])
        oo = work.tile([128, 64], F32, tag="oo", name="oo")
        nc.vector.tensor_scalar(out=oo[:], in0=ex[:], scalar1=rz[:],
                                scalar2=None, op0=OP.mult)
        oo16 = work.tile([128, 64], F16, tag="oo16", name="oo16")
        nc.vector.tensor_copy(oo16[:], oo[:])
        nc.sync.dma_start(out_ap[rt * 128:(rt + 1) * 128, :], oo16[:])


# ============================================================================
# 8-core SPMD wrapper with a cached PJRT dispatcher: kernel(**inputs) -> out
# ============================================================================
_CACHE = {}


def _get_program():
    if 'nc' not in _CACHE:
        nc = bacc.Bacc("TRN2", target_bir_lowering=False, debug=False)
        hi, out_ap = declare_io(nc)
        with tile.TileContext(nc, trace_sim=False) as tc:
            with ExitStack() as ctx:
                build(ctx, tc, hi, out_ap)
        nc.compile()
        _CACHE['nc'] = nc
    return _CACHE['nc']


def _get_dispatcher():
    """One cached jit(shard_map(...)) wrapper -- same execution path as
    bass_utils.run_bass_kernel_spmd under axon (bass2jax/_bass_exec_p via
    PJRT), but without rebuilding/retracing the wrapper on every call."""
    if 'disp' in _CACHE:
        return _CACHE['disp']
    import jax
    from jax.sharding import Mesh, PartitionSpec
    from jax.experimental.shard_map import shard_map
    from concourse import bass2jax

    nc = _get_program()
    bass2jax.install_neuronx_cc_hook()
    partition_name = (nc.partition_id_tensor.name
                      if nc.partition_id_tensor else None)
    in_names, out_names, out_avals, zero_tmpl = [], [], [], []
    for alloc in nc.m.functions[0].allocations:
        if not isinstance(alloc, mybir.MemoryLocationSet):
            continue
        name = alloc.memorylocations[0].name
        if alloc.kind == "ExternalInput":
            if name != partition_name:
                in_names.append(name)
        elif alloc.kind == "ExternalOutput":
            shape = tuple(alloc.tensor_shape)
            dtype = mybir.dt.np(alloc.dtype)
            out_avals.append(jax.core.ShapedArray(shape, dtype))
            zero_tmpl.append((shape, dtype))
            out_names.append(name)
    n_params = len(in_names)
    n_outs = len(out_avals)
    all_in_names = list(in_names) + list(out_names)
    if partition_name is not None:
        all_in_names.append(partition_name)
    donate = tuple(range(n_params, n_params + n_outs))

    def _body(*args):
        operands = list(args)
        if partition_name is not None:
            operands.append(bass2jax.partition_id_tensor())
        outs = bass2jax._bass_exec_p.bind(
            *operands, out_avals=tuple(out_avals),
            in_names=tuple(all_in_names), out_names=tuple(out_names),
            lowering_input_output_aliases=(), sim_require_finite=True,
            sim_require_nnan=True, nc=nc)
        return tuple(outs)

    devices = jax.devices()[:8]
    mesh = Mesh(np.asarray(devices), ("core",))
    sharded = jax.jit(
        shard_map(_body, mesh=mesh,
                  in_specs=(PartitionSpec("core"),) * (n_params + n_outs),
                  out_specs=(PartitionSpec("core"),) * n_outs,
                  check_rep=False),
        donate_argnums=donate, keep_unused=True)

    # donated output buffers are allocated+zeroed ON DEVICE (no tunnel bytes)
    import jax.numpy as jnp
    from jax.sharding import NamedSharding
    zsh = NamedSharding(mesh, PartitionSpec("core"))
    zfn = jax.jit(
        lambda: tuple(jnp.zeros((8 * s[0], *s[1:]), d) for (s, d) in zero_tmpl),
        out_shardings=(zsh,) * n_outs)

    def dispatch(in_maps):
        concat_in = [
            np.concatenate([np.asarray(in_maps[c][nm]) for c in range(8)], 0)
            for nm in in_names]
        cz = zfn()
        outs = sharded(*concat_in, *cz)
        return [
            {nm: np.asarray(outs[i]).reshape(8, *out_avals[i].shape)[c]
             for i, nm in enumerate(out_names)}
            for c in range(8)]

    _CACHE['disp'] = dispatch
    return dispatch


def kernel(**inputs):
    dispatch = _get_dispatcher()
    in_maps = [host_inputs(inputs, core) for core in range(8)]
    res = dispatch(in_maps)
    outs = [np.asarray(res[c]['out'], np.float32) for c in range(8)]
    full = np.concatenate(outs, 0)          # [16384, 64] rows = (b, L)
    return full.reshape(64, 256, 64)


# revision 36
# speedup vs baseline: 9.3447x; 1.0698x over previous
"""Bass/Tile kernel for nn_DeepRelativeST on 8 NeuronCores (1/8 data-parallel
shard over the flat (b*L) row axis; 8 batches = 32 contiguous l-blocks per
core, so attention is core-local).

Per-core: R=2048 rows (8 batches x 256 pos), D=512, DFF=2048, H=8, dep=64,
Ll=32 local l values, 256 (l,h) softmax pairs split into two l-parity tiles:
tile p holds pair (h, l=2q+p) at partition h*16+q.

Key math (derived from reference.py):
  qs[l,h,j] = (x @ wq_headsum)[l*64+j, h]     (full Q GEMM never needed)
  ks likewise; V = x @ wv (full GEMM).
  abar[l,h,k,m] = rel[l,h,k,m-k+63] * (m<=k)  (host-gathered skew)
  r1 = sum_m abar*ks ; t = sum_m abar*m (HOST precomputed from rel)
  r2 = r1 + NEG*t ; cu = sc^2 * R1 * qs with R1 = sum_m colsum[m]*ks[m]
  (colsum = sum_k abar[.,k,.] HOST precomputed: keeps cu exact so abar can
   ship at very low precision -- r1's error only shifts logits that softmax
   renders irrelevant; int4 abar validated to 9.2e-4 max rel err end-to-end.)
  logits[j,k] = cu[j]*r2[k] (+ causal NEG mask)
  p = softmax_k ; o = p @ V-block
  out row = l*64 + h*8 + j//8, col = (j%8)*64 + n   (torch raw-reshape scramble)

Transfer plan (the dispatch wall-clock is dominated by the ~30 MB/s serial
axon tunnel, so bytes-on-wire is everything): abar ships as triangle-packed
int4 codes + per-(l,h) fp32 scales, with exact fp32 t/colsum sidecars; X and
the q/k-path weights stay fp32 (attention selection is chaotic -- any
quantization there flips softmax winners); FFN weights + dec_wv2 + W_out
ship fp16 and are upcast on device; all replicated weights ship as 1/8
row-shards and are AllGathered on-device (HBM Shared scratch); the causal
mask is built on device from a [1,4096] row; the donated output buffers are
zeroed on device; output returns as fp16 and is upcast on host.
"""
import numpy as np
from contextlib import ExitStack

import ml_dtypes

import concourse.bass as bass
import concourse.tile as tile
from concourse import bacc
from concourse import mybir

F32 = mybir.dt.float32
FP8 = mybir.dt.float8e4
F16 = mybir.dt.float16
BF16 = mybir.dt.bfloat16
U8 = mybir.dt.uint8
AX = mybir.AxisListType
OP = mybir.AluOpType
ACTF = mybir.ActivationFunctionType

R, D, DFF, NH, DEP, LL = 2048, 512, 2048, 8, 64, 32
NEG, EPS, SC2 = -1e9, 1e-5, 1.0 / 64.0
RT, DT, FT = R // 128, D // 128, DFF // 128
NC8 = [[0, 1, 2, 3, 4, 5, 6, 7]]

# replicated weights: name -> full (rows, cols); shipped as [rows//8, cols]
# REPW32: fp32 (attention-selection critical -- qs/ks path must be exact).
# REPW16: fp16 on the wire, upcast to fp32 on device (FFN/out path; validated
# to ~2e-4 host-side).
REPW32 = {
    'W_in': (64, 512),
    'enc_wv': (512, 512), 'dec_wv1': (512, 512),
    'enc_wqk': (512, 16), 'dec_wqk1': (512, 16), 'dec_wqk2': (512, 16),
    'I128': (128, 128),
}
REPW16 = {
    'W_out': (512, 64), 'dec_wv2': (512, 512),
}
# REPW8: int8 per-row quantized on the wire (value = (code-128)*scale[row]);
# codes ship as 1/8 row-shards + AllGather, fp32 row-scales ship replicated
# (tiny). Validated 8.3e-3 max rel err end-to-end on the host mirror.
REPW8 = {
    'enc_w1': (512, 2048), 'enc_w2': (2048, 512),
    'dec_w1': (512, 2048), 'dec_w2': (2048, 512),
}
# A ships as int4 codes (two per byte), triangle-packed into 8 row-segments
# of 8 k's, each padded to width 8*(s+1): row k in segment s=k//8 keeps
# columns m=0..8(s+1)-1 (superset of the valid m<=k). Within a segment, the
# hi nibble holds rows 8s..8s+3, the lo nibble rows 8s+4..8s+7.
# value = (code - 8) * scale[l,h], scale shipped fp32 per (l,h).
# 1152 bytes/partition vs 4096 dense fp32=16384. r1's precision headroom is
# enormous (selection is set by exact t/colsum sidecars): int4 measured
# 9.2e-4 end-to-end on the host mirror.
NSEG = 8
SEG_OFF = [16 * s * (s + 1) for s in range(NSEG)]
APK = 1152


def host_inputs(inp, core):
    f = lambda k: np.ascontiguousarray(np.asarray(inp[k], np.float32))
    bs = slice(core * 8, core * 8 + 8)
    ls = slice(core * 32, core * 32 + 32)
    Xe = f('X_en')[bs].reshape(R, 64)
    Xd = f('X_de')[bs].reshape(R, 64)

    def wqk_heads(wq, wk):
        a = wq.reshape(D, NH, DEP).sum(-1)
        b = wk.reshape(D, NH, DEP).sum(-1)
        return np.ascontiguousarray(np.concatenate([a, b], 1))  # [512,16]

    km = np.arange(64)
    kk, mm = np.meshgrid(km, km, indexing='ij')   # [k, m]

    def rel_arrange(rel):
        r = rel[ls]                                # [32,8,64,64] = [l,h,k,c]
        # abar[l,h,k,m] = r[l,h,k,m-k+63] if m<=k else 0
        c = mm - kk + 63
        valid = (mm <= kk)
        cs = np.clip(c, 0, 63)
        ab = np.take_along_axis(
            r.reshape(LL, NH, 64, 64), cs.reshape(1, 1, 64, 64), axis=3)
        ab = ab * valid.reshape(1, 1, 64, 64)
        t = (ab * mm.reshape(1, 1, 64, 64)).sum(-1)     # [l,h,k]   exact
        csum = ab.sum(-2)                               # [l,h,m]   exact
        abT = ab.transpose(1, 0, 2, 3)                  # [h,l,k,m]
        tT = t.transpose(1, 0, 2)                       # [h,l,k]
        cT = csum.transpose(1, 0, 2)                    # [h,l,m]
        A4 = np.zeros((2, 128, APK), np.uint8)
        Sc = np.empty((2, 128, 1), np.float32)
        Tt = np.empty((2, 128, 64), np.float32)
        Cs = np.empty((2, 128, 64), np.float32)
        for p in range(2):
            d = abT[:, p::2].reshape(128, 64, 64)          # [a, k, m]
            scale = np.maximum(np.abs(d).max((1, 2)), 1e-30) / 7.0
            codes = (np.clip(np.round(d / scale[:, None, None]), -8, 7)
                     + 8).astype(np.uint8)
            for s in range(NSEG):
                ws = 8 * (s + 1)
                blk = codes[:, 8 * s:8 * (s + 1), 0:ws].reshape(128, 8 * ws)
                half = 4 * ws
                A4[p][:, SEG_OFF[s]:SEG_OFF[s] + half] = \
                    (blk[:, :half] << 4) | blk[:, half:]
            Sc[p] = scale.reshape(128, 1)
            Tt[p] = tT[:, p::2].reshape(128, 64)
            Cs[p] = cT[:, p::2].reshape(128, 64)
        return A4, Sc, Tt, Cs

    A_e, s_e, t_e, c_e = rel_arrange(f('enc_rel'))
    A_d1, s_d1, t_d1, c_d1 = rel_arrange(f('dec_rel1'))
    A_d2, s_d2, t_d2, c_d2 = rel_arrange(f('dec_rel2'))
    caus_row = np.ascontiguousarray(
        np.triu(np.full((64, 64), NEG, np.float32), 1).reshape(1, 4096))

    out = {
        'XeT': np.ascontiguousarray(Xe.T), 'XdT': np.ascontiguousarray(Xd.T),
        'B_in': f('B_in').reshape(1, D),
        'enc_A': A_e, 'enc_scl': s_e, 'enc_t': t_e, 'enc_cs': c_e,
        'dec1_A': A_d1, 'dec1_scl': s_d1, 'dec1_t': t_d1, 'dec1_cs': c_d1,
        'dec2_A': A_d2, 'dec2_scl': s_d2, 'dec2_t': t_d2, 'dec2_cs': c_d2,
        'enc_b1': f('enc_b1').reshape(1, DFF), 'enc_b2': f('enc_b2').reshape(1, D),
        'dec_b1': f('dec_b1').reshape(1, DFF), 'dec_b2': f('dec_b2').reshape(1, D),
        'B_out': f('B_out').reshape(1, 64),
        'caus_row': caus_row,
    }
    fulls = {
        'W_in': f('W_in'),
        'enc_wv': f('enc_wv'), 'dec_wv1': f('dec_wv1'), 'dec_wv2': f('dec_wv2'),
        'enc_wqk': wqk_heads(f('enc_wq'), f('enc_wk')),
        'dec_wqk1': wqk_heads(f('dec_wq1'), f('dec_wk1')),
        'dec_wqk2': wqk_heads(f('dec_wq2'), f('dec_wk2')),
        'I128': np.eye(128, dtype=np.float32),
    }
    for nm, (r, c) in REPW32.items():
        sh = r // 8
        out[nm] = np.ascontiguousarray(fulls[nm][core * sh:(core + 1) * sh])
    for nm, (r, c) in REPW16.items():
        sh = r // 8
        out[nm] = np.ascontiguousarray(
            f(nm)[core * sh:(core + 1) * sh].astype(np.float16))
    for nm, (r, c) in REPW8.items():
        w = f(nm)
        scale = np.maximum(np.abs(w).max(1, keepdims=True), 1e-30) / 127.0
        codes = (np.clip(np.round(w / scale), -127, 127) + 128).astype(np.uint8)
        sh = r // 8
        out[nm] = np.ascontiguousarray(codes[core * sh:(core + 1) * sh])
        out[nm + '_scl'] = np.ascontiguousarray(scale.astype(np.float32))
    return out


IN_SHAPES = {
    'XeT': ((64, R), F32), 'XdT': ((64, R), F32), 'B_in': ((1, D), F32),
    'enc_A': ((2, 128, APK), U8), 'dec1_A': ((2, 128, APK), U8),
    'dec2_A': ((2, 128, APK), U8),
    'enc_scl': ((2, 128, 1), F32), 'dec1_scl': ((2, 128, 1), F32),
    'dec2_scl': ((2, 128, 1), F32),
    'enc_t': ((2, 128, 64), F32), 'dec1_t': ((2, 128, 64), F32),
    'dec2_t': ((2, 128, 64), F32),
    'enc_cs': ((2, 128, 64), F32), 'dec1_cs': ((2, 128, 64), F32),
    'dec2_cs': ((2, 128, 64), F32),
    'enc_b1': ((1, DFF), F32), 'enc_b2': ((1, D), F32),
    'dec_b1': ((1, DFF), F32), 'dec_b2': ((1, D), F32),
    'B_out': ((1, 64), F32), 'caus_row': ((1, 4096), F32),
    **{nm: ((r // 8, c), F32) for nm, (r, c) in REPW32.items()},
    **{nm: ((r // 8, c), F16) for nm, (r, c) in REPW16.items()},
    **{nm: ((r // 8, c), U8) for nm, (r, c) in REPW8.items()},
    **{nm + '_scl': ((r, 1), F32) for nm, (r, c) in REPW8.items()},
}


def declare_io(nc):
    hi = {k: nc.dram_tensor(k, list(s), dt, kind="ExternalInput").ap()
          for k, (s, dt) in IN_SHAPES.items()}
    out = nc.dram_tensor('out', [R, 64], F16, kind="ExternalOutput").ap()
    return hi, out


def build(ctx: ExitStack, tc: tile.TileContext, hi, out_ap, dbg=None):
    nc = tc.nc
    consts = ctx.enter_context(tc.tile_pool(name="consts", bufs=1))
    wpool = ctx.enter_context(tc.tile_pool(name="wpool", bufs=1))
    work = ctx.enter_context(tc.tile_pool(name="work", bufs=3))
    preQ = ctx.enter_context(tc.tile_pool(name="preQ", bufs=8))
    small = ctx.enter_context(tc.tile_pool(name="small", bufs=1))
    bigP = ctx.enter_context(tc.tile_pool(name="bigP", bufs=1))
    psA = ctx.enter_context(tc.tile_pool(name="psA", bufs=3, space="PSUM"))
    psB = ctx.enter_context(tc.tile_pool(name="psB", bufs=4, space="PSUM"))
    dram = ctx.enter_context(tc.tile_pool(name="dram", bufs=1, space="DRAM"))

    # ---------- gather replicated weights from 1/8 shards -------------------
    gw = {}
    for nm, (r, c) in REPW32.items():
        loc = dram.tile([r // 8, c], F32, tag=f"agl_{nm}", name=f"agl_{nm}")
        nc.sync.dma_start(loc[:], hi[nm][:])
        full = dram.tile([r, c], F32, addr_space="Shared",
                         tag=f"agf_{nm}", name=f"agf_{nm}")
        nc.gpsimd.collective_compute(
            "AllGather", OP.bypass, replica_groups=NC8,
            ins=[loc[:]], outs=[full[:]])
        gw[nm] = full
    for nm, (r, c) in REPW16.items():
        loc = dram.tile([r // 8, c], F16, tag=f"agl_{nm}", name=f"agl_{nm}")
        nc.sync.dma_start(loc[:], hi[nm][:])
        full16 = dram.tile([r, c], F16, addr_space="Shared",
                           tag=f"agh_{nm}", name=f"agh_{nm}")
        nc.gpsimd.collective_compute(
            "AllGather", OP.bypass, replica_groups=NC8,
            ins=[loc[:]], outs=[full16[:]])
        full = dram.tile([r, c], F32, tag=f"agf_{nm}", name=f"agf_{nm}")
        for r0 in range(0, r, 128):
            for c0 in range(0, c, 512):
                cw = min(512, c - c0)
                t16 = work.tile([128, 512], F16, tag="u16", name="u16", bufs=2)
                nc.sync.dma_start(t16[:, 0:cw],
                                  full16[r0:r0 + 128, c0:c0 + cw])
                t32 = work.tile([128, 512], F32, tag="xcT", name="u32")
                nc.vector.tensor_copy(t32[:, 0:cw], t16[:, 0:cw])
                nc.sync.dma_start(full[r0:r0 + 128, c0:c0 + cw], t32[:, 0:cw])
        gw[nm] = full
    for nm, (r, c) in REPW8.items():
        loc = dram.tile([r // 8, c], U8, tag=f"agl_{nm}", name=f"agl_{nm}")
        nc.sync.dma_start(loc[:], hi[nm][:])
        full8 = dram.tile([r, c], U8, addr_space="Shared",
                          tag=f"agh_{nm}", name=f"agh_{nm}")
        nc.gpsimd.collective_compute(
            "AllGather", OP.bypass, replica_groups=NC8,
            ins=[loc[:]], outs=[full8[:]])
        full = dram.tile([r, c], F32, tag=f"agf_{nm}", name=f"agf_{nm}")
        for r0 in range(0, r, 128):
            scl = work.tile([128, 1], F32, tag="w8scl", name="w8scl", bufs=2)
            nc.sync.dma_start(scl[:], hi[nm + '_scl'][r0:r0 + 128, :])
            for c0 in range(0, c, 512):
                t8 = work.tile([128, 512], U8, tag="u8w", name="u8w", bufs=2)
                nc.sync.dma_start(t8[:], full8[r0:r0 + 128, c0:c0 + 512])
                t32 = work.tile([128, 512], F32, tag="xcT", name="u32b")
                nc.vector.tensor_copy(t32[:], t8[:])
                nc.vector.tensor_scalar(out=t32[:], in0=t32[:], scalar1=128.0,
                                        scalar2=scl[:, 0:1], op0=OP.subtract,
                                        op1=OP.mult)
                nc.sync.dma_start(full[r0:r0 + 128, c0:c0 + 512], t32[:])
        gw[nm] = full

    I128 = consts.tile([128, 128], F32, tag="I128", name="I128")
    nc.sync.dma_start(I128[:], gw['I128'][:])
    ones1 = consts.tile([1, D], F32, tag="ones1", name="ones1")
    nc.vector.memset(ones1[:], 1.0)
    epsc = consts.tile([128, 1], F32, tag="epsc", name="epsc")
    nc.vector.memset(epsc[:], EPS)
    W_in = consts.tile([64, D], F32, tag="W_in", name="W_in")
    nc.sync.dma_start(W_in[:], gw['W_in'][:])
    B_in = consts.tile([1, D], F32, tag="B_in", name="B_in")
    nc.sync.dma_start(B_in[:], hi['B_in'][:])

    # causal mask [128, 4096] built on device from the [1,4096] row into
    # DRAM scratch (PE partition-broadcast), streamed back at use like the
    # baseline's shipped CAUS.
    causD = dram.tile([128, 4096], F32, tag="causD", name="causD")
    for q in range(8):
        cr = work.tile([1, 512], F32, tag="xin", name="crowc")
        nc.sync.dma_start(cr[:], hi['caus_row'][:, q * 512:(q + 1) * 512])
        ps = psA.tile([128, 512], F32, tag="psa", name="psa")
        nc.tensor.matmul(ps[:], lhsT=ones1[:, 0:128], rhs=cr[:],
                         start=True, stop=True)
        st = work.tile([128, 512], F32, tag="toD", name="toD", bufs=2)
        nc.scalar.copy(st[:], ps[:])
        nc.sync.dma_start(causD[:, q * 512:(q + 1) * 512], st[:])

    # DRAM scratch: transposed activations live here, streamed at use.
    xTd = {nm: dram.tile([DT, 128, R], F32, tag=f"xTd_{nm}", name=f"xTd_{nm}")
           for nm in ('xe', 'xd', 'm', 'o1', 'eo', 'c', 'of')}
    aD = dram.tile([R, D], F32, tag="aD", name="aD")
    vD = dram.tile([R, D], F32, tag="vD", name="vD")
    mnD = dram.tile([R, D], F32, tag="mnD", name="mnD")

    def copy_ps(dst, src):
        nc.scalar.copy(dst, src)

    # ---------- embed: x.T = (X@W_in+B).T streamed to DRAM ------------------
    def embed_T_toD(x_in_ap, dst):
        for ct in range(DT):
            for rc in range(4):
                xin = work.tile([64, 512], F32, tag="xin", name="xin")
                nc.sync.dma_start(xin[:], x_in_ap[:, rc * 512:(rc + 1) * 512])
                ps = psA.tile([128, 512], F32, tag="psa", name="psa")
                nc.tensor.matmul(ps[:], lhsT=W_in[:, ct * 128:(ct + 1) * 128],
                                 rhs=xin[:], start=True, stop=False)
                nc.tensor.matmul(ps[:], lhsT=B_in[:, ct * 128:(ct + 1) * 128],
                                 rhs=ones1[:, 0:512], start=False, stop=True)
                t = work.tile([128, 512], F32, tag="toD", name="toD", bufs=2)
                copy_ps(t[:], ps[:])
                nc.sync.dma_start(dst[ct, :, rc * 512:(rc + 1) * 512], t[:])

    def embed_nat_ps(x_in_ap, rt):
        xin = work.tile([64, 128], F32, tag="xin2", name="xin2")
        nc.sync.dma_start(xin[:], x_in_ap[:, rt * 128:(rt + 1) * 128])
        ps = psA.tile([128, 512], F32, tag="psa", name="psa")
        nc.tensor.matmul(ps[:], lhsT=xin[:], rhs=W_in[:], start=True, stop=False)
        nc.tensor.matmul(ps[:], lhsT=ones1[:, 0:128], rhs=B_in[:],
                         start=False, stop=True)
        return ps

    # ---------- layernorm over one group of 4 row-tiles ---------------------
    def ln_group4(g, pre_fn, out_cb):
        """pre_fn(rt) -> [128,512] AP (lazy); out_cb(rt, src, nmu, rstd)."""
        if True:
            sx = small.tile([128, 4], F32, tag="sx", name="sx", bufs=2)
            sx2 = small.tile([128, 4], F32, tag="sx2", name="sx2", bufs=2)
            pres = []
            for i in range(4):
                pa = pre_fn(g * 4 + i)
                pres.append(pa)
                scr = work.tile([128, D], F32, tag="lnscr", name="lnscr")
                nc.scalar.activation(scr[:], pa, ACTF.Copy,
                                     accum_out=sx[:, i:i + 1])
                nc.scalar.activation(scr[:], pa, ACTF.Square,
                                     accum_out=sx2[:, i:i + 1])
            negmu = small.tile([128, 4], F32, tag="negmu", name="negmu", bufs=2)
            nc.vector.tensor_scalar(out=negmu[:], in0=sx[:], scalar1=-1.0 / D,
                                    scalar2=None, op0=OP.mult)
            mu2 = small.tile([128, 4], F32, tag="mu2", name="mu2", bufs=2)
            nc.vector.tensor_tensor(out=mu2[:], in0=negmu[:], in1=negmu[:],
                                    op=OP.mult)
            var = small.tile([128, 4], F32, tag="var", name="var", bufs=2)
            nc.vector.scalar_tensor_tensor(out=var[:], in0=sx2[:],
                                           scalar=1.0 / D, in1=mu2[:],
                                           op0=OP.mult, op1=OP.subtract)
            std = small.tile([128, 4], F32, tag="std", name="std", bufs=2)
            nc.scalar.activation(std[:], var[:], ACTF.Sqrt, bias=epsc[:])
            rstd = small.tile([128, 4], F32, tag="rstd", name="rstd", bufs=2)
            nc.vector.reciprocal(rstd[:], std[:])
            for i in range(4):
                out_cb(g * 4 + i, pres[i], negmu[:, i:i + 1], rstd[:, i:i + 1])

    # ---------- attention ---------------------------------------------------
    def attention(xqTd, xkvTd, wv_ap, wqk_ap, A_ap, scl_ap, t_ap, cs_ap, causal):
        # V GEMM (x.T-stationary tiles streamed from DRAM) -> vD
        wv = wpool.tile([128, 4 * D], F32, tag="wv", name="wv")
        for dt in range(DT):
            nc.sync.dma_start(wv[:, dt * D:(dt + 1) * D],
                              wv_ap[dt * 128:(dt + 1) * 128, :])
        for rt in range(RT):
            ps = psA.tile([128, 512], F32, tag="psa", name="psa")
            for dt in range(DT):
                xl = work.tile([128, 128], F32, tag="xlT", name="xlT")
                nc.sync.dma_start(xl[:], xkvTd[dt, :, rt * 128:(rt + 1) * 128])
                nc.tensor.matmul(ps[:], lhsT=xl[:],
                                 rhs=wv[:, dt * D:(dt + 1) * D],
                                 start=(dt == 0), stop=(dt == DT - 1))
            vt = work.tile([128, D], F32, tag="Vtile", name="Vtile")
            copy_ps(vt[:], ps[:])
            nc.sync.dma_start(vD[rt * 128:(rt + 1) * 128, :], vt[:])

        # qs / ks GEMMs (W-stationary, M=8)
        wqk = wpool.tile([128, 4 * 16], F32, tag="wqk", name="wqk")
        for dt in range(DT):
            nc.sync.dma_start(wqk[:, dt * 16:(dt + 1) * 16],
                              wqk_ap[dt * 128:(dt + 1) * 128, :])
        qT = work.tile([8, R], F32, tag="qT", name="qT", bufs=1)
        kT = work.tile([8, R], F32, tag="kT", name="kT", bufs=1)
        for (dst, colofs, srcTd) in ((qT, 0, xqTd), (kT, 8, xkvTd)):
            for rc in range(4):
                ps = psB.tile([8, 512], F32, tag="psbq", name="psbq", bufs=1)
                for dt in range(DT):
                    xc = work.tile([128, 512], F32, tag="xcT", name="xcT")
                    nc.sync.dma_start(xc[:], srcTd[dt, :, rc * 512:(rc + 1) * 512])
                    nc.tensor.matmul(
                        ps[:], lhsT=wqk[:, dt * 16 + colofs: dt * 16 + colofs + 8],
                        rhs=xc[:], start=(dt == 0), stop=(dt == DT - 1))
                copy_ps(dst[:, rc * 512:(rc + 1) * 512], ps[:])

        qs_pp = small.tile([128, 2 * 64], F32, tag="qs_pp", name="qs_pp")
        ks_pp = small.tile([128, 2 * 64], F32, tag="ks_pp", name="ks_pp")
        qD = dram.tile([8, R], F32, tag="qD", name="qD")
        kD = dram.tile([8, R], F32, tag="kD", name="kD")
        for (src, bounce, dst) in ((qT, qD, qs_pp), (kT, kD, ks_pp)):
            nc.sync.dma_start(bounce[:], src[:])
            nc.sync.dma_start(
                dst[:], bounce[:].rearrange("h (q f) -> (h q) f", q=16))

        # r1 = sum_m abar*ks. abar arrives as int4 nibble pairs, triangle-
        # packed in 8 segments of 8 k-rows padded to width 8(s+1); unpack
        # (shift/mask -> u8->f32 -> fused (x-8)*scale) then mult-reduce.
        scl_pp = small.tile([128, 2], F32, tag="scl_pp", name="scl_pp")
        nc.sync.dma_start(scl_pp[:].rearrange("a (p k) -> a p k", p=2),
                          scl_ap[:].rearrange("p a k -> a p k"))
        r1 = small.tile([128, 2 * 64], F32, tag="r1", name="r1")
        for p in range(2):
            for s in range(NSEG):
                ws = 8 * (s + 1)
                width = 8 * ws
                half = 4 * ws
                off = SEG_OFF[s]
                A4t = work.tile([128, 256], U8, tag="A8chunk", name="A8chunk",
                                bufs=1)
                nc.scalar.dma_start(A4t[:, 0:half], A_ap[p][:, off:off + half])
                hiu = work.tile([128, 256], U8, tag="hiu", name="hiu", bufs=1)
                nc.vector.tensor_scalar(out=hiu[:, 0:half], in0=A4t[:, 0:half],
                                        scalar1=4, scalar2=None,
                                        op0=OP.logical_shift_right)
                lou = work.tile([128, 256], U8, tag="lou", name="lou", bufs=1)
                nc.vector.tensor_scalar(out=lou[:, 0:half], in0=A4t[:, 0:half],
                                        scalar1=15, scalar2=None,
                                        op0=OP.bitwise_and)
                A = work.tile([128, 512], F32, tag="Achunk", name="Achunk", bufs=1)
                nc.vector.tensor_copy(A[:, 0:half], hiu[:, 0:half])
                nc.vector.tensor_copy(A[:, half:width], lou[:, 0:half])
                nc.vector.tensor_scalar(out=A[:, 0:width], in0=A[:, 0:width],
                                        scalar1=8.0, scalar2=scl_pp[:, p:p + 1],
                                        op0=OP.subtract, op1=OP.mult)
                A3 = A[:, 0:width].rearrange("a (k m) -> a k m", k=8)
                nc.gpsimd.tensor_tensor(
                    out=A3, in0=A3,
                    in1=ks_pp[:, p * 64:p * 64 + ws][:, None, :]
                        .broadcast_to([128, 8, ws]), op=OP.mult)
                nc.vector.tensor_reduce(
                    out=r1[:, p * 64 + s * 8: p * 64 + (s + 1) * 8],
                    in_=A3, axis=AX.X, op=OP.add)
        tH = small.tile([128, 2 * 64], F32, tag="tH", name="tH")
        nc.sync.dma_start(tH[:].rearrange("a (p k) -> a p k", p=2),
                          t_ap[:].rearrange("p a k -> a p k"))
        r2 = small.tile([128, 2 * 64], F32, tag="r2", name="r2")
        nc.vector.scalar_tensor_tensor(out=r2[:], in0=tH[:], scalar=NEG,
                                       in1=r1[:], op0=OP.mult, op1=OP.add)
        # R1 exact via host colsum: R1[p] = sum_m colsum[m]*ks[m]
        csH = small.tile([128, 2 * 64], F32, tag="csH", name="csH")
        nc.sync.dma_start(csH[:].rearrange("a (p k) -> a p k", p=2),
                          cs_ap[:].rearrange("p a k -> a p k"))
        csk = small.tile([128, 2 * 64], F32, tag="csk", name="csk")
        nc.vector.tensor_tensor(out=csk[:], in0=csH[:], in1=ks_pp[:], op=OP.mult)
        R1s = small.tile([128, 2], F32, tag="R1s", name="R1s")
        nc.vector.tensor_reduce(out=R1s[:],
                                in_=csk[:].rearrange("a (p k) -> a p k", p=2),
                                axis=AX.X, op=OP.add)
        nc.vector.tensor_scalar(out=R1s[:], in0=R1s[:], scalar1=SC2,
                                scalar2=None, op0=OP.mult)
        cu = small.tile([128, 2 * 64], F32, tag="cu", name="cu")
        for p in range(2):
            nc.vector.tensor_scalar(out=cu[:, p * 64:(p + 1) * 64],
                                    in0=qs_pp[:, p * 64:(p + 1) * 64],
                                    scalar1=R1s[:, p:p + 1], scalar2=None,
                                    op0=OP.mult)

        # M = rowmax of logits (rank-1 trick; scans for causal)
        M = small.tile([128, 2 * 64], F32, tag="Mm", name="Mm")
        t1 = small.tile([128, 64], F32, tag="Mt1", name="Mt1")
        t2 = small.tile([128, 64], F32, tag="Mt2", name="Mt2")
        if not causal:
            wmax = small.tile([128, 2], F32, tag="wmax", name="wmax")
            wmin = small.tile([128, 2], F32, tag="wmin", name="wmin")
            nc.vector.tensor_reduce(out=wmax[:],
                                    in_=r2[:].rearrange("a (p k) -> a p k", p=2),
                                    axis=AX.X, op=OP.max)
            nc.vector.tensor_reduce(out=wmin[:],
                                    in_=r2[:].rearrange("a (p k) -> a p k", p=2),
                                    axis=AX.X, op=OP.min)
            for p in range(2):
                sl = slice(p * 64, (p + 1) * 64)
                nc.vector.tensor_scalar(out=M[:, sl], in0=cu[:, sl],
                                        scalar1=wmax[:, p:p + 1], scalar2=None,
                                        op0=OP.mult)
                nc.vector.tensor_scalar(out=t1[:], in0=cu[:, sl],
                                        scalar1=wmin[:, p:p + 1], scalar2=None,
                                        op0=OP.mult)
                nc.vector.tensor_tensor(out=M[:, sl], in0=M[:, sl], in1=t1[:],
                                        op=OP.max)
        else:
            pm = small.tile([128, 128], F32, tag="pm", name="pm")
            pn = small.tile([128, 128], F32, tag="pn", name="pn")
            sm = small.tile([128, 128], F32, tag="sm", name="sm")
            sn = small.tile([128, 128], F32, tag="sn", name="sn")
            for p in range(2):
                sl = slice(p * 64, (p + 1) * 64)
                w_ = r2[:, sl]
                wr = r2[:, sl][:, ::-1]
                nc.vector.tensor_tensor_scan(out=pm[:, sl], data0=w_, data1=w_,
                                             initial=-3e38, op0=OP.max, op1=OP.bypass)
                nc.vector.tensor_tensor_scan(out=pn[:, sl], data0=w_, data1=w_,
                                             initial=3e38, op0=OP.min, op1=OP.bypass)
                nc.vector.tensor_tensor_scan(out=sm[:, sl][:, ::-1], data0=wr,
                                             data1=wr, initial=-3e38,
                                             op0=OP.max, op1=OP.bypass)
                nc.vector.tensor_tensor_scan(out=sn[:, sl][:, ::-1], data0=wr,
                                             data1=wr, initial=3e38,
                                             op0=OP.min, op1=OP.bypass)
            for p in range(2):
                sl = slice(p * 64, (p + 1) * 64)
                nc.vector.tensor_tensor(out=M[:, sl], in0=cu[:, sl],
                                        in1=pm[:, sl], op=OP.mult)
                nc.vector.tensor_tensor(out=t1[:], in0=cu[:, sl], in1=pn[:, sl],
                                        op=OP.mult)
                nc.vector.tensor_tensor(out=M[:, sl], in0=M[:, sl], in1=t1[:],
                                        op=OP.max)
                j63 = slice(p * 64, p * 64 + 63)
                cs = cu[:, j63]
                nc.vector.tensor_tensor(out=t1[:, 0:63], in0=cs,
                                        in1=sm[:, p * 64 + 1:(p + 1) * 64],
                                        op=OP.mult)
                nc.vector.tensor_tensor(out=t2[:, 0:63], in0=cs,
                                        in1=sn[:, p * 64 + 1:(p + 1) * 64],
                                        op=OP.mult)
                nc.vector.tensor_tensor(out=t1[:, 0:63], in0=t1[:, 0:63],
                                        in1=t2[:, 0:63], op=OP.max)
                nc.vector.tensor_scalar(out=t1[:, 0:63], in0=t1[:, 0:63],
                                        scalar1=NEG, scalar2=None, op0=OP.add)
                nc.vector.tensor_tensor(out=M[:, j63], in0=M[:, j63],
                                        in1=t1[:, 0:63], op=OP.max)

        # E chunks of 16 j: build/mask/-M/exp/Z/scale -> transpose to PT -> PV
        Zrec = small.tile([128, 2 * 64], F32, tag="Zrec", name="Zrec")
        for p in range(2):
            PT = bigP.tile([64, 64 * 128], F32, tag="PT", name="PT")
            PT4 = PT[:].rearrange("k (j pp) -> k j pp", j=64)
            for jc in range(4):
                jsl = slice(p * 64 + jc * 16, p * 64 + (jc + 1) * 16)
                E = work.tile([128, 1024], F32, tag="Echunk", name="Echunk", bufs=2)
                E3 = E[:].rearrange("a (j k) -> a j k", j=16)
                nc.vector.tensor_tensor(
                    out=E3, in0=cu[:, jsl][:, :, None].broadcast_to([128, 16, 64]),
                    in1=r2[:, p * 64:(p + 1) * 64][:, None, :]
                        .broadcast_to([128, 16, 64]), op=OP.mult)
                if causal:
                    CS = work.tile([128, 1024], F32, tag="CSchunk", name="CSchunk",
                                   bufs=2)
                    nc.scalar.dma_start(CS[:], causD[:, jc * 1024:(jc + 1) * 1024])
                    nc.gpsimd.tensor_tensor(out=E[:], in0=E[:], in1=CS[:], op=OP.add)
                nc.vector.tensor_tensor(
                    out=E3, in0=E3,
                    in1=M[:, jsl][:, :, None].broadcast_to([128, 16, 64]),
                    op=OP.subtract)
                nc.scalar.activation(E[:], E[:], ACTF.Exp)
                nc.vector.tensor_reduce(out=Zrec[:, jsl], in_=E3, axis=AX.X,
                                        op=OP.add)
                nc.vector.reciprocal(Zrec[:, jsl], Zrec[:, jsl])
                nc.gpsimd.tensor_tensor(
                    out=E3, in0=E3,
                    in1=Zrec[:, jsl][:, :, None].broadcast_to([128, 16, 64]),
                    op=OP.mult)
                for jb in range(0, 16, 4):
                    ps = psB.tile([64, 512], F32, tag="psb", name="psb")
                    for q in range(4):
                        nc.tensor.transpose(
                            ps[:, q * 128:(q + 1) * 128],
                            E[:, (jb + q) * 64:(jb + q + 1) * 64], I128[:])
                    copy_ps(PT[:, (jc * 16 + jb) * 128:(jc * 16 + jb + 4) * 128],
                            ps[:])

            # PV for this parity: half-banks [64, 512], pairs (h, q=b)
            for b in range(RT):
                vt = work.tile([64, D], F32, tag="Vload", name="Vload")
                nc.scalar.dma_start(vt[:], vD[(2 * b + p) * 64:(2 * b + p + 1) * 64, :])
                bank = psA.tile([64, 512], F32, tag="psa", name="psa")
                for h in range(NH):
                    pr = h * 16 + b
                    nc.tensor.matmul(
                        bank[:, h * 64:(h + 1) * 64],
                        lhsT=PT4[:, :, pr],
                        rhs=vt[:, h * 64:(h + 1) * 64],
                        start=True, stop=True)
                stag = work.tile([64, 512], F32, tag="stag", name="stag")
                copy_ps(stag[:], bank[:])
                for h in range(NH):
                    base = (2 * b + p) * 64 + h * 8
                    nc.sync.dma_start(
                        aD[base:base + 8, :],
                        stag[:, h * 64:(h + 1) * 64])

    # ---------- residual + LN from aD -------------------------------------
    def resid_ln(other_nat_cb, out_cb):
        def pre_fn(rt):
            at = work.tile([128, D], F32, tag="aload", name="aload")
            nc.sync.dma_start(at[:], aD[rt * 128:(rt + 1) * 128, :])
            pt = preQ.tile([128, D], F32, tag="pre", name="pre")
            nc.vector.tensor_tensor(out=pt[:], in0=at[:], in1=other_nat_cb(rt),
                                    op=OP.add)
            return pt[:]
        for g in range(RT // 4):
            ln_group4(g, pre_fn, out_cb)

    def ln_out_to_TD(dst_dram, also_nat_dram=None):
        """LN out_cb that immediately transposes each tile into dst_dram."""
        def cb(rt, src, negmu, rstd):
            ot = work.tile([128, D], F32, tag="lnout", name="lnout", bufs=4)
            nc.vector.tensor_scalar(out=ot[:], in0=src, scalar1=negmu,
                                    scalar2=rstd, op0=OP.add, op1=OP.mult)
            if also_nat_dram is not None:
                nc.sync.dma_start(also_nat_dram[rt * 128:(rt + 1) * 128, :], ot[:])
            ps = psB.tile([128, 512], F32, tag="psb", name="psb")
            for cb_ in range(4):
                nc.tensor.transpose(ps[:, cb_ * 128:(cb_ + 1) * 128],
                                    ot[:, cb_ * 128:(cb_ + 1) * 128], I128[:])
            t = work.tile([128, 512], F32, tag="toD", name="toD", bufs=2)
            copy_ps(t[:], ps[:])
            nc.sync.dma_start(
                dst_dram[:, :, rt * 128:(rt + 1) * 128].rearrange("c a r -> a c r"),
                t[:].rearrange("a (c r) -> a c r", c=4))
        return cb

    # ---------- FFN ---------------------------------------------------------
    def ffn(xTd, resTd, w1_ap, b1_ap, w2_ap, b2_ap, out_cb):
        b2 = small.tile([1, D], F32, tag="b2", name="b2")
        nc.sync.dma_start(b2[:], b2_ap[:])
        for rc in range(4):
            xcs = []
            for dt in range(DT):
                xc = work.tile([128, 512], F32, tag=f"xfc{dt}", name=f"xfc{dt}",
                               bufs=1)
                nc.sync.dma_start(xc[:], xTd[dt, :, rc * 512:(rc + 1) * 512])
                xcs.append(xc)
            ps2 = [psB.tile([128, 512], F32, tag="psb", name="psb")
                   for _ in range(4)]
            for ff in range(FT):
                w1f = work.tile([128, 512], F32, tag="w1f", name="w1f")
                nc.scalar.dma_start(
                    w1f[:].rearrange("a (d c) -> a d c", d=4),
                    w1_ap[:, ff * 128:(ff + 1) * 128]
                        .rearrange("(d a) c -> a d c", d=4))
                b1f = small.tile([1, 128], F32, tag="b1f", name="b1f", bufs=3)
                nc.sync.dma_start(b1f[:], b1_ap[:, ff * 128:(ff + 1) * 128])
                ps1 = psA.tile([128, 512], F32, tag="psa", name="psa")
                for dt in range(DT):
                    nc.tensor.matmul(ps1[:],
                                     lhsT=w1f[:, dt * 128:(dt + 1) * 128],
                                     rhs=xcs[dt][:], start=(dt == 0), stop=False)
                nc.tensor.matmul(ps1[:], lhsT=b1f[:], rhs=ones1[:, 0:512],
                                 start=False, stop=True)
                f1f = work.tile([128, 512], F32, tag="f1f", name="f1f")
                nc.scalar.activation(f1f[:], ps1[:], ACTF.Relu)
                w2f = work.tile([128, 512], F32, tag="w2f", name="w2f")
                nc.sync.dma_start(w2f[:], w2_ap[ff * 128:(ff + 1) * 128, :])
                for rl in range(4):
                    nc.tensor.matmul(ps2[rl][:],
                                     lhsT=f1f[:, rl * 128:(rl + 1) * 128],
                                     rhs=w2f[:], start=(ff == 0), stop=False)
            def pre_fn(rt):
                rl = rt % 4
                nc.tensor.matmul(ps2[rl][:], lhsT=ones1[:, 0:128], rhs=b2[:],
                                 start=False, stop=False)
                for ct in range(DT):
                    rtl = work.tile([128, 128], F32, tag="rload", name="rload",
                                    bufs=4)
                    nc.scalar.dma_start(rtl[:], resTd[ct, :, rt * 128:(rt + 1) * 128])
                    nc.tensor.matmul(ps2[rl][:, ct * 128:(ct + 1) * 128],
                                     lhsT=rtl[:], rhs=I128[:], start=False,
                                     stop=(ct == DT - 1))
                pt = preQ.tile([128, D], F32, tag="pre", name="pre")
                copy_ps(pt[:], ps2[rl][:])
                return pt[:]
            ln_group4(rc, pre_fn, out_cb)

    # ======================= pipeline =======================
    # P1: dec1 (causal) on x_de
    embed_T_toD(hi['XdT'], xTd['xd'])
    attention(xTd['xd'], xTd['xd'], gw['dec_wv1'][:], gw['dec_wqk1'][:],
              [hi['dec1_A'][p] for p in range(2)], hi['dec1_scl'],
              hi['dec1_t'], hi['dec1_cs'], True)
    resid_ln(lambda rt: embed_nat_ps(hi['XdT'], rt)[:],
             ln_out_to_TD(xTd['m'], also_nat_dram=mnD))

    # P2: encoder self-attn on x_en
    embed_T_toD(hi['XeT'], xTd['xe'])
    attention(xTd['xe'], xTd['xe'], gw['enc_wv'][:], gw['enc_wqk'][:],
              [hi['enc_A'][p] for p in range(2)], hi['enc_scl'],
              hi['enc_t'], hi['enc_cs'], False)
    resid_ln(lambda rt: embed_nat_ps(hi['XeT'], rt)[:], ln_out_to_TD(xTd['o1']))

    # P3: encoder FFN
    ffn(xTd['o1'], xTd['o1'], gw['enc_w1'][:], hi['enc_b1'], gw['enc_w2'][:],
        hi['enc_b2'], ln_out_to_TD(xTd['eo']))

    # P4: dec2 cross-attn
    attention(xTd['m'], xTd['eo'], gw['dec_wv2'][:], gw['dec_wqk2'][:],
              [hi['dec2_A'][p] for p in range(2)], hi['dec2_scl'],
              hi['dec2_t'], hi['dec2_cs'], False)

    def m_reload(rt):
        t = work.tile([128, D], F32, tag="mload", name="mload", bufs=2)
        nc.sync.dma_start(t[:], mnD[rt * 128:(rt + 1) * 128, :])
        return t[:]
    resid_ln(m_reload, ln_out_to_TD(xTd['c']))

    # P5: decoder FFN
    ffn(xTd['c'], xTd['c'], gw['dec_w1'][:], hi['dec_b1'], gw['dec_w2'][:],
        hi['dec_b2'], ln_out_to_TD(xTd['of']))

    # P6: final projection + softmax (output ships as bf16)
    Wo = wpool.tile([128, 4 * 64], F32, tag="Wo", name="Wo")
    for dt in range(DT):
        nc.sync.dma_start(Wo[:, dt * 64:(dt + 1) * 64],
                          gw['W_out'][dt * 128:(dt + 1) * 128, :])
    Bo = small.tile([1, 64], F32, tag="Bo", name="Bo")
    nc.sync.dma_start(Bo[:], hi['B_out'][:])
    for rt in range(RT):
        ps = psB.tile([128, 64], F32, tag="psbq", name="psbo", bufs=1)
        for dt in range(DT):
            ol = work.tile([128, 128], F32, tag="rload", name="rload", bufs=4)
            nc.sync.dma_start(ol[:], xTd['of'][dt, :, rt * 128:(rt + 1) * 128])
            nc.tensor.matmul(ps[:], lhsT=ol[:], rhs=Wo[:, dt * 64:(dt + 1) * 64],
                             start=(dt == 0), stop=False)
        nc.tensor.matmul(ps[:], lhsT=ones1[:, 0:128], rhs=Bo[:],
                         start=False, stop=True)
        mx = small.tile([128, 1], F32, tag="mx", name="mx")
        nc.vector.tensor_reduce(out=mx[:], in_=ps[:], axis=AX.X, op=OP.max,
                                negate=True)
        ex = work.tile([128, 64], F32, tag="ex", name="ex")
        nc.scalar.activation(ex[:], ps[:], ACTF.Exp, bias=mx[:])
        zs = small.tile([128, 1], F32, tag="zs", name="zs")
        nc.vector.tensor_reduce(out=zs[:], in_=ex[:], axis=AX.X, op=OP.add)
        rz = small.tile([128, 1], F32, tag="rz", name="rz")
        nc.vector.reciprocal(rz[:], zs[:])
        oo = work.tile([128, 64], F32, tag="oo", name="oo")
        nc.vector.tensor_scalar(out=oo[:], in0=ex[:], scalar1=rz[:],
                                scalar2=None, op0=OP.mult)
        oo16 = work.tile([128, 64], F16, tag="oo16", name="oo16")
        nc.vector.tensor_copy(oo16[:], oo[:])
        nc.sync.dma_start(out_ap[rt * 128:(rt + 1) * 128, :], oo16[:])


# ============================================================================
# 8-core SPMD wrapper with a cached PJRT dispatcher: kernel(**inputs) -> out
# ============================================================================
_CACHE = {}


def _get_program():
    if 'nc' not in _CACHE:
        nc = bacc.Bacc("TRN2", target_bir_lowering=False, debug=False)
        hi, out_ap = declare_io(nc)
        with tile.TileContext(nc, trace_sim=False) as tc:
            with ExitStack() as ctx:
                build(ctx, tc, hi, out_ap)
        nc.compile()
        _CACHE['nc'] = nc
    return _CACHE['nc']


def _get_dispatcher():
    """One cached jit(shard_map(...)) wrapper -- same execution path as
    bass_utils.run_bass_kernel_spmd under axon (bass2jax/_bass_exec_p via
    PJRT), but without rebuilding/retracing the wrapper on every call."""
    if 'disp' in _CACHE:
        return _CACHE['disp']
    import jax
    from jax.sharding import Mesh, PartitionSpec
    from jax.experimental.shard_map import shard_map
    from concourse import bass2jax

    nc = _get_program()
    bass2jax.install_neuronx_cc_hook()
    partition_name = (nc.partition_id_tensor.name
                      if nc.partition_id_tensor else None)
    in_names, out_names, out_avals, zero_tmpl = [], [], [], []
    for alloc in nc.m.functions[0].allocations:
        if not isinstance(alloc, mybir.MemoryLocationSet):
            continue
        name = alloc.memorylocations[0].name
        if alloc.kind == "ExternalInput":
            if name != partition_name:
                in_names.append(name)
        elif alloc.kind == "ExternalOutput":
            shape = tuple(alloc.tensor_shape)
            dtype = mybir.dt.np(alloc.dtype)
            out_avals.append(jax.core.ShapedArray(shape, dtype))
            zero_tmpl.append((shape, dtype))
            out_names.append(name)
    n_params = len(in_names)
    n_outs = len(out_avals)
    all_in_names = list(in_names) + list(out_names)
    if partition_name is not None:
        all_in_names.append(partition_name)
    donate = tuple(range(n_params, n_params + n_outs))

    def _body(*args):
        operands = list(args)
        if partition_name is not None:
            operands.append(bass2jax.partition_id_tensor())
        outs = bass2jax._bass_exec_p.bind(
            *operands, out_avals=tuple(out_avals),
            in_names=tuple(all_in_names), out_names=tuple(out_names),
            lowering_input_output_aliases=(), sim_require_finite=True,
            sim_require_nnan=True, nc=nc)
        return tuple(outs)

    devices = jax.devices()[:8]
    mesh = Mesh(np.asarray(devices), ("core",))
    sharded = jax.jit(
        shard_map(_body, mesh=mesh,
                  in_specs=(PartitionSpec("core"),) * (n_params + n_outs),
                  out_specs=(PartitionSpec("core"),) * n_outs,
                  check_rep=False),
        donate_argnums=donate, keep_unused=True)

    # donated output buffers are allocated+zeroed ON DEVICE (no tunnel bytes)
    import jax.numpy as jnp
    from jax.sharding import NamedSharding
    zsh = NamedSharding(mesh, PartitionSpec("core"))
    zfn = jax.jit(
        lambda: tuple(jnp.zeros((8 * s[0], *s[1:]), d) for (s, d) in zero_tmpl),
        out_shardings=(zsh,) * n_outs)

    def dispatch(in_maps):
        concat_in = [
            np.concatenate([np.asarray(in_maps[c][nm]) for c in range(8)], 0)
            for nm in in_names]
        cz = zfn()
        outs = sharded(*concat_in, *cz)
        return [
            {nm: np.asarray(outs[i]).reshape(8, *out_avals[i].shape)[c]
             for i, nm in enumerate(out_names)}
            for c in range(8)]

    _CACHE['disp'] = dispatch
    return dispatch


def kernel(**inputs):
    dispatch = _get_dispatcher()
    in_maps = [host_inputs(inputs, core) for core in range(8)]
    res = dispatch(in_maps)
    outs = [np.asarray(res[c]['out'], np.float32) for c in range(8)]
    full = np.concatenate(outs, 0)          # [16384, 64] rows = (b, L)
    return full.reshape(64, 256, 64)
